# revision 1
# baseline (speedup 1.0000x reference)
"""Trainium2 Bass kernel for nn_Net_74259984548321 (video-caption LSTM net).

Strategy: data-parallel over batch (8 rows/core, 8 cores).
  P1 : encoder input projection feat @ e1_Wih.T (+bias) streamed k-outer,
       cap_proj = caption @ d2_Wih[:, :H].T (+d2_b); both bounce through
       DRAM scratch so per-step [8, 4H] slices come back partition-aligned.
  P2 : 80-step 2-layer encoder LSTM recurrence (batch in partitions)
  P3 : 31-step decoder recurrence (d1 cell, d2 cell, dot-attention)
  P4 : chunked output projection + online logsumexp + one-hot max-mask
       target gather, emitted twice (rows t=0..15, t=16..30), partial CE
       reduced on-device to a scalar; host sums the 8 per-core scalars.
"""

import numpy as np

B, T, FEAT, H, V, L = 64, 80, 4096, 256, 8000, 32
DEC = L - 1            # 31 decoder steps
NCORES = 8
BS = B // NCORES       # 8 batch rows per core
G = 4 * H              # 1024 gates
NCH = 16               # logit chunks
CSZ = V // NCH         # 500
ROWS = DEC * BS        # 248 (t, b) rows per core
KF = FEAT // 128       # 32 k-chunks of the feature dim

_cache = {}


def _build_program():
    import concourse.tile as tile
    from concourse import bacc, mybir
    from concourse.bass import ts, ds
    from concourse.masks import make_identity

    fp = mybir.dt.float32
    AF = mybir.ActivationFunctionType
    ALU = mybir.AluOpType
    AX = mybir.AxisListType

    nc = bacc.Bacc(None, target_bir_lowering=False)

    featT_d = nc.dram_tensor("featT", [KF, 128, T * BS], fp, kind="ExternalInput")
    capT_d = nc.dram_tensor("capT", [128, 2, ROWS], fp, kind="ExternalInput")
    oh_d = nc.dram_tensor("ohrows", [ROWS, V], fp, kind="ExternalInput")
    w1T_d = nc.dram_tensor("w1T", [KF, 128, G], fp, kind="ExternalInput")
    w1hhT_d = nc.dram_tensor("w1hhT", [128, 2, G], fp, kind="ExternalInput")
    w2T_d = nc.dram_tensor("w2T", [128, 4, G], fp, kind="ExternalInput")
    wd1T_d = nc.dram_tensor("wd1T", [128, 2, G], fp, kind="ExternalInput")
    wd2lT_d = nc.dram_tensor("wd2lT", [128, 2, G], fp, kind="ExternalInput")
    wd2T_d = nc.dram_tensor("wd2T", [128, 4, G], fp, kind="ExternalInput")
    woT_d = nc.dram_tensor("woT", [128, 2, V], fp, kind="ExternalInput")
    b1_d = nc.dram_tensor("b1row", [1, G], fp, kind="ExternalInput")
    b2_d = nc.dram_tensor("b2row", [1, G], fp, kind="ExternalInput")
    bd1_d = nc.dram_tensor("bd1row", [1, G], fp, kind="ExternalInput")
    bd2_d = nc.dram_tensor("bd2row", [1, G], fp, kind="ExternalInput")
    bo_d = nc.dram_tensor("borow", [1, V], fp, kind="ExternalInput")
    g1scr = nc.dram_tensor("g1scr", [5, 128, G], fp)
    capscr = nc.dram_tensor("capscr", [2, 128, G], fp)
    out_d = nc.dram_tensor("partial", [1, 1], fp, kind="ExternalOutput")

    with tile.TileContext(nc) as tc:
        from contextlib import ExitStack

        with ExitStack() as ctx:
            const = ctx.enter_context(tc.tile_pool(name="const", bufs=1))
            wpool = ctx.enter_context(tc.tile_pool(name="w", bufs=1))
            state = ctx.enter_context(tc.tile_pool(name="state", bufs=1))
            acts = ctx.enter_context(tc.tile_pool(name="acts", bufs=2))
            p3sb = ctx.enter_context(tc.tile_pool(name="p3sb", bufs=2))
            smsb = ctx.enter_context(tc.tile_pool(name="smsb", bufs=4))

            # ---- constants / identities / biases ----
            ident8 = const.tile([BS, BS], fp, tag="id8")
            make_identity(nc, ident8)
            ident128 = const.tile([128, 128], fp, tag="id128")
            make_identity(nc, ident128)
            ones1x128 = const.tile([1, 128], fp, tag="onesr")
            nc.vector.memset(ones1x128, 1.0)
            ones128 = const.tile([128, 1], fp, tag="onesc")
            nc.vector.memset(ones128, 1.0)
            b2row = const.tile([1, G], fp, tag="b2")
            nc.sync.dma_start(b2row, b2_d[:, :])
            bd1row = const.tile([1, G], fp, tag="bd1")
            nc.sync.dma_start(bd1row, bd1_d[:, :])
            bd2row = const.tile([1, G], fp, tag="bd2")
            nc.sync.dma_start(bd2row, bd2_d[:, :])

            # ---- persistent weights ----
            w1hh = wpool.tile([128, 2, G], fp, tag="w1hh")
            nc.sync.dma_start(w1hh, w1hhT_d[:, :, :])
            w2 = wpool.tile([128, 4, G], fp, tag="w2")
            nc.sync.dma_start(w2, w2T_d[:, :, :])
            wd1 = wpool.tile([128, 2, G], fp, tag="wd1")
            nc.sync.dma_start(wd1, wd1T_d[:, :, :])
            wd2 = wpool.tile([128, 4, G], fp, tag="wd2")
            nc.sync.dma_start(wd2, wd2T_d[:, :, :])

            # ---- persistent activations/state ----
            h2seqT = state.tile([128, 2, T, BS], fp, tag="h2seq")
            h2decT = state.tile([128, 2, DEC, BS], fp, tag="h2dec")
            A_sb = state.tile([T, BS, H], fp, tag="Asb")
            h1T = state.tile([128, 2, BS], fp, tag="h1T")
            h2aT = state.tile([128, 2, BS], fp, tag="h2aT")
            c1 = state.tile([BS, H], fp, tag="c1")
            nc.vector.memset(c1, 0.0)
            c2 = state.tile([BS, H], fp, tag="c2")
            nc.vector.memset(c2, 0.0)
            ce_parts = state.tile([1, 2], fp, tag="cep")

            # ================= P1: projections (scoped pools) =================
            with ExitStack() as p1ctx:
                w1s = p1ctx.enter_context(tc.tile_pool(name="w1s", bufs=3))
                fts = p1ctx.enter_context(tc.tile_pool(name="fts", bufs=3))
                stg = p1ctx.enter_context(tc.tile_pool(name="stg", bufs=3))
                p1w = p1ctx.enter_context(tc.tile_pool(name="p1w", bufs=1))
                p1ps = p1ctx.enter_context(
                    tc.tile_pool(name="p1ps", bufs=5, space="PSUM")
                )
                b1row = p1w.tile([1, G], fp, tag="b1")
                nc.sync.dma_start(b1row, b1_d[:, :])

                # cap_proj = caption @ d2_Wih_l.T + d2_b -> capscr[t//16, 8*(t%16)+b, :]
                capT = p1w.tile([128, 2, ROWS], fp, tag="capT")
                nc.sync.dma_start(capT, capT_d[:, :, :])
                wd2l = p1w.tile([128, 2, G], fp, tag="wd2l")
                nc.sync.dma_start(wd2l, wd2lT_d[:, :, :])
                for mi in range(2):
                    R = 128 if mi == 0 else ROWS - 128
                    for nj in range(2):
                        ps = p1ps.tile([128, 512], fp, tag="p1")
                        nc.tensor.matmul(
                            ps[:R], ones1x128[:, :R], bd2row[:, ts(nj, 512)],
                            start=True, stop=False,
                        )
                        for kc in range(2):
                            nc.tensor.matmul(
                                ps[:R],
                                capT[:, kc, ds(mi * 128, R)],
                                wd2l[:, kc, ts(nj, 512)],
                                start=False, stop=(kc == 1),
                            )
                        st = stg.tile([128, 512], fp, tag="stg")
                        nc.vector.tensor_copy(st[:R], ps[:R])
                        nc.sync.dma_start(capscr[mi, ds(0, R), ts(nj, 512)], st[:R])

                # G1 = feat @ e1_Wih.T + e1_b -> g1scr[t//16, 8*(t%16)+b, :]
                for nj in range(2):
                    pss = [
                        p1ps.tile([128, 512], fp, tag="p1", name=f"p1acc{nj}_{m}")
                        for m in range(5)
                    ]
                    for m in range(5):
                        nc.tensor.matmul(
                            pss[m], ones1x128, b1row[:, ts(nj, 512)],
                            start=True, stop=False,
                        )
                    for k in range(KF):
                        w1t = w1s.tile([128, 512], fp, tag="w1")
                        nc.sync.dma_start(w1t, w1T_d[k, :, ts(nj, 512)])
                        ft = fts.tile([128, T * BS], fp, tag="ft")
                        nc.sync.dma_start(ft, featT_d[k, :, :])
                        for m in range(5):
                            nc.tensor.matmul(
                                pss[m], ft[:, ts(m, 128)], w1t,
                                start=False, stop=(k == KF - 1),
                            )
                    for m in range(5):
                        st = stg.tile([128, 512], fp, tag="stg")
                        nc.vector.tensor_copy(st, pss[m])
                        nc.sync.dma_start(g1scr[m, :, ts(nj, 512)], st)

            # ============ compute-phase PSUM pools (after p1ps freed) ============
            gps = ctx.enter_context(tc.tile_pool(name="gps", bufs=4, space="PSUM"))
            trp = ctx.enter_context(tc.tile_pool(name="trp", bufs=2, space="PSUM"))
            smp = ctx.enter_context(tc.tile_pool(name="smp", bufs=2, space="PSUM"))
            g1s = ctx.enter_context(tc.tile_pool(name="g1s", bufs=3))

            def lstm_elem(gA, gB, c_st):
                """gA = gates[0:512] (i, f), gB = gates[512:1024] (g, o)."""
                sif = acts.tile([BS, 2 * H], fp, tag="sif")
                tg = acts.tile([BS, H], fp, tag="tg")
                so = acts.tile([BS, H], fp, tag="so")
                th = acts.tile([BS, H], fp, tag="th")
                t1 = acts.tile([BS, H], fp, tag="t1")
                h = acts.tile([BS, H], fp, tag="h")
                nc.scalar.activation(sif, gA, AF.Sigmoid)
                nc.scalar.activation(tg, gB[:, 0:H], AF.Tanh)
                nc.scalar.activation(so, gB[:, H : 2 * H], AF.Sigmoid)
                nc.vector.tensor_mul(t1, sif[:, 0:H], tg)
                nc.vector.tensor_mul(c_st, sif[:, H : 2 * H], c_st)
                nc.vector.tensor_add(c_st, c_st, t1)
                nc.scalar.activation(th, c_st, AF.Tanh)
                nc.vector.tensor_mul(h, so, th)
                return h

            def tr_to(dst, h):
                """Transpose h [8, 256] into dst [128, 2, 8] (psum bounce)."""
                pt = trp.tile([128, 2, BS], fp, tag="tr")
                nc.tensor.transpose(pt[:, 0, :], h[:, 0:128], ident8)
                nc.tensor.transpose(pt[:, 1, :], h[:, 128:256], ident8)
                nc.vector.tensor_copy(dst, pt)

            # ================= P2: encoder =================
            for t in range(T):
                g1t = g1s.tile([BS, G], fp, tag="g1t")
                nc.sync.dma_start(g1t, g1scr[t // 16, ds(BS * (t % 16), BS), :])
                if t == 0:
                    gA, gB = g1t[:, 0:512], g1t[:, 512:G]
                else:
                    psA = gps.tile([BS, 512], fp, tag="g")
                    psB = gps.tile([BS, 512], fp, tag="g")
                    for nj, p in ((0, psA), (1, psB)):
                        for kc in range(2):
                            nc.tensor.matmul(
                                p, h1T[:, kc, :], w1hh[:, kc, ts(nj, 512)],
                                start=(kc == 0), stop=(kc == 1),
                            )
                    gA = acts.tile([BS, 512], fp, tag="gA")
                    gB = acts.tile([BS, 512], fp, tag="gB")
                    nc.vector.tensor_add(gA, psA, g1t[:, 0:512])
                    nc.vector.tensor_add(gB, psB, g1t[:, 512:G])
                h1n = lstm_elem(gA, gB, c1)
                tr_to(h1T, h1n)
                # cell 2: gates = e2_b + h1' @ e2_Wih_r.T + h2 @ e2_Whh.T
                psA2 = gps.tile([BS, 512], fp, tag="g")
                psB2 = gps.tile([BS, 512], fp, tag="g")
                nkc = 2 if t == 0 else 4
                for nj, p in ((0, psA2), (1, psB2)):
                    nc.tensor.matmul(
                        p, ones1x128[:, :BS], b2row[:, ts(nj, 512)],
                        start=True, stop=False,
                    )
                    for kc in range(nkc):
                        lhs = h1T[:, kc, :] if kc < 2 else h2seqT[:, kc - 2, t - 1, :]
                        nc.tensor.matmul(
                            p, lhs, w2[:, kc, ts(nj, 512)],
                            start=False, stop=(kc == nkc - 1),
                        )
                h2n = lstm_elem(psA2, psB2, c2)
                tr_to(h2seqT[:, :, t, :], h2n)

            # A_sb[t, b, :] = h2seq[b, t, :]  (untransposed copy for attention)
            for b in range(BS):
                for kc in range(2):
                    pA = smp.tile([T, 128], fp, tag="sm")
                    nc.tensor.transpose(pA, h2seqT[:, kc, :, b], ident128)
                    nc.vector.tensor_copy(A_sb[:, b, ts(kc, 128)], pA)

            # P4 emitter: logits + lse + target gather for one row-tile
            def emit_p4(mi, ohs, wos, junk, bos):
                R = 128 if mi == 0 else ROWS - 128
                tn = 16 if mi == 0 else DEC - 16
                nm_all = p3sb.tile([128, NCH], fp, tag="nm_all")
                s_all = p3sb.tile([128, NCH], fp, tag="s_all")
                tv_all = p3sb.tile([128, NCH], fp, tag="tv_all")
                mo_all = p3sb.tile([128, NCH], fp, tag="mo_all")
                for c in range(NCH):
                    oht = ohs.tile([128, CSZ], fp, tag="oh")
                    nc.sync.dma_start(oht[:R], oh_d[ds(128 * mi, R), ts(c, CSZ)])
                    nc.vector.reduce_max(mo_all[:R, c : c + 1], oht[:R], axis=AX.X)
                Moh = p3sb.tile([128, 1], fp, tag="Moh")
                nc.vector.reduce_max(Moh[:R], mo_all[:R], axis=AX.X)
                for c in range(NCH):
                    wot = wos.tile([128, 2, CSZ], fp, tag="wo")
                    nc.sync.dma_start(wot, woT_d[:, :, ts(c, CSZ)])
                    oht = ohs.tile([128, CSZ], fp, tag="oh")
                    nc.sync.dma_start(oht[:R], oh_d[ds(128 * mi, R), ts(c, CSZ)])
                    bot = bos.tile([1, CSZ], fp, tag="bot")
                    nc.sync.dma_start(bot, bo_d[:, ts(c, CSZ)])
                    psL = gps.tile([128, CSZ], fp, tag="g")
                    nc.tensor.matmul(
                        psL[:R], ones1x128[:, :R], bot,
                        start=True, stop=False,
                    )
                    for kc in range(2):
                        nc.tensor.matmul(
                            psL[:R],
                            h2decT[:, kc, ds(16 * mi, tn), :],
                            wot[:, kc, :],
                            start=False, stop=(kc == 1),
                        )
                    nc.vector.reduce_max(
                        nm_all[:R, c : c + 1], psL[:R], axis=AX.X, negate=True
                    )
                    ej = junk.tile([128, CSZ], fp, tag="jk")
                    nc.scalar.activation(
                        ej[:R], psL[:R], AF.Exp,
                        bias=nm_all[:R, c : c + 1],
                        accum_out=s_all[:R, c : c + 1],
                    )
                    tj = junk.tile([128, CSZ], fp, tag="jk")
                    nc.vector.scalar_tensor_tensor(
                        tj[:R], oht[:R], Moh[:R], psL[:R],
                        op0=ALU.is_equal, op1=ALU.mult,
                        accum_out=tv_all[:R, c : c + 1],
                    )
                # combine chunks: lse = log(sum_c s_c * exp(m_c - M)) + M
                m_all = p3sb.tile([128, NCH], fp, tag="m_all")
                nc.vector.tensor_scalar_mul(m_all[:R], nm_all[:R], -1.0)
                negM = p3sb.tile([128, 1], fp, tag="negM")
                nc.vector.reduce_max(negM[:R], m_all[:R], axis=AX.X, negate=True)
                dmt = p3sb.tile([128, NCH], fp, tag="dmt")
                nc.scalar.activation(dmt[:R], m_all[:R], AF.Exp, bias=negM[:R])
                prod = p3sb.tile([128, NCH], fp, tag="prod")
                nc.vector.tensor_mul(prod[:R], s_all[:R], dmt[:R])
                S = p3sb.tile([128, 1], fp, tag="S")
                nc.vector.reduce_sum(S[:R], prod[:R], axis=AX.X)
                lse = p3sb.tile([128, 1], fp, tag="lse")
                nc.scalar.activation(lse[:R], S[:R], AF.Ln)
                ce = p3sb.tile([128, 1], fp, tag="ce")
                nc.vector.tensor_sub(ce[:R], lse[:R], negM[:R])
                tv = p3sb.tile([128, 1], fp, tag="tv")
                nc.vector.reduce_sum(tv[:R], tv_all[:R], axis=AX.X)
                nc.vector.tensor_sub(ce[:R], ce[:R], tv[:R])
                lps = smp.tile([1, 1], fp, tag="sm")
                nc.tensor.matmul(lps, ce[:R], ones128[:R], start=True, stop=True)
                nc.vector.tensor_copy(ce_parts[:, mi : mi + 1], lps)

            # ================= P3: decoder =================
            ohs = ctx.enter_context(tc.tile_pool(name="ohs", bufs=4))
            wos = ctx.enter_context(tc.tile_pool(name="wos", bufs=3))
            junk = ctx.enter_context(tc.tile_pool(name="junk", bufs=2))
            bos = ctx.enter_context(tc.tile_pool(name="bos", bufs=2))

            for t in range(DEC):
                # d1: gates = d1_b + h1 @ d1_Whh.T
                psA = gps.tile([BS, 512], fp, tag="g")
                psB = gps.tile([BS, 512], fp, tag="g")
                for nj, p in ((0, psA), (1, psB)):
                    nc.tensor.matmul(
                        p, ones1x128[:, :BS], bd1row[:, ts(nj, 512)],
                        start=True, stop=False,
                    )
                    for kc in range(2):
                        nc.tensor.matmul(
                            p, h1T[:, kc, :], wd1[:, kc, ts(nj, 512)],
                            start=False, stop=(kc == 1),
                        )
                h1n = lstm_elem(psA, psB, c1)
                tr_to(h1T, h1n)
                # d2: gates = cap_proj[t] + h1' @ d2_Wih_r.T + h2 @ d2_Whh.T
                capt = g1s.tile([BS, G], fp, tag="capt")
                nc.sync.dma_start(
                    capt, capscr[t // 16, ds(BS * (t % 16), BS), :]
                )
                psA2 = gps.tile([BS, 512], fp, tag="g")
                psB2 = gps.tile([BS, 512], fp, tag="g")
                for nj, p in ((0, psA2), (1, psB2)):
                    for kc in range(4):
                        if kc < 2:
                            lhs = h1T[:, kc, :]
                        elif t == 0:
                            lhs = h2seqT[:, kc - 2, T - 1, :]
                        else:
                            lhs = h2aT[:, kc - 2, :]
                        nc.tensor.matmul(
                            p, lhs, wd2[:, kc, ts(nj, 512)],
                            start=(kc == 0), stop=(kc == 3),
                        )
                gA2 = acts.tile([BS, 512], fp, tag="gA")
                gB2 = acts.tile([BS, 512], fp, tag="gB")
                nc.vector.tensor_add(gA2, psA2, capt[:, 0:512])
                nc.vector.tensor_add(gB2, psB2, capt[:, 512:G])
                h2l = lstm_elem(gA2, gB2, c2)
                tr_to(h2decT[:, :, t, :], h2l)
                # attention: scoresT -> softmax (row layout) -> weighted sum
                stps = smp.tile([T, BS], fp, tag="sm")
                for b in range(BS):
                    for kc in range(2):
                        nc.tensor.matmul(
                            stps[:, b : b + 1],
                            h2seqT[:, kc, :, b],
                            h2decT[:, kc, t, b : b + 1],
                            start=(kc == 0), stop=(kc == 1),
                        )
                sT_sb = acts.tile([T, BS], fp, tag="sT")
                nc.vector.tensor_copy(sT_sb, stps)
                scps = smp.tile([BS, T], fp, tag="sm")
                nc.tensor.transpose(scps, sT_sb, ident128[0:T, 0:T])
                negmax = smsb.tile([BS, 1], fp, tag="nmx")
                nc.vector.reduce_max(negmax, scps, axis=AX.X, negate=True)
                e_sb = acts.tile([BS, T], fp, tag="esb")
                sume = smsb.tile([BS, 1], fp, tag="sume")
                nc.scalar.activation(e_sb, scps, AF.Exp, bias=negmax, accum_out=sume)
                recip = smsb.tile([BS, 1], fp, tag="rcp")
                nc.vector.reciprocal(recip, sume)
                attn = acts.tile([BS, T], fp, tag="attn")
                nc.vector.tensor_scalar_mul(attn, e_sb, recip)
                atps = smp.tile([T, BS], fp, tag="sm")
                nc.tensor.transpose(atps, attn, ident8)
                attnT = acts.tile([T, BS], fp, tag="attnT")
                nc.vector.tensor_copy(attnT, atps)
                ctps = trp.tile([128, 2, BS], fp, tag="tr")
                for b in range(BS):
                    for hc in range(2):
                        nc.tensor.matmul(
                            ctps[:, hc, b : b + 1],
                            A_sb[:, b, ts(hc, 128)],
                            attnT[:, b : b + 1],
                            start=True, stop=True,
                        )
                nc.vector.tensor_copy(h2aT, ctps)
                # epilogue tiles once their rows are complete
                if t == 15:
                    emit_p4(0, ohs, wos, junk, bos)
                elif t == DEC - 1:
                    emit_p4(1, ohs, wos, junk, bos)

            # final: loss = (ce0 + ce1) / B^2
            tot = smsb.tile([1, 1], fp, tag="tot")
            nc.vector.reduce_sum(tot, ce_parts, axis=AX.X)
            outsb = smsb.tile([1, 1], fp, tag="osb")
            nc.scalar.mul(outsb, tot, 1.0 / (B * B))
            nc.sync.dma_start(out_d[:, :], outsb)

    nc.compile()
    return nc


def _shard_inputs(inputs):
    """Host-side relayout + shard. Returns list of 8 in_maps."""
    f32 = np.float32
    feat = np.asarray(inputs["feat"], f32)
    caption = np.asarray(inputs["caption"], f32)
    oh = np.asarray(inputs["caption_one_hot"], f32)

    def w(name):
        return np.asarray(inputs[name], f32)

    w1T = np.ascontiguousarray(w("e1_Wih").T.reshape(KF, 128, G))
    w1hhT = np.ascontiguousarray(w("e1_Whh").T.reshape(2, 128, G).transpose(1, 0, 2))
    w2T = np.ascontiguousarray(
        np.concatenate([w("e2_Wih")[:, H:], w("e2_Whh")], axis=1)
        .T.reshape(4, 128, G).transpose(1, 0, 2)
    )
    wd1T = np.ascontiguousarray(w("d1_Whh").T.reshape(2, 128, G).transpose(1, 0, 2))
    wd2lT = np.ascontiguousarray(
        w("d2_Wih")[:, :H].T.reshape(2, 128, G).transpose(1, 0, 2)
    )
    wd2T = np.ascontiguousarray(
        np.concatenate([w("d2_Wih")[:, H:], w("d2_Whh")], axis=1)
        .T.reshape(4, 128, G).transpose(1, 0, 2)
    )
    woT = np.ascontiguousarray(w("out_W").T.reshape(2, 128, V).transpose(1, 0, 2))

    shared = dict(
        w1T=w1T, w1hhT=w1hhT, w2T=w2T, wd1T=wd1T, wd2lT=wd2lT, wd2T=wd2T, woT=woT,
        b1row=w("e1_b").reshape(1, G), b2row=w("e2_b").reshape(1, G),
        bd1row=w("d1_b").reshape(1, G), bd2row=w("d2_b").reshape(1, G),
        borow=w("out_b").reshape(1, V),
    )
    shared = {k: np.ascontiguousarray(v) for k, v in shared.items()}

    in_maps = []
    for c in range(NCORES):
        b0 = c * BS
        featT = np.ascontiguousarray(
            feat[b0 : b0 + BS].transpose(2, 1, 0).reshape(KF, 128, T * BS)
        )
        capT = np.ascontiguousarray(
            caption[b0 : b0 + BS, : DEC]
            .transpose(2, 1, 0).reshape(2, 128, ROWS).transpose(1, 0, 2)
        )
        ohrows = np.ascontiguousarray(
            oh[b0 : b0 + BS, 1:].transpose(1, 0, 2).reshape(ROWS, V)
        )
        m = dict(shared)
        m.update(featT=featT, capT=capT, ohrows=ohrows)
        in_maps.append(m)
    return in_maps


def kernel(**inputs):
    from concourse.bass_utils import run_bass_kernel_spmd

    if "nc" not in _cache:
        _cache["nc"] = _build_program()
    nc = _cache["nc"]
    in_maps = _shard_inputs(inputs)
    res = run_bass_kernel_spmd(nc, in_maps, core_ids=list(range(NCORES)))
    total = np.float32(0.0)
    for r in res.results:
        total += np.float32(r["partial"][0, 0])
    return np.asarray(total, np.float32)



# revision 4
# speedup vs baseline: 3.1103x; 3.1103x over previous
"""Trainium2 Bass kernel for nn_Net_74259984548321 (video-caption LSTM net).

v2 design (vs v1: all-fp32, row-layout gates, device-side one-hot argmax):
  * all matmuls bf16 (fp32 matmuls cost 4 cycles/row on trn2).
  * recurrence in transposed "gatesT" layout: weight tiles [K=128, M=128]
    stationary, gates/h/c live as [128 gate/h dims, batch] so elementwise
    runs on 128 partitions with tiny free dims (8-64 elems) instead of 8
    partitions x 256-512 elems; h is born in the layout the next matmul
    needs (no per-step PE transposes).
  * biases folded into precomputed addends (g1/cap projections) or constant
    broadcast tiles (one DVE add per cell, no K=1 bias matmuls in the loop).
  * attention: scores -> exp (no max subtraction; scores are O(1)) ->
    unnormalized context + reciprocal-broadcast matmul; no transposes.
  * CE: targets argmax'd on HOST, out_W target rows gathered on HOST;
    device computes full-vocab LSE (streamed out_W, online accum) plus a
    per-row dot with the gathered rows. caption_one_hot never touches HBM.
  * feat projection G1 (the only big GEMM) runs one 16-step slice ahead of
    the encoder, interleaved into the encoder's PE idle gaps.
Per core: 8 batch rows; host sums 8 partial scalars + target-bias term.
"""

import numpy as np

B, T, FEAT, H, V, L = 64, 80, 4096, 256, 8000, 32
DEC = L - 1            # 31 decoder steps
NCORES = 8
BS = B // NCORES       # 8 batch rows per core
G = 4 * H              # 1024 gates
KF = FEAT // 128       # 32 feat contraction chunks
ROWS = DEC * BS        # 248 decoder (t, b) rows per core
NCH = 16               # vocab chunks for LSE
CSZ = V // NCH         # 500
TB = T * BS            # 640 encoder (t, b) rows per core
# gate chunk order used on-chip: i0 i1 f0 f1 o0 o1 g0 g1 (source chunks)
PERM = np.array([0, 1, 2, 3, 6, 7, 4, 5])

_cache = {}


def _build_program():
    import concourse.tile as tile
    from concourse import bacc, mybir
    from concourse.bass import ts, ds
    from concourse.masks import make_identity

    fp = mybir.dt.float32
    bf = mybir.dt.bfloat16
    AF = mybir.ActivationFunctionType
    AX = mybir.AxisListType

    nc = bacc.Bacc(None, target_bir_lowering=False)

    featT_d = nc.dram_tensor("featT", [128, KF, TB], bf, kind="ExternalInput")
    w1_d = nc.dram_tensor("w1t", [128, 8, KF, 128], bf, kind="ExternalInput")
    w1hh_d = nc.dram_tensor("w1hht", [128, 2, 8, 128], bf, kind="ExternalInput")
    w2_d = nc.dram_tensor("w2t", [128, 4, 8, 128], bf, kind="ExternalInput")
    wd1_d = nc.dram_tensor("wd1t", [128, 2, 8, 128], bf, kind="ExternalInput")
    wd2_d = nc.dram_tensor("wd2t", [128, 4, 8, 128], bf, kind="ExternalInput")
    wd2l_d = nc.dram_tensor("wd2lt", [128, 2, 8, 128], bf, kind="ExternalInput")
    b1col_d = nc.dram_tensor("b1col", [128, 8], fp, kind="ExternalInput")
    b2bc_d = nc.dram_tensor("b2bc", [128, 8, BS], fp, kind="ExternalInput")
    bd1bc_d = nc.dram_tensor("bd1bc", [128, 8, BS], fp, kind="ExternalInput")
    bd2col_d = nc.dram_tensor("bd2col", [128, 8], fp, kind="ExternalInput")
    capT_d = nc.dram_tensor("capT", [128, 2, ROWS], bf, kind="ExternalInput")
    wo_d = nc.dram_tensor("wot", [128, 2, V], bf, kind="ExternalInput")
    bo_d = nc.dram_tensor("bot", [1, V], bf, kind="ExternalInput")
    wtgt_d = nc.dram_tensor("wtgt", [128, 2, DEC, BS], bf, kind="ExternalInput")
    out_d = nc.dram_tensor("partial", [1, 1], fp, kind="ExternalOutput")

    with tile.TileContext(nc) as tc:
        from contextlib import ExitStack

        with ExitStack() as ctx:
            const = ctx.enter_context(tc.tile_pool(name="const", bufs=1))
            state = ctx.enter_context(tc.tile_pool(name="state", bufs=1))
            acts = ctx.enter_context(tc.tile_pool(name="acts", bufs=2))
            smsb = ctx.enter_context(tc.tile_pool(name="smsb", bufs=2))
            # psum pools: 3 + 2 + 2 = 7 banks
            gpp = ctx.enter_context(tc.tile_pool(name="gpp", bufs=3, space="PSUM"))
            bigp = ctx.enter_context(tc.tile_pool(name="bigp", bufs=2, space="PSUM"))
            attp = ctx.enter_context(tc.tile_pool(name="attp", bufs=2, space="PSUM"))

            # ---- constants ----
            identb = const.tile([128, 128], bf, tag="idb")
            make_identity(nc, identb)
            ones80 = const.tile([T, 1], bf, tag="o80")
            nc.vector.memset(ones80, 1.0)
            ones1x128b = const.tile([1, 128], bf, tag="o1r")
            nc.vector.memset(ones1x128b, 1.0)
            ones1x128f = const.tile([1, 128], fp, tag="o1rf")
            nc.vector.memset(ones1x128f, 1.0)
            ones128b = const.tile([128, 1], bf, tag="o1c")
            nc.vector.memset(ones128b, 1.0)
            ones128f = const.tile([128, 1], fp, tag="o1cf")
            nc.vector.memset(ones128f, 1.0)

            # ---- persistent weights / addends ----
            w1hh = state.tile([128, 2, 8, 128], bf, tag="w1hh")
            nc.sync.dma_start(w1hh, w1hh_d[:, :, :, :])
            w2 = state.tile([128, 4, 8, 128], bf, tag="w2")
            nc.sync.dma_start(w2, w2_d[:, :, :, :])
            wd1 = state.tile([128, 2, 8, 128], bf, tag="wd1")
            nc.sync.dma_start(wd1, wd1_d[:, :, :, :])
            wd2 = state.tile([128, 4, 8, 128], bf, tag="wd2")
            nc.sync.dma_start(wd2, wd2_d[:, :, :, :])
            b1col = state.tile([128, 8], fp, tag="b1c")
            nc.sync.dma_start(b1col, b1col_d[:, :])
            b2bc = state.tile([128, 8, BS], fp, tag="b2bc")
            nc.sync.dma_start(b2bc, b2bc_d[:, :, :])
            bd1bc = state.tile([128, 8, BS], fp, tag="bd1bc")
            nc.sync.dma_start(bd1bc, bd1bc_d[:, :, :])
            bd2col = state.tile([128, 8], fp, tag="bd2c")
            nc.sync.dma_start(bd2col, bd2col_d[:, :])
            wtgt = state.tile([128, 2, DEC, BS], bf, tag="wtgt")
            nc.sync.dma_start(wtgt, wtgt_d[:, :, :, :])

            # ---- persistent activations ----
            h2seqT = state.tile([128, 2, T, BS], bf, tag="h2seq")
            h2decT = state.tile([128, 2, DEC, BS], bf, tag="h2dec")
            A_sb = state.tile([T, BS, H], bf, tag="Asb")
            capgT = state.tile([128, 8, ROWS], fp, tag="capg")
            ce_acc = state.tile([1, 4], fp, tag="cea")

            def lstm_elem_T(gates, c_old, out_h, tg_suffix):
                """gates [128, 8, BS] in chunk order i0 i1 f0 f1 o0 o1 g0 g1.
                Writes h (bf16) to out_h [128, 2, BS]; returns new c tile."""
                sio = acts.tile([128, 6, BS], fp, tag="sio" + tg_suffix)
                nc.scalar.activation(sio, gates[:, 0:6, :], AF.Sigmoid)
                tg = acts.tile([128, 2, BS], fp, tag="tg" + tg_suffix)
                nc.scalar.activation(tg, gates[:, 6:8, :], AF.Tanh)
                t1 = acts.tile([128, 2, BS], fp, tag="t1" + tg_suffix)
                nc.vector.tensor_mul(t1, sio[:, 0:2, :], tg)
                if c_old is None:
                    c_new = t1
                else:
                    cm = acts.tile([128, 2, BS], fp, tag="cm" + tg_suffix)
                    nc.vector.tensor_mul(cm, sio[:, 2:4, :], c_old)
                    c_new = acts.tile([128, 2, BS], fp, tag="c" + tg_suffix)
                    nc.vector.tensor_add(c_new, cm, t1)
                th = acts.tile([128, 2, BS], fp, tag="th" + tg_suffix)
                nc.scalar.activation(th, c_new, AF.Tanh)
                nc.vector.tensor_mul(out_h, sio[:, 4:6, :], th)
                return c_new

            # ================ P1 + encoder (scoped: feat/w1/g1 freed after) ====
            with ExitStack() as p1ctx:
                p1w = p1ctx.enter_context(tc.tile_pool(name="p1w", bufs=1))

                capT = p1w.tile([128, 2, ROWS], bf, tag="capT")
                nc.sync.dma_start(capT, capT_d[:, :, :])
                wd2l = p1w.tile([128, 2, 8, 128], bf, tag="wd2l")
                nc.sync.dma_start(wd2l, wd2l_d[:, :, :, :])
                feat_sb = p1w.tile([128, KF, TB], bf, tag="feat")
                nc.sync.dma_start(feat_sb, featT_d[:, :, :])
                w1sb = p1w.tile([128, 8, KF, 128], bf, tag="w1")
                nc.sync.dma_start(w1sb, w1_d[:, :, :, :])
                g1Ts = [
                    p1w.tile([128, 8, 128], bf, tag=f"g1s{s}", name=f"g1s{s}")
                    for s in range(5)
                ]

                # cap_proj: capgT[:, m, r] = (cap @ d2_Wih_l.T + d2_b) gatesT
                for m in range(8):
                    ps = bigp.tile([128, CSZ], fp, tag="big")
                    for kc in range(2):
                        nc.tensor.matmul(
                            ps[:, 0:ROWS], wd2l[:, kc, m, :], capT[:, kc, :],
                            start=(kc == 0), stop=(kc == 1),
                        )
                    nc.vector.tensor_scalar_add(
                        capgT[:, m, :], ps[:, 0:ROWS], bd2col[:, m : m + 1]
                    )

                # G1 slice builder: 32-k accumulation for (s, m), in halves
                g1ps_box = [None]

                def g1_group(s, m, half):
                    if half == 0:
                        g1ps_box[0] = bigp.tile(
                            [128, CSZ], fp, tag="big", name=f"g1ps{s}_{m}"
                        )
                    ps = g1ps_box[0]
                    for k in range(16 * half, 16 * half + 16):
                        nc.tensor.matmul(
                            ps[:, 0:128],
                            w1sb[:, m, k, :],
                            feat_sb[:, k, ds(128 * s, 128)],
                            start=(k == 0), stop=(k == KF - 1),
                        )
                    if half == 1:
                        nc.vector.tensor_scalar_add(
                            g1Ts[s][:, m, :], ps[:, 0:128], b1col[:, m : m + 1]
                        )

                for m in range(8):  # slice 0 up front
                    g1_group(0, m, 0)
                    g1_group(0, m, 1)

                # ================ P2: encoder ================
                h1T = None
                c1 = None
                c2 = None
                for t in range(T):
                    s = t // 16
                    # ---- cell 1 ----
                    if t == 0:
                        gates1 = g1Ts[0][:, :, 0:BS]
                    else:
                        ps = gpp.tile([128, 8, BS], fp, tag="g")
                        for m in range(8):
                            for kc in range(2):
                                nc.tensor.matmul(
                                    ps[:, m, :], w1hh[:, kc, m, :], h1T[:, kc, :],
                                    start=(kc == 0), stop=(kc == 1),
                                )
                        gates1 = acts.tile([128, 8, BS], fp, tag="ga")
                        nc.vector.tensor_add(
                            gates1, ps, g1Ts[s][:, :, ds(BS * (t % 16), BS)]
                        )
                    h1T_new = acts.tile([128, 2, BS], bf, tag="h1")
                    c1 = lstm_elem_T(gates1, c1, h1T_new, "1")
                    h1T = h1T_new
                    # ---- cell 2 ----
                    ps2 = gpp.tile([128, 8, BS], fp, tag="g")
                    nkc = 2 if t == 0 else 4
                    for m in range(8):
                        for kc in range(nkc):
                            rhs = (
                                h1T[:, kc, :]
                                if kc < 2
                                else h2seqT[:, kc - 2, t - 1, :]
                            )
                            nc.tensor.matmul(
                                ps2[:, m, :], w2[:, kc, m, :], rhs,
                                start=(kc == 0), stop=(kc == nkc - 1),
                            )
                    gates2 = acts.tile([128, 8, BS], fp, tag="ga")
                    nc.vector.tensor_add(gates2, ps2, b2bc)
                    c2 = lstm_elem_T(gates2, c2, h2seqT[:, :, t, :], "2")
                    # ---- G1 interleave: slice 1 + t//16, one half-group per step
                    if t < 64:
                        g1_group(1 + t // 16, (t % 16) // 2, t % 2)

            # A_sb[te, b, :] = h2seq[b, te, :] (row layout for context matmul)
            for b in range(BS):
                for kc in range(2):
                    pA = attp.tile([T, 128], bf, tag="at")
                    nc.tensor.transpose(pA, h2seqT[:, kc, :, b], identb)
                    nc.vector.tensor_copy(A_sb[:, b, ts(kc, 128)], pA)

            # ================ P4 emitter ================
            wos = ctx.enter_context(tc.tile_pool(name="wos", bufs=3))
            junk = ctx.enter_context(tc.tile_pool(name="junk", bufs=2))
            p4sb = ctx.enter_context(tc.tile_pool(name="p4sb", bufs=2))

            def emit_p4(mi):
                t0 = 16 * mi
                tn = 16 if mi == 0 else DEC - 16
                R = tn * BS
                s_all = p4sb.tile([128, NCH], fp, tag="sall")
                for c in range(NCH):
                    wot = wos.tile([128, 2, CSZ], bf, tag="wo")
                    nc.sync.dma_start(wot, wo_d[:, :, ts(c, CSZ)])
                    bot = wos.tile([1, CSZ], bf, tag="bo")
                    nc.sync.dma_start(bot, bo_d[:, ts(c, CSZ)])
                    psL = bigp.tile([128, CSZ], fp, tag="big")
                    nc.tensor.matmul(
                        psL[:R], ones1x128b[:, :R], bot, start=True, stop=False
                    )
                    for kc in range(2):
                        nc.tensor.matmul(
                            psL[:R],
                            h2decT[:, kc, t0 : t0 + tn, :],
                            wot[:, kc, :],
                            start=False, stop=(kc == 1),
                        )
                    jk = junk.tile([128, CSZ], bf, tag="jk")
                    nc.scalar.activation(
                        jk[:R], psL[:R], AF.Exp, accum_out=s_all[:R, c : c + 1]
                    )
                ssum = p4sb.tile([128, 1], fp, tag="ssum")
                nc.vector.reduce_sum(ssum[:R], s_all[:R], axis=AX.X)
                lse = p4sb.tile([128, 1], fp, tag="lse")
                nc.scalar.activation(lse[:R], ssum[:R], AF.Ln)
                at = attp.tile([128, 168], fp, tag="at")
                nc.tensor.matmul(
                    at[0:1, 33:34], lse[:R], ones128f[:R], start=True, stop=True
                )
                nc.vector.tensor_copy(ce_acc[:, 2 * mi : 2 * mi + 1], at[0:1, 33:34])
                # target-row dot: sum_rows h2dec . w_tgt
                prod = p4sb.tile([128, 2, 16, BS], bf, tag="prod")
                for kc in range(2):
                    nc.vector.tensor_mul(
                        prod[:, kc, 0:tn, :],
                        h2decT[:, kc, t0 : t0 + tn, :],
                        wtgt[:, kc, t0 : t0 + tn, :],
                    )
                for kc in range(2):
                    nc.tensor.matmul(
                        at[0:1, 40 : 40 + R], ones128b, prod[:, kc, 0:tn, :],
                        start=(kc == 0), stop=(kc == 1),
                    )
                ltsum = p4sb.tile([1, 1], fp, tag="lts")
                nc.vector.reduce_sum(ltsum, at[0:1, 40 : 40 + R], axis=AX.X)
                nc.vector.tensor_copy(ce_acc[:, 2 * mi + 1 : 2 * mi + 2], ltsum)

            # ================ P3: decoder ================
            h2aT = None
            for t in range(DEC):
                # d1: gates = d1_b + h1 @ d1_Whh.T
                ps = gpp.tile([128, 8, BS], fp, tag="g")
                for m in range(8):
                    for kc in range(2):
                        nc.tensor.matmul(
                            ps[:, m, :], wd1[:, kc, m, :], h1T[:, kc, :],
                            start=(kc == 0), stop=(kc == 1),
                        )
                gates1 = acts.tile([128, 8, BS], fp, tag="ga")
                nc.vector.tensor_add(gates1, ps, bd1bc)
                h1T_new = acts.tile([128, 2, BS], bf, tag="h1")
                c1 = lstm_elem_T(gates1, c1, h1T_new, "1")
                h1T = h1T_new
                # d2: gates = cap_proj[t] + h1' @ d2_Wih_r.T + h2 @ d2_Whh.T
                ps2 = gpp.tile([128, 8, BS], fp, tag="g")
                for m in range(8):
                    for kc in range(4):
                        if kc < 2:
                            rhs = h1T[:, kc, :]
                        elif t == 0:
                            rhs = h2seqT[:, kc - 2, T - 1, :]
                        else:
                            rhs = h2aT[:, kc - 2, :]
                        nc.tensor.matmul(
                            ps2[:, m, :], wd2[:, kc, m, :], rhs,
                            start=(kc == 0), stop=(kc == 3),
                        )
                gates2 = acts.tile([128, 8, BS], fp, tag="ga")
                nc.vector.tensor_add(gates2, ps2, capgT[:, :, ds(BS * t, BS)])
                c2 = lstm_elem_T(gates2, c2, h2decT[:, :, t, :], "2")
                # attention: h2 <- softmax(h2seq . h2) . h2seq
                at = attp.tile([128, 168], fp, tag="at")
                q = h2decT[:, :, t, :]
                for b in range(BS):
                    for kc in range(2):
                        nc.tensor.matmul(
                            at[0:T, b : b + 1],
                            h2seqT[:, kc, :, b],
                            q[:, kc, b : b + 1],
                            start=(kc == 0), stop=(kc == 1),
                        )
                expT = smsb.tile([T, BS], bf, tag="exp")
                nc.scalar.activation(expT, at[0:T, 0:BS], AF.Exp)
                nc.tensor.matmul(
                    at[0:1, 32:40], ones80, expT, start=True, stop=True
                )
                recip = smsb.tile([1, BS], fp, tag="rcp")
                nc.vector.reciprocal(recip, at[0:1, 32:40])
                nc.tensor.matmul(
                    at[:, 24:32], ones1x128f, recip, start=True, stop=True
                )
                bcs = smsb.tile([128, BS], bf, tag="bcs")
                nc.vector.tensor_copy(bcs, at[:, 24:32])
                for b in range(BS):
                    for hc in range(2):
                        col = 8 + 8 * hc + b
                        nc.tensor.matmul(
                            at[:, col : col + 1],
                            A_sb[:, b, ts(hc, 128)],
                            expT[:, b : b + 1],
                            start=True, stop=True,
                        )
                h2aT = smsb.tile([128, 2, BS], bf, tag="h2a")
                for hc in range(2):
                    nc.vector.tensor_mul(
                        h2aT[:, hc, :], at[:, ds(8 + 8 * hc, 8)], bcs
                    )
                if t == 15:
                    emit_p4(0)
                elif t == DEC - 1:
                    emit_p4(1)

            # final: partial = (lse0 - dot0 + lse1 - dot1) / B^2
            d0 = smsb.tile([1, 1], fp, tag="d0")
            nc.vector.tensor_sub(d0, ce_acc[:, 0:1], ce_acc[:, 1:2])
            d1_ = smsb.tile([1, 1], fp, tag="d1")
            nc.vector.tensor_sub(d1_, ce_acc[:, 2:3], ce_acc[:, 3:4])
            tot = smsb.tile([1, 1], fp, tag="tot")
            nc.vector.tensor_add(tot, d0, d1_)
            outsb = smsb.tile([1, 1], fp, tag="osb")
            nc.scalar.mul(outsb, tot, 1.0 / (B * B))
            nc.sync.dma_start(out_d[:, :], outsb)

    nc.compile()
    return nc


def _shard_inputs(inputs):
    """Host-side relayout + shard. Returns (list of 8 in_maps, host_bias)."""
    import ml_dtypes

    f32 = np.float32
    bft = ml_dtypes.bfloat16
    feat = np.asarray(inputs["feat"], f32)
    caption = np.asarray(inputs["caption"], f32)
    oh = np.asarray(inputs["caption_one_hot"], f32)

    def w(name):
        return np.asarray(inputs[name], f32)

    def wtile(Wt):  # Wt [K, 1024] -> [128, K//128, 8, 128] bf16, m permuted
        Kc = Wt.shape[0] // 128
        a = Wt.reshape(Kc, 128, 8, 128)[:, :, PERM, :]
        return np.ascontiguousarray(a.transpose(1, 0, 2, 3).astype(bft))

    def bcol(bv):  # [1024] -> [128, 8] fp32, m permuted
        return np.ascontiguousarray(bv.reshape(8, 128)[PERM].T.astype(f32))

    def bbc(bv):  # [1024] -> [128, 8, BS] broadcast over batch
        return np.ascontiguousarray(
            np.repeat(bcol(bv)[:, :, None], BS, axis=2)
        )

    out_W = w("out_W")
    shared = dict(
        w1t=wtile(w("e1_Wih").T),
        w1hht=wtile(w("e1_Whh").T),
        w2t=wtile(np.concatenate([w("e2_Wih")[:, H:], w("e2_Whh")], 1).T),
        wd1t=wtile(w("d1_Whh").T),
        wd2t=wtile(np.concatenate([w("d2_Wih")[:, H:], w("d2_Whh")], 1).T),
        wd2lt=wtile(w("d2_Wih")[:, :H].T),
        b1col=bcol(w("e1_b")),
        b2bc=bbc(w("e2_b")),
        bd1bc=bbc(w("d1_b")),
        bd2col=bcol(w("d2_b")),
        wot=np.ascontiguousarray(
            out_W.T.reshape(2, 128, V).transpose(1, 0, 2).astype(bft)
        ),
        bot=np.ascontiguousarray(w("out_b").reshape(1, V).astype(bft)),
    )

    tgt = np.argmax(oh, axis=2)[:, 1:]  # [B, DEC]
    host_bias = f32(w("out_b")[tgt].sum()) / f32(B * B)

    in_maps = []
    for c in range(NCORES):
        b0 = c * BS
        featT = np.ascontiguousarray(
            feat[b0 : b0 + BS]
            .transpose(2, 1, 0).reshape(KF, 128, TB).transpose(1, 0, 2)
            .astype(bft)
        )
        capT = np.ascontiguousarray(
            caption[b0 : b0 + BS, :DEC]
            .transpose(2, 1, 0).reshape(2, 128, ROWS).transpose(1, 0, 2)
            .astype(bft)
        )
        wt = out_W[tgt[b0 : b0 + BS]]  # [BS, DEC, H]
        wtgt = np.ascontiguousarray(
            wt.transpose(2, 1, 0).reshape(2, 128, DEC, BS)
            .transpose(1, 0, 2, 3).astype(bft)
        )
        m = dict(shared)
        m.update(featT=featT, capT=capT, wtgt=wtgt)
        in_maps.append(m)
    return in_maps, host_bias


def kernel(**inputs):
    from concourse.bass_utils import run_bass_kernel_spmd

    if "nc" not in _cache:
        _cache["nc"] = _build_program()
    nc = _cache["nc"]
    in_maps, host_bias = _shard_inputs(inputs)
    res = run_bass_kernel_spmd(nc, in_maps, core_ids=list(range(NCORES)))
    total = np.float32(0.0)
    for r in res.results:
        total += np.float32(r["partial"][0, 0])
    total -= host_bias
    return np.asarray(total, np.float32)


# revision 9
# speedup vs baseline: 4.0714x; 1.3090x over previous
"""Trainium2 Bass kernel for nn_Net_74259984548321 (video-caption LSTM net).

v2 design (vs v1: all-fp32, row-layout gates, device-side one-hot argmax):
  * all matmuls bf16 (fp32 matmuls cost 4 cycles/row on trn2).
  * recurrence in transposed "gatesT" layout: weight tiles [K=128, M=128]
    stationary, gates/h/c live as [128 gate/h dims, batch] so elementwise
    runs on 128 partitions with tiny free dims (8-64 elems) instead of 8
    partitions x 256-512 elems; h is born in the layout the next matmul
    needs (no per-step PE transposes).
  * biases folded into precomputed addends (g1/cap projections) or constant
    broadcast tiles (one DVE add per cell, no K=1 bias matmuls in the loop).
  * attention: scores -> exp (no max subtraction; scores are O(1)) ->
    unnormalized context + reciprocal-broadcast matmul; no transposes.
  * CE: targets argmax'd on HOST, out_W target rows gathered on HOST;
    device computes full-vocab LSE (streamed out_W, online accum) plus a
    per-row dot with the gathered rows. caption_one_hot never touches HBM.
  * feat projection G1 (the only big GEMM) runs one 16-step slice ahead of
    the encoder, interleaved into the encoder's PE idle gaps.
Per core: 8 batch rows; host sums 8 partial scalars + target-bias term.
"""

import numpy as np

B, T, FEAT, H, V, L = 64, 80, 4096, 256, 8000, 32
DEC = L - 1            # 31 decoder steps
NCORES = 8
BS = B // NCORES       # 8 batch rows per core
G = 4 * H              # 1024 gates
KF = FEAT // 128       # 32 feat contraction chunks
ROWS = DEC * BS        # 248 decoder (t, b) rows per core
NCH = 16               # vocab chunks for LSE
CSZ = V // NCH         # 500
TB = T * BS            # 640 encoder (t, b) rows per core
# gate chunk order used on-chip: i0 i1 f0 f1 o0 o1 g0 g1 (source chunks)
PERM = np.array([0, 1, 2, 3, 6, 7, 4, 5])

_cache = {}


def _build_program():
    import concourse.tile as tile
    from concourse import bacc, mybir
    from concourse.bass import ts, ds
    from concourse.masks import make_identity

    fp = mybir.dt.float32
    bf = mybir.dt.bfloat16
    AF = mybir.ActivationFunctionType
    AX = mybir.AxisListType

    nc = bacc.Bacc(None, target_bir_lowering=False)

    featT_d = nc.dram_tensor("featT", [128, KF, TB], bf, kind="ExternalInput")
    w1_d = nc.dram_tensor("w1t", [128, 8, KF, 128], bf, kind="ExternalInput")
    w1hh_d = nc.dram_tensor("w1hht", [128, 2, 8, 128], bf, kind="ExternalInput")
    w2_d = nc.dram_tensor("w2t", [128, 4, 8, 128], bf, kind="ExternalInput")
    wd1_d = nc.dram_tensor("wd1t", [128, 2, 8, 128], bf, kind="ExternalInput")
    wd2_d = nc.dram_tensor("wd2t", [128, 4, 8, 128], bf, kind="ExternalInput")
    wd2l_d = nc.dram_tensor("wd2lt", [128, 2, 8, 128], bf, kind="ExternalInput")
    b1col_d = nc.dram_tensor("b1col", [128, 8], fp, kind="ExternalInput")
    b2bc_d = nc.dram_tensor("b2bc", [128, 8, BS], fp, kind="ExternalInput")
    bd1bc_d = nc.dram_tensor("bd1bc", [128, 8, BS], fp, kind="ExternalInput")
    bd2col_d = nc.dram_tensor("bd2col", [128, 8], fp, kind="ExternalInput")
    capT_d = nc.dram_tensor("capT", [128, 2, ROWS], bf, kind="ExternalInput")
    wo_d = nc.dram_tensor("wot", [128, 2, V], bf, kind="ExternalInput")
    bo_d = nc.dram_tensor("bot", [1, V], bf, kind="ExternalInput")
    wtgt_d = nc.dram_tensor("wtgt", [128, 2, DEC, BS], bf, kind="ExternalInput")
    out_d = nc.dram_tensor("partial", [1, 1], fp, kind="ExternalOutput")

    with tile.TileContext(nc) as tc:
        from contextlib import ExitStack

        with ExitStack() as ctx:
            const = ctx.enter_context(tc.tile_pool(name="const", bufs=1))
            state = ctx.enter_context(tc.tile_pool(name="state", bufs=1))
            acts = ctx.enter_context(tc.tile_pool(name="acts", bufs=2))
            smsb = ctx.enter_context(tc.tile_pool(name="smsb", bufs=2))
            # psum pools: 3 + 2 + 2 = 7 banks
            gpp = ctx.enter_context(tc.tile_pool(name="gpp", bufs=3, space="PSUM"))
            bigp = ctx.enter_context(tc.tile_pool(name="bigp", bufs=2, space="PSUM"))
            attp = ctx.enter_context(tc.tile_pool(name="attp", bufs=2, space="PSUM"))

            # ---- constants ----
            identb = const.tile([128, 128], bf, tag="idb")
            make_identity(nc, identb)
            ones80 = const.tile([T, 1], bf, tag="o80")
            nc.vector.memset(ones80, 1.0)
            ones1x128b = const.tile([1, 128], bf, tag="o1r")
            nc.vector.memset(ones1x128b, 1.0)
            ones1x128f = const.tile([1, 128], fp, tag="o1rf")
            nc.vector.memset(ones1x128f, 1.0)
            ones128b = const.tile([128, 1], bf, tag="o1c")
            nc.vector.memset(ones128b, 1.0)
            ones128f = const.tile([128, 1], fp, tag="o1cf")
            nc.vector.memset(ones128f, 1.0)

            # ---- persistent weights / addends ----
            w1hh = state.tile([128, 2, 8, 128], bf, tag="w1hh")
            nc.sync.dma_start(w1hh, w1hh_d[:, :, :, :])
            w2 = state.tile([128, 4, 8, 128], bf, tag="w2")
            nc.sync.dma_start(w2, w2_d[:, :, :, :])
            wd1 = state.tile([128, 2, 8, 128], bf, tag="wd1")
            nc.sync.dma_start(wd1, wd1_d[:, :, :, :])
            wd2 = state.tile([128, 4, 8, 128], bf, tag="wd2")
            nc.sync.dma_start(wd2, wd2_d[:, :, :, :])
            b1col = state.tile([128, 8], fp, tag="b1c")
            nc.sync.dma_start(b1col, b1col_d[:, :])
            b2bc = state.tile([128, 8, BS], fp, tag="b2bc")
            nc.sync.dma_start(b2bc, b2bc_d[:, :, :])
            bd1bc = state.tile([128, 8, BS], fp, tag="bd1bc")
            nc.sync.dma_start(bd1bc, bd1bc_d[:, :, :])
            bd2col = state.tile([128, 8], fp, tag="bd2c")
            nc.sync.dma_start(bd2col, bd2col_d[:, :])
            wtgt = state.tile([128, 2, DEC, BS], bf, tag="wtgt")
            nc.sync.dma_start(wtgt, wtgt_d[:, :, :, :])

            # ---- persistent activations ----
            h2seqT = state.tile([128, 2, T, BS], bf, tag="h2seq")
            h2decT = state.tile([128, 2, DEC, BS], bf, tag="h2dec")
            A_sb = state.tile([T, BS, H], bf, tag="Asb")
            capgT = state.tile([128, 8, ROWS], fp, tag="capg")
            ce_acc = state.tile([1, 4], fp, tag="cea")

            ALU = mybir.AluOpType

            def lstm_elem_T(gates, c_old, out_h, tg_suffix):
                """gates [128, 8, BS] in chunk order i0 i1 f0 f1 o0 o1 g0 g1.
                Writes h (bf16) to out_h [128, 2, BS]; returns new c tile.
                sigmoid(x) = 0.5*tanh(0.5x) + 0.5 so the only ACT funcs in
                the loops are Tanh/Exp (one shared table, no reloads)."""
                tio = acts.tile([128, 6, BS], fp, tag="tio" + tg_suffix)
                nc.scalar.activation(tio, gates[:, 0:6, :], AF.Tanh, scale=0.5)
                sio = acts.tile([128, 6, BS], fp, tag="sio" + tg_suffix)
                nc.vector.tensor_scalar(sio, tio, 0.5, 0.5, ALU.mult, ALU.add)
                tg = acts.tile([128, 2, BS], fp, tag="tg" + tg_suffix)
                nc.scalar.activation(tg, gates[:, 6:8, :], AF.Tanh)
                t1 = acts.tile([128, 2, BS], fp, tag="t1" + tg_suffix)
                nc.vector.tensor_mul(t1, sio[:, 0:2, :], tg)
                if c_old is None:
                    c_new = t1
                else:
                    cm = acts.tile([128, 2, BS], fp, tag="cm" + tg_suffix)
                    nc.vector.tensor_mul(cm, sio[:, 2:4, :], c_old)
                    c_new = acts.tile([128, 2, BS], fp, tag="c" + tg_suffix)
                    nc.vector.tensor_add(c_new, cm, t1)
                th = acts.tile([128, 2, BS], fp, tag="th" + tg_suffix)
                nc.scalar.activation(th, c_new, AF.Tanh)
                nc.vector.tensor_mul(out_h, sio[:, 4:6, :], th)
                return c_new

            # ================ P1 + encoder (scoped: feat/w1/g1 freed after) ====
            with ExitStack() as p1ctx:
                p1w = p1ctx.enter_context(tc.tile_pool(name="p1w", bufs=1))

                capT = p1w.tile([128, 2, ROWS], bf, tag="capT")
                nc.sync.dma_start(capT, capT_d[:, :, :])
                wd2l = p1w.tile([128, 2, 8, 128], bf, tag="wd2l")
                nc.sync.dma_start(wd2l, wd2l_d[:, :, :, :])
                feat_sb = p1w.tile([128, KF, TB], bf, tag="feat")
                nc.sync.dma_start(feat_sb, featT_d[:, :, :])
                w1sb = p1w.tile([128, 8, KF, 128], bf, tag="w1")
                nc.sync.dma_start(w1sb, w1_d[:, :, :, :])
                g1Ts = [
                    p1w.tile([128, 8, 128], bf, tag=f"g1s{s}", name=f"g1s{s}")
                    for s in range(5)
                ]

                # cap_proj: capgT[:, m, r] = (cap @ d2_Wih_l.T + d2_b) gatesT
                for m in range(8):
                    ps = bigp.tile([128, CSZ], fp, tag="big")
                    for kc in range(2):
                        nc.tensor.matmul(
                            ps[:, 0:ROWS], wd2l[:, kc, m, :], capT[:, kc, :],
                            start=(kc == 0), stop=(kc == 1),
                        )
                    nc.vector.tensor_scalar_add(
                        capgT[:, m, :], ps[:, 0:ROWS], bd2col[:, m : m + 1]
                    )

                # G1 slice builder: 32-k accumulation for (s, m), in halves
                g1ps_box = [None]

                def g1_group(s, m, half):
                    if half == 0:
                        g1ps_box[0] = bigp.tile(
                            [128, CSZ], fp, tag="big", name=f"g1ps{s}_{m}"
                        )
                    ps = g1ps_box[0]
                    for k in range(16 * half, 16 * half + 16):
                        nc.tensor.matmul(
                            ps[:, 0:128],
                            w1sb[:, m, k, :],
                            feat_sb[:, k, ds(128 * s, 128)],
                            start=(k == 0), stop=(k == KF - 1),
                        )
                    if half == 1:
                        nc.vector.tensor_scalar_add(
                            g1Ts[s][:, m, :], ps[:, 0:128], b1col[:, m : m + 1]
                        )

                for m in range(8):  # slice 0 up front
                    g1_group(0, m, 0)
                    g1_group(0, m, 1)

                # ================ P2: encoder ================
                h1T = None
                c1 = None
                c2 = None
                for t in range(T):
                    s = t // 16
                    # ---- cell 1 ----
                    if t == 0:
                        gates1 = g1Ts[0][:, :, 0:BS]
                    else:
                        ps = gpp.tile([128, 8, BS], fp, tag="g")
                        for m in range(8):
                            for kc in range(2):
                                nc.tensor.matmul(
                                    ps[:, m, :], w1hh[:, kc, m, :], h1T[:, kc, :],
                                    start=(kc == 0), stop=(kc == 1),
                                )
                        gates1 = acts.tile([128, 8, BS], fp, tag="ga")
                        nc.vector.tensor_add(
                            gates1, ps, g1Ts[s][:, :, ds(BS * (t % 16), BS)]
                        )
                    h1T_new = acts.tile([128, 2, BS], bf, tag="h1")
                    c1 = lstm_elem_T(gates1, c1, h1T_new, "1")
                    h1T = h1T_new
                    # ---- cell 2 ----
                    ps2 = gpp.tile([128, 8, BS], fp, tag="g")
                    nkc = 2 if t == 0 else 4
                    for m in range(8):
                        for kc in range(nkc):
                            rhs = (
                                h1T[:, kc, :]
                                if kc < 2
                                else h2seqT[:, kc - 2, t - 1, :]
                            )
                            nc.tensor.matmul(
                                ps2[:, m, :], w2[:, kc, m, :], rhs,
                                start=(kc == 0), stop=(kc == nkc - 1),
                            )
                    gates2 = acts.tile([128, 8, BS], fp, tag="ga")
                    nc.vector.tensor_add(gates2, ps2, b2bc)
                    c2 = lstm_elem_T(gates2, c2, h2seqT[:, :, t, :], "2")
                    # ---- G1 interleave: slice 1 + t//16, one half-group per step
                    if t < 64:
                        g1_group(1 + t // 16, (t % 16) // 2, t % 2)

            # A_sb[te, b, :] = h2seq[b, te, :] (row layout for context matmul)
            for b in range(BS):
                for kc in range(2):
                    pA = attp.tile([T, 128], bf, tag="at")
                    nc.tensor.transpose(pA, h2seqT[:, kc, :, b], identb)
                    nc.vector.tensor_copy(A_sb[:, b, ts(kc, 128)], pA)

            # ================ P4 emitter ================
            wos = ctx.enter_context(tc.tile_pool(name="wos", bufs=3))
            junk = ctx.enter_context(tc.tile_pool(name="junk", bufs=2))
            p4sb = ctx.enter_context(tc.tile_pool(name="p4sb", bufs=2))

            def p4_begin(mi):
                t0 = 16 * mi
                tn = 16 if mi == 0 else DEC - 16
                s_all = p4sb.tile([128, NCH], fp, tag="sall", name=f"sall{mi}")
                return dict(mi=mi, t0=t0, tn=tn, R=tn * BS, s_all=s_all)

            def p4_chunk(st, c):
                t0, tn, R = st["t0"], st["tn"], st["R"]
                wot = wos.tile([128, 2, CSZ], bf, tag="wo")
                nc.sync.dma_start(wot, wo_d[:, :, ts(c, CSZ)])
                bot = wos.tile([1, CSZ], bf, tag="bo")
                nc.sync.dma_start(bot, bo_d[:, ts(c, CSZ)])
                psL = bigp.tile([128, CSZ], fp, tag="big")
                nc.tensor.matmul(
                    psL[:R], ones1x128b[:, :R], bot, start=True, stop=False
                )
                for kc in range(2):
                    nc.tensor.matmul(
                        psL[:R],
                        h2decT[:, kc, t0 : t0 + tn, :],
                        wot[:, kc, :],
                        start=False, stop=(kc == 1),
                    )
                jk = junk.tile([128, CSZ], bf, tag="jk")
                nc.scalar.activation(
                    jk[:R], psL[:R], AF.Exp,
                    accum_out=st["s_all"][:R, c : c + 1],
                )

            def p4_finish(st):
                mi, t0, tn, R = st["mi"], st["t0"], st["tn"], st["R"]
                ssum = p4sb.tile([128, 1], fp, tag="ssum")
                nc.vector.reduce_sum(ssum[:R], st["s_all"][:R], axis=AX.X)
                lse = p4sb.tile([128, 1], fp, tag="lse")
                nc.scalar.activation(lse[:R], ssum[:R], AF.Ln)
                at = attp.tile([128, 168], fp, tag="at")
                nc.tensor.matmul(
                    at[0:1, 33:34], lse[:R], ones128f[:R], start=True, stop=True
                )
                nc.vector.tensor_copy(ce_acc[:, 2 * mi : 2 * mi + 1], at[0:1, 33:34])
                # target-row dot: sum_rows h2dec . w_tgt
                prod = p4sb.tile([128, 2, 16, BS], bf, tag="prod")
                for kc in range(2):
                    nc.vector.tensor_mul(
                        prod[:, kc, 0:tn, :],
                        h2decT[:, kc, t0 : t0 + tn, :],
                        wtgt[:, kc, t0 : t0 + tn, :],
                    )
                for kc in range(2):
                    nc.tensor.matmul(
                        at[0:1, 40 : 40 + R], ones128b, prod[:, kc, 0:tn, :],
                        start=(kc == 0), stop=(kc == 1),
                    )
                ltsum = p4sb.tile([1, 1], fp, tag="lts")
                nc.vector.reduce_sum(ltsum, at[0:1, 40 : 40 + R], axis=AX.X)
                nc.vector.tensor_copy(ce_acc[:, 2 * mi + 1 : 2 * mi + 2], ltsum)

            # ================ P3: decoder ================
            h2aT = None
            p4st = [None, None]
            for t in range(DEC):
                # d1: gates = d1_b + h1 @ d1_Whh.T
                ps = gpp.tile([128, 8, BS], fp, tag="g")
                for m in range(8):
                    for kc in range(2):
                        nc.tensor.matmul(
                            ps[:, m, :], wd1[:, kc, m, :], h1T[:, kc, :],
                            start=(kc == 0), stop=(kc == 1),
                        )
                gates1 = acts.tile([128, 8, BS], fp, tag="ga")
                nc.vector.tensor_add(gates1, ps, bd1bc)
                h1T_new = acts.tile([128, 2, BS], bf, tag="h1")
                c1 = lstm_elem_T(gates1, c1, h1T_new, "1")
                h1T = h1T_new
                # d2: gates = cap_proj[t] + h1' @ d2_Wih_r.T + h2 @ d2_Whh.T
                ps2 = gpp.tile([128, 8, BS], fp, tag="g")
                for m in range(8):
                    for kc in range(4):
                        if kc < 2:
                            rhs = h1T[:, kc, :]
                        elif t == 0:
                            rhs = h2seqT[:, kc - 2, T - 1, :]
                        else:
                            rhs = h2aT[:, kc - 2, :]
                        nc.tensor.matmul(
                            ps2[:, m, :], wd2[:, kc, m, :], rhs,
                            start=(kc == 0), stop=(kc == 3),
                        )
                gates2 = acts.tile([128, 8, BS], fp, tag="ga")
                nc.vector.tensor_add(gates2, ps2, capgT[:, :, ds(BS * t, BS)])
                c2 = lstm_elem_T(gates2, c2, h2decT[:, :, t, :], "2")
                if t == DEC - 1:
                    # h2aT is never consumed after the last step; skip
                    # attention and emit the second CE tile immediately.
                    p4st[1] = p4_begin(1)
                    for c in range(NCH):
                        p4_chunk(p4st[1], c)
                    p4_finish(p4st[1])
                    break
                # attention: h2 <- softmax(h2seq . h2) . h2seq
                at = attp.tile([128, 168], fp, tag="at")
                q = h2decT[:, :, t, :]
                for b in range(BS):
                    for kc in range(2):
                        nc.tensor.matmul(
                            at[0:T, b : b + 1],
                            h2seqT[:, kc, :, b],
                            q[:, kc, b : b + 1],
                            start=(kc == 0), stop=(kc == 1),
                        )
                expT = smsb.tile([T, BS], bf, tag="exp")
                nc.scalar.activation(expT, at[0:T, 0:BS], AF.Exp)
                nc.tensor.matmul(
                    at[0:1, 32:40], ones80, expT, start=True, stop=True
                )
                recip = smsb.tile([1, BS], fp, tag="rcp")
                nc.vector.reciprocal(recip, at[0:1, 32:40])
                nc.tensor.matmul(
                    at[:, 24:32], ones1x128f, recip, start=True, stop=True
                )
                bcs = smsb.tile([128, BS], bf, tag="bcs")
                nc.vector.tensor_copy(bcs, at[:, 24:32])
                for b in range(BS):
                    for hc in range(2):
                        col = 8 + 8 * hc + b
                        nc.tensor.matmul(
                            at[:, col : col + 1],
                            A_sb[:, b, ts(hc, 128)],
                            expT[:, b : b + 1],
                            start=True, stop=True,
                        )
                h2aT = smsb.tile([128, 2, BS], bf, tag="h2a")
                for hc in range(2):
                    nc.vector.tensor_mul(
                        h2aT[:, hc, :], at[:, ds(8 + 8 * hc, 8)], bcs
                    )
                # interleave first CE tile (rows of steps 0..15) two vocab
                # chunks per step once its h2dec rows are complete
                if t == 16:
                    p4st[0] = p4_begin(0)
                if 16 <= t <= 23:
                    p4_chunk(p4st[0], 2 * (t - 16))
                    p4_chunk(p4st[0], 2 * (t - 16) + 1)
                elif t == 24:
                    p4_finish(p4st[0])

            # final: partial = (lse0 - dot0 + lse1 - dot1) / B^2
            d0 = smsb.tile([1, 1], fp, tag="d0")
            nc.vector.tensor_sub(d0, ce_acc[:, 0:1], ce_acc[:, 1:2])
            d1_ = smsb.tile([1, 1], fp, tag="d1")
            nc.vector.tensor_sub(d1_, ce_acc[:, 2:3], ce_acc[:, 3:4])
            tot = smsb.tile([1, 1], fp, tag="tot")
            nc.vector.tensor_add(tot, d0, d1_)
            outsb = smsb.tile([1, 1], fp, tag="osb")
            nc.scalar.mul(outsb, tot, 1.0 / (B * B))
            nc.sync.dma_start(out_d[:, :], outsb)

    nc.compile()
    return nc


def _shard_inputs(inputs):
    """Host-side relayout + shard. Returns (list of 8 in_maps, host_bias)."""
    import ml_dtypes

    f32 = np.float32
    bft = ml_dtypes.bfloat16
    feat = np.asarray(inputs["feat"], f32)
    caption = np.asarray(inputs["caption"], f32)
    oh = np.asarray(inputs["caption_one_hot"], f32)

    def w(name):
        return np.asarray(inputs[name], f32)

    def wtile(Wt):  # Wt [K, 1024] -> [128, K//128, 8, 128] bf16, m permuted
        Kc = Wt.shape[0] // 128
        a = Wt.reshape(Kc, 128, 8, 128)[:, :, PERM, :]
        return np.ascontiguousarray(a.transpose(1, 0, 2, 3).astype(bft))

    def bcol(bv):  # [1024] -> [128, 8] fp32, m permuted
        return np.ascontiguousarray(bv.reshape(8, 128)[PERM].T.astype(f32))

    def bbc(bv):  # [1024] -> [128, 8, BS] broadcast over batch
        return np.ascontiguousarray(
            np.repeat(bcol(bv)[:, :, None], BS, axis=2)
        )

    out_W = w("out_W")
    shared = dict(
        w1t=wtile(w("e1_Wih").T),
        w1hht=wtile(w("e1_Whh").T),
        w2t=wtile(np.concatenate([w("e2_Wih")[:, H:], w("e2_Whh")], 1).T),
        wd1t=wtile(w("d1_Whh").T),
        wd2t=wtile(np.concatenate([w("d2_Wih")[:, H:], w("d2_Whh")], 1).T),
        wd2lt=wtile(w("d2_Wih")[:, :H].T),
        b1col=bcol(w("e1_b")),
        b2bc=bbc(w("e2_b")),
        bd1bc=bbc(w("d1_b")),
        bd2col=bcol(w("d2_b")),
        wot=np.ascontiguousarray(
            out_W.T.reshape(2, 128, V).transpose(1, 0, 2).astype(bft)
        ),
        bot=np.ascontiguousarray(w("out_b").reshape(1, V).astype(bft)),
    )

    tgt = np.argmax(oh, axis=2)[:, 1:]  # [B, DEC]
    host_bias = f32(w("out_b")[tgt].sum()) / f32(B * B)

    in_maps = []
    for c in range(NCORES):
        b0 = c * BS
        featT = np.ascontiguousarray(
            feat[b0 : b0 + BS]
            .transpose(2, 1, 0).reshape(KF, 128, TB).transpose(1, 0, 2)
            .astype(bft)
        )
        capT = np.ascontiguousarray(
            caption[b0 : b0 + BS, :DEC]
            .transpose(2, 1, 0).reshape(2, 128, ROWS).transpose(1, 0, 2)
            .astype(bft)
        )
        wt = out_W[tgt[b0 : b0 + BS]]  # [BS, DEC, H]
        wtgt = np.ascontiguousarray(
            wt.transpose(2, 1, 0).reshape(2, 128, DEC, BS)
            .transpose(1, 0, 2, 3).astype(bft)
        )
        m = dict(shared)
        m.update(featT=featT, capT=capT, wtgt=wtgt)
        in_maps.append(m)
    return in_maps, host_bias


def kernel(**inputs):
    from concourse.bass_utils import run_bass_kernel_spmd

    if "nc" not in _cache:
        _cache["nc"] = _build_program()
    nc = _cache["nc"]
    in_maps, host_bias = _shard_inputs(inputs)
    res = run_bass_kernel_spmd(nc, in_maps, core_ids=list(range(NCORES)))
    total = np.float32(0.0)
    for r in res.results:
        total += np.float32(r["partial"][0, 0])
    total -= host_bias
    return np.asarray(total, np.float32)


# revision 13
# speedup vs baseline: 4.2298x; 1.0389x over previous
"""Trainium2 Bass kernel for nn_Net_74259984548321 (video-caption LSTM net).

v2 design (vs v1: all-fp32, row-layout gates, device-side one-hot argmax):
  * all matmuls bf16 (fp32 matmuls cost 4 cycles/row on trn2).
  * recurrence in transposed "gatesT" layout: weight tiles [K=128, M=128]
    stationary, gates/h/c live as [128 gate/h dims, batch] so elementwise
    runs on 128 partitions with tiny free dims (8-64 elems) instead of 8
    partitions x 256-512 elems; h is born in the layout the next matmul
    needs (no per-step PE transposes).
  * biases folded into precomputed addends (g1/cap projections) or constant
    broadcast tiles (one DVE add per cell, no K=1 bias matmuls in the loop).
  * attention: scores -> exp (no max subtraction; scores are O(1)) ->
    unnormalized context + reciprocal-broadcast matmul; no transposes.
  * CE: targets argmax'd on HOST, out_W target rows gathered on HOST;
    device computes full-vocab LSE (streamed out_W, online accum) plus a
    per-row dot with the gathered rows. caption_one_hot never touches HBM.
  * feat projection G1 (the only big GEMM) runs one 16-step slice ahead of
    the encoder, interleaved into the encoder's PE idle gaps.
Per core: 8 batch rows; host sums 8 partial scalars + target-bias term.
"""

import numpy as np

B, T, FEAT, H, V, L = 64, 80, 4096, 256, 8000, 32
DEC = L - 1            # 31 decoder steps
NCORES = 8
BS = B // NCORES       # 8 batch rows per core
G = 4 * H              # 1024 gates
KF = FEAT // 128       # 32 feat contraction chunks
ROWS = DEC * BS        # 248 decoder (t, b) rows per core
NCH = 16               # vocab chunks for LSE
CSZ = V // NCH         # 500
TB = T * BS            # 640 encoder (t, b) rows per core
# gate chunk order used on-chip: i0 i1 f0 f1 o0 o1 g0 g1 (source chunks)
PERM = np.array([0, 1, 2, 3, 6, 7, 4, 5])

_cache = {}


def _build_program():
    import concourse.tile as tile
    from concourse import bacc, mybir
    from concourse.bass import ts, ds
    from concourse.masks import make_identity

    fp = mybir.dt.float32
    bf = mybir.dt.bfloat16
    AF = mybir.ActivationFunctionType
    AX = mybir.AxisListType

    nc = bacc.Bacc(None, target_bir_lowering=False)

    featT_d = nc.dram_tensor("featT", [128, KF, TB], bf, kind="ExternalInput")
    w1_d = nc.dram_tensor("w1t", [128, 8, KF, 128], bf, kind="ExternalInput")
    w1hh_d = nc.dram_tensor("w1hht", [128, 2, 8, 128], bf, kind="ExternalInput")
    w2_d = nc.dram_tensor("w2t", [128, 4, 8, 128], bf, kind="ExternalInput")
    wd1_d = nc.dram_tensor("wd1t", [128, 2, 8, 128], bf, kind="ExternalInput")
    wd2_d = nc.dram_tensor("wd2t", [128, 4, 8, 128], bf, kind="ExternalInput")
    wd2l_d = nc.dram_tensor("wd2lt", [128, 2, 8, 128], bf, kind="ExternalInput")
    b1col_d = nc.dram_tensor("b1col", [128, 8], fp, kind="ExternalInput")
    b2bc_d = nc.dram_tensor("b2bc", [128, 8, BS], bf, kind="ExternalInput")
    bd1bc_d = nc.dram_tensor("bd1bc", [128, 8, BS], bf, kind="ExternalInput")
    bd2col_d = nc.dram_tensor("bd2col", [128, 8], fp, kind="ExternalInput")
    capT_d = nc.dram_tensor("capT", [128, 2, ROWS], bf, kind="ExternalInput")
    wo_d = nc.dram_tensor("wot", [128, 2, V], bf, kind="ExternalInput")
    bo_d = nc.dram_tensor("bot", [1, V], bf, kind="ExternalInput")
    wtgt_d = nc.dram_tensor("wtgt", [128, 2, DEC, BS], bf, kind="ExternalInput")
    out_d = nc.dram_tensor("partial", [1, 1], fp, kind="ExternalOutput")

    with tile.TileContext(nc) as tc:
        from contextlib import ExitStack

        with ExitStack() as ctx:
            const = ctx.enter_context(tc.tile_pool(name="const", bufs=1))
            state = ctx.enter_context(tc.tile_pool(name="state", bufs=1))
            acts = ctx.enter_context(tc.tile_pool(name="acts", bufs=2))
            smsb = ctx.enter_context(tc.tile_pool(name="smsb", bufs=2))
            # psum pools: 3 + 2 + 2 = 7 banks
            gpp = ctx.enter_context(tc.tile_pool(name="gpp", bufs=3, space="PSUM"))
            bigp = ctx.enter_context(tc.tile_pool(name="bigp", bufs=2, space="PSUM"))
            attp = ctx.enter_context(tc.tile_pool(name="attp", bufs=2, space="PSUM"))

            # ---- constants ----
            identb = const.tile([128, 128], bf, tag="idb")
            make_identity(nc, identb)
            ones80 = const.tile([T, 1], bf, tag="o80")
            nc.vector.memset(ones80, 1.0)
            ones1x128b = const.tile([1, 128], bf, tag="o1r")
            nc.vector.memset(ones1x128b, 1.0)
            ones1x128f = const.tile([1, 128], fp, tag="o1rf")
            nc.vector.memset(ones1x128f, 1.0)
            ones128b = const.tile([128, 1], bf, tag="o1c")
            nc.vector.memset(ones128b, 1.0)
            ones128f = const.tile([128, 1], fp, tag="o1cf")
            nc.vector.memset(ones128f, 1.0)

            # ---- persistent weights / addends ----
            w1hh = state.tile([128, 2, 8, 128], bf, tag="w1hh")
            nc.sync.dma_start(w1hh, w1hh_d[:, :, :, :])
            w2 = state.tile([128, 4, 8, 128], bf, tag="w2")
            nc.sync.dma_start(w2, w2_d[:, :, :, :])
            wd1 = state.tile([128, 2, 8, 128], bf, tag="wd1")
            nc.sync.dma_start(wd1, wd1_d[:, :, :, :])
            wd2 = state.tile([128, 4, 8, 128], bf, tag="wd2")
            nc.sync.dma_start(wd2, wd2_d[:, :, :, :])
            b1col = state.tile([128, 8], fp, tag="b1c")
            nc.sync.dma_start(b1col, b1col_d[:, :])
            b2bc = state.tile([128, 8, BS], bf, tag="b2bc")
            nc.sync.dma_start(b2bc, b2bc_d[:, :, :])
            bd1bc = state.tile([128, 8, BS], bf, tag="bd1bc")
            nc.sync.dma_start(bd1bc, bd1bc_d[:, :, :])
            bd2col = state.tile([128, 8], fp, tag="bd2c")
            nc.sync.dma_start(bd2col, bd2col_d[:, :])
            wtgt = state.tile([128, 2, DEC, BS], bf, tag="wtgt")
            nc.sync.dma_start(wtgt, wtgt_d[:, :, :, :])

            # ---- persistent activations ----
            h2seqT = state.tile([128, 2, T, BS], bf, tag="h2seq")
            h2decT = state.tile([128, 2, DEC, BS], bf, tag="h2dec")
            A_sb = state.tile([T, BS, H], bf, tag="Asb")
            capgT = state.tile([128, 8, ROWS], bf, tag="capg")
            ce_acc = state.tile([1, 4], fp, tag="cea")

            ALU = mybir.AluOpType

            def lstm_elem_T(gates, c_old, out_h, tg_suffix):
                """gates [128, 8, BS] in chunk order i0 i1 f0 f1 o0 o1 g0 g1.
                Writes h (bf16) to out_h [128, 2, BS]; returns new c tile.
                sigmoid(x) = 0.5*tanh(0.5x) + 0.5 so the only ACT funcs in
                the loops are Tanh/Exp (one shared table, no reloads)."""
                tio = acts.tile([128, 6, BS], fp, tag="tio" + tg_suffix)
                nc.scalar.activation(tio, gates[:, 0:6, :], AF.Tanh, scale=0.5)
                sio = acts.tile([128, 6, BS], fp, tag="sio" + tg_suffix)
                nc.vector.tensor_scalar(sio, tio, 0.5, 0.5, ALU.mult, ALU.add)
                tg = acts.tile([128, 2, BS], fp, tag="tg" + tg_suffix)
                nc.scalar.activation(tg, gates[:, 6:8, :], AF.Tanh)
                t1 = acts.tile([128, 2, BS], fp, tag="t1" + tg_suffix)
                nc.vector.tensor_mul(t1, sio[:, 0:2, :], tg)
                if c_old is None:
                    c_new = t1
                else:
                    cm = acts.tile([128, 2, BS], fp, tag="cm" + tg_suffix)
                    nc.vector.tensor_mul(cm, sio[:, 2:4, :], c_old)
                    c_new = acts.tile([128, 2, BS], fp, tag="c" + tg_suffix)
                    nc.vector.tensor_add(c_new, cm, t1)
                th = acts.tile([128, 2, BS], fp, tag="th" + tg_suffix)
                nc.scalar.activation(th, c_new, AF.Tanh)
                nc.vector.tensor_mul(out_h, sio[:, 4:6, :], th)
                return c_new

            # ================ P1 + encoder (scoped: feat/w1/g1 freed after) ====
            with ExitStack() as p1ctx:
                p1w = p1ctx.enter_context(tc.tile_pool(name="p1w", bufs=1))

                capT = p1w.tile([128, 2, ROWS], bf, tag="capT")
                nc.sync.dma_start(capT, capT_d[:, :, :])
                wd2l = p1w.tile([128, 2, 8, 128], bf, tag="wd2l")
                nc.sync.dma_start(wd2l, wd2l_d[:, :, :, :])
                feat_sb = p1w.tile([128, KF, TB], bf, tag="feat")
                nc.sync.dma_start(feat_sb, featT_d[:, :, :])
                w1sb = p1w.tile([128, 8, KF, 128], bf, tag="w1")
                nc.sync.dma_start(w1sb, w1_d[:, :, :, :])
                g1Ts = [
                    p1w.tile([128, 8, 128], bf, tag=f"g1s{s}", name=f"g1s{s}")
                    for s in range(5)
                ]

                # cap_proj: capgT[:, m, r] = (cap @ d2_Wih_l.T + d2_b) gatesT
                for m in range(8):
                    ps = bigp.tile([128, CSZ], fp, tag="big")
                    for kc in range(2):
                        nc.tensor.matmul(
                            ps[:, 0:ROWS], wd2l[:, kc, m, :], capT[:, kc, :],
                            start=(kc == 0), stop=(kc == 1),
                        )
                    nc.vector.tensor_scalar_add(
                        capgT[:, m, :], ps[:, 0:ROWS], bd2col[:, m : m + 1]
                    )

                # G1 slice builder: 32-k accumulation for (s, m), in halves
                g1ps_box = [None]

                def g1_group(s, m, half):
                    if half == 0:
                        g1ps_box[0] = bigp.tile(
                            [128, CSZ], fp, tag="big", name=f"g1ps{s}_{m}"
                        )
                    ps = g1ps_box[0]
                    for k in range(16 * half, 16 * half + 16):
                        nc.tensor.matmul(
                            ps[:, 0:128],
                            w1sb[:, m, k, :],
                            feat_sb[:, k, ds(128 * s, 128)],
                            start=(k == 0), stop=(k == KF - 1),
                        )
                    if half == 1:
                        nc.vector.tensor_scalar_add(
                            g1Ts[s][:, m, :], ps[:, 0:128], b1col[:, m : m + 1]
                        )

                for m in range(8):  # slice 0 up front
                    g1_group(0, m, 0)
                    g1_group(0, m, 1)

                # ================ P2: encoder ================
                h1T = None
                c1 = None
                c2 = None
                for t in range(T):
                    s = t // 16
                    # ---- cell 1 (addend folded into the psum group) ----
                    if t == 0:
                        gates1 = g1Ts[0][:, :, 0:BS]
                    else:
                        ps = gpp.tile([128, 8, BS], fp, tag="g")
                        nc.tensor.matmul(
                            ps, identb, g1Ts[s][:, :, ds(BS * (t % 16), BS)],
                            start=True, stop=False,
                        )
                        for m in range(8):
                            for kc in range(2):
                                nc.tensor.matmul(
                                    ps[:, m, :], w1hh[:, kc, m, :], h1T[:, kc, :],
                                    start=False,
                                    stop=(m == 7 and kc == 1),
                                )
                        gates1 = ps
                    h1T_new = acts.tile([128, 2, BS], bf, tag="h1")
                    c1 = lstm_elem_T(gates1, c1, h1T_new, "1")
                    h1T = h1T_new
                    # ---- G1 interleave fills the PE gap while cell1's
                    # elementwise chain runs
                    if t < 64:
                        g1_group(1 + t // 16, (t % 16) // 2, t % 2)
                    # ---- cell 2 ----
                    ps2 = gpp.tile([128, 8, BS], fp, tag="g")
                    nc.tensor.matmul(ps2, identb, b2bc, start=True, stop=False)
                    nkc = 2 if t == 0 else 4
                    for m in range(8):
                        for kc in range(nkc):
                            rhs = (
                                h1T[:, kc, :]
                                if kc < 2
                                else h2seqT[:, kc - 2, t - 1, :]
                            )
                            nc.tensor.matmul(
                                ps2[:, m, :], w2[:, kc, m, :], rhs,
                                start=False,
                                stop=(m == 7 and kc == nkc - 1),
                            )
                    c2 = lstm_elem_T(ps2, c2, h2seqT[:, :, t, :], "2")

            # A_sb[te, b, :] = h2seq[b, te, :] (row layout for context matmul)
            for b in range(BS):
                for kc in range(2):
                    pA = attp.tile([T, 128], bf, tag="at")
                    nc.tensor.transpose(pA, h2seqT[:, kc, :, b], identb)
                    nc.vector.tensor_copy(A_sb[:, b, ts(kc, 128)], pA)

            # ================ P4 emitter ================
            wos = ctx.enter_context(tc.tile_pool(name="wos", bufs=3))
            junk = ctx.enter_context(tc.tile_pool(name="junk", bufs=2))
            p4sb = ctx.enter_context(tc.tile_pool(name="p4sb", bufs=2))

            def p4_begin(mi):
                t0 = 16 * mi
                tn = 16 if mi == 0 else DEC - 16
                s_all = p4sb.tile([128, NCH], fp, tag="sall", name=f"sall{mi}")
                return dict(mi=mi, t0=t0, tn=tn, R=tn * BS, s_all=s_all)

            def p4_chunk(st, c):
                t0, tn, R = st["t0"], st["tn"], st["R"]
                wot = wos.tile([128, 2, CSZ], bf, tag="wo")
                nc.sync.dma_start(wot, wo_d[:, :, ts(c, CSZ)])
                bot = wos.tile([1, CSZ], bf, tag="bo")
                nc.sync.dma_start(bot, bo_d[:, ts(c, CSZ)])
                psL = bigp.tile([128, CSZ], fp, tag="big")
                nc.tensor.matmul(
                    psL[:R], ones1x128b[:, :R], bot, start=True, stop=False
                )
                for kc in range(2):
                    nc.tensor.matmul(
                        psL[:R],
                        h2decT[:, kc, t0 : t0 + tn, :],
                        wot[:, kc, :],
                        start=False, stop=(kc == 1),
                    )
                jk = junk.tile([128, CSZ], bf, tag="jk")
                nc.scalar.activation(
                    jk[:R], psL[:R], AF.Exp,
                    accum_out=st["s_all"][:R, c : c + 1],
                )

            def p4_finish(st):
                mi, t0, tn, R = st["mi"], st["t0"], st["tn"], st["R"]
                ssum = p4sb.tile([128, 1], fp, tag="ssum")
                nc.vector.reduce_sum(ssum[:R], st["s_all"][:R], axis=AX.X)
                lse = p4sb.tile([128, 1], fp, tag="lse")
                nc.scalar.activation(lse[:R], ssum[:R], AF.Ln)
                at = attp.tile([128, 168], fp, tag="at")
                nc.tensor.matmul(
                    at[0:1, 33:34], lse[:R], ones128f[:R], start=True, stop=True
                )
                nc.vector.tensor_copy(ce_acc[:, 2 * mi : 2 * mi + 1], at[0:1, 33:34])
                # target-row dot: sum_rows h2dec . w_tgt
                prod = p4sb.tile([128, 2, 16, BS], bf, tag="prod")
                for kc in range(2):
                    nc.vector.tensor_mul(
                        prod[:, kc, 0:tn, :],
                        h2decT[:, kc, t0 : t0 + tn, :],
                        wtgt[:, kc, t0 : t0 + tn, :],
                    )
                for kc in range(2):
                    nc.tensor.matmul(
                        at[0:1, 40 : 40 + R], ones128b, prod[:, kc, 0:tn, :],
                        start=(kc == 0), stop=(kc == 1),
                    )
                ltsum = p4sb.tile([1, 1], fp, tag="lts")
                nc.vector.reduce_sum(ltsum, at[0:1, 40 : 40 + R], axis=AX.X)
                nc.vector.tensor_copy(ce_acc[:, 2 * mi + 1 : 2 * mi + 2], ltsum)

            # ================ P3: decoder ================
            # Software-pipelined: attention for step t-1 is emitted between
            # d2_t's h1-half (kc 0,1) and h2-half (kc 2,3), so d1_t and half
            # of d2_t overlap the previous step's attention chain.
            def attn_step(tq):
                """h2 <- softmax(h2seq . h2dec[tq]) . h2seq, returns h2aT."""
                at = attp.tile([128, 168], fp, tag="at", name=f"at{tq}")
                q = h2decT[:, :, tq, :]
                for b in range(BS):
                    for kc in range(2):
                        nc.tensor.matmul(
                            at[0:T, b : b + 1],
                            h2seqT[:, kc, :, b],
                            q[:, kc, b : b + 1],
                            start=(kc == 0), stop=(kc == 1),
                        )
                expT = smsb.tile([T, BS], bf, tag="exp")
                nc.scalar.activation(expT, at[0:T, 0:BS], AF.Exp)
                nc.tensor.matmul(
                    at[0:1, 32:40], ones80, expT, start=True, stop=True
                )
                recip = smsb.tile([1, BS], fp, tag="rcp")
                nc.vector.reciprocal(recip, at[0:1, 32:40])
                nc.tensor.matmul(
                    at[:, 24:32], ones1x128f, recip, start=True, stop=True
                )
                bcs = smsb.tile([128, BS], bf, tag="bcs")
                nc.vector.tensor_copy(bcs, at[:, 24:32])
                for b in range(BS):
                    for hc in range(2):
                        col = 8 + 8 * hc + b
                        nc.tensor.matmul(
                            at[:, col : col + 1],
                            A_sb[:, b, ts(hc, 128)],
                            expT[:, b : b + 1],
                            start=True, stop=True,
                        )
                h2a = smsb.tile([128, 2, BS], bf, tag="h2a")
                for hc in range(2):
                    nc.vector.tensor_mul(
                        h2a[:, hc, :], at[:, ds(8 + 8 * hc, 8)], bcs
                    )
                return h2a

            p4st = [None, None]
            for t in range(DEC):
                # d1: gates = d1_b + h1 @ d1_Whh.T
                ps = gpp.tile([128, 8, BS], fp, tag="g")
                nc.tensor.matmul(ps, identb, bd1bc, start=True, stop=False)
                for m in range(8):
                    for kc in range(2):
                        nc.tensor.matmul(
                            ps[:, m, :], wd1[:, kc, m, :], h1T[:, kc, :],
                            start=False, stop=(m == 7 and kc == 1),
                        )
                h1T_new = acts.tile([128, 2, BS], bf, tag="h1")
                c1 = lstm_elem_T(ps, c1, h1T_new, "1")
                h1T = h1T_new
                # d2 h1-half: addend + kc 0,1 (independent of attention t-1)
                ps2 = gpp.tile([128, 8, BS], fp, tag="g")
                nc.tensor.matmul(
                    ps2, identb, capgT[:, :, ds(BS * t, BS)],
                    start=True, stop=False,
                )
                for m in range(8):
                    for kc in range(2):
                        nc.tensor.matmul(
                            ps2[:, m, :], wd2[:, kc, m, :], h1T[:, kc, :],
                            start=False, stop=False,
                        )
                # attention for the previous step (produces this step's h2)
                h2rhs = h2seqT[:, :, T - 1, :] if t == 0 else attn_step(t - 1)
                for m in range(8):
                    for kc in range(2, 4):
                        nc.tensor.matmul(
                            ps2[:, m, :], wd2[:, kc, m, :], h2rhs[:, kc - 2, :],
                            start=False, stop=(m == 7 and kc == 3),
                        )
                c2 = lstm_elem_T(ps2, c2, h2decT[:, :, t, :], "2")
                # interleave first CE tile (rows of steps 0..15) two vocab
                # chunks per step once its h2dec rows are complete
                if t == 16:
                    p4st[0] = p4_begin(0)
                if 16 <= t <= 23:
                    p4_chunk(p4st[0], 2 * (t - 16))
                    p4_chunk(p4st[0], 2 * (t - 16) + 1)
                elif t == 24:
                    p4_finish(p4st[0])
            # second CE tile right after the last d2 (its attention is
            # never consumed, so it is skipped entirely)
            p4st[1] = p4_begin(1)
            for c in range(NCH):
                p4_chunk(p4st[1], c)
            p4_finish(p4st[1])

            # final: partial = (lse0 - dot0 + lse1 - dot1) / B^2
            d0 = smsb.tile([1, 1], fp, tag="d0")
            nc.vector.tensor_sub(d0, ce_acc[:, 0:1], ce_acc[:, 1:2])
            d1_ = smsb.tile([1, 1], fp, tag="d1")
            nc.vector.tensor_sub(d1_, ce_acc[:, 2:3], ce_acc[:, 3:4])
            tot = smsb.tile([1, 1], fp, tag="tot")
            nc.vector.tensor_add(tot, d0, d1_)
            outsb = smsb.tile([1, 1], fp, tag="osb")
            nc.scalar.mul(outsb, tot, 1.0 / (B * B))
            nc.sync.dma_start(out_d[:, :], outsb)

    nc.compile()
    return nc


def _shard_inputs(inputs):
    """Host-side relayout + shard. Returns (list of 8 in_maps, host_bias)."""
    import ml_dtypes

    f32 = np.float32
    bft = ml_dtypes.bfloat16
    feat = np.asarray(inputs["feat"], f32)
    caption = np.asarray(inputs["caption"], f32)
    oh = np.asarray(inputs["caption_one_hot"], f32)

    def w(name):
        return np.asarray(inputs[name], f32)

    def wtile(Wt):  # Wt [K, 1024] -> [128, K//128, 8, 128] bf16, m permuted
        Kc = Wt.shape[0] // 128
        a = Wt.reshape(Kc, 128, 8, 128)[:, :, PERM, :]
        return np.ascontiguousarray(a.transpose(1, 0, 2, 3).astype(bft))

    def bcol(bv):  # [1024] -> [128, 8] fp32, m permuted
        return np.ascontiguousarray(bv.reshape(8, 128)[PERM].T.astype(f32))

    def bbc(bv):  # [1024] -> [128, 8, BS] broadcast over batch, bf16
        return np.ascontiguousarray(
            np.repeat(bcol(bv)[:, :, None], BS, axis=2).astype(bft)
        )

    out_W = w("out_W")
    shared = dict(
        w1t=wtile(w("e1_Wih").T),
        w1hht=wtile(w("e1_Whh").T),
        w2t=wtile(np.concatenate([w("e2_Wih")[:, H:], w("e2_Whh")], 1).T),
        wd1t=wtile(w("d1_Whh").T),
        wd2t=wtile(np.concatenate([w("d2_Wih")[:, H:], w("d2_Whh")], 1).T),
        wd2lt=wtile(w("d2_Wih")[:, :H].T),
        b1col=bcol(w("e1_b")),
        b2bc=bbc(w("e2_b")),
        bd1bc=bbc(w("d1_b")),
        bd2col=bcol(w("d2_b")),
        wot=np.ascontiguousarray(
            out_W.T.reshape(2, 128, V).transpose(1, 0, 2).astype(bft)
        ),
        bot=np.ascontiguousarray(w("out_b").reshape(1, V).astype(bft)),
    )

    tgt = np.argmax(oh, axis=2)[:, 1:]  # [B, DEC]
    host_bias = f32(w("out_b")[tgt].sum()) / f32(B * B)

    in_maps = []
    for c in range(NCORES):
        b0 = c * BS
        featT = np.ascontiguousarray(
            feat[b0 : b0 + BS]
            .transpose(2, 1, 0).reshape(KF, 128, TB).transpose(1, 0, 2)
            .astype(bft)
        )
        capT = np.ascontiguousarray(
            caption[b0 : b0 + BS, :DEC]
            .transpose(2, 1, 0).reshape(2, 128, ROWS).transpose(1, 0, 2)
            .astype(bft)
        )
        wt = out_W[tgt[b0 : b0 + BS]]  # [BS, DEC, H]
        wtgt = np.ascontiguousarray(
            wt.transpose(2, 1, 0).reshape(2, 128, DEC, BS)
            .transpose(1, 0, 2, 3).astype(bft)
        )
        m = dict(shared)
        m.update(featT=featT, capT=capT, wtgt=wtgt)
        in_maps.append(m)
    return in_maps, host_bias


def kernel(**inputs):
    from concourse.bass_utils import run_bass_kernel_spmd

    if "nc" not in _cache:
        _cache["nc"] = _build_program()
    nc = _cache["nc"]
    in_maps, host_bias = _shard_inputs(inputs)
    res = run_bass_kernel_spmd(nc, in_maps, core_ids=list(range(NCORES)))
    total = np.float32(0.0)
    for r in res.results:
        total += np.float32(r["partial"][0, 0])
    total -= host_bias
    return np.asarray(total, np.float32)


# revision 15
# speedup vs baseline: 4.2380x; 1.0019x over previous
"""Trainium2 Bass kernel for nn_Net_74259984548321 (video-caption LSTM net).

v2 design (vs v1: all-fp32, row-layout gates, device-side one-hot argmax):
  * all matmuls bf16 (fp32 matmuls cost 4 cycles/row on trn2).
  * recurrence in transposed "gatesT" layout: weight tiles [K=128, M=128]
    stationary, gates/h/c live as [128 gate/h dims, batch] so elementwise
    runs on 128 partitions with tiny free dims (8-64 elems) instead of 8
    partitions x 256-512 elems; h is born in the layout the next matmul
    needs (no per-step PE transposes).
  * biases folded into precomputed addends (g1/cap projections) or constant
    broadcast tiles (one DVE add per cell, no K=1 bias matmuls in the loop).
  * attention: scores -> exp (no max subtraction; scores are O(1)) ->
    unnormalized context + reciprocal-broadcast matmul; no transposes.
  * CE: targets argmax'd on HOST, out_W target rows gathered on HOST;
    device computes full-vocab LSE (streamed out_W, online accum) plus a
    per-row dot with the gathered rows. caption_one_hot never touches HBM.
  * feat projection G1 (the only big GEMM) runs one 16-step slice ahead of
    the encoder, interleaved into the encoder's PE idle gaps.
Per core: 8 batch rows; host sums 8 partial scalars + target-bias term.
"""

import numpy as np

B, T, FEAT, H, V, L = 64, 80, 4096, 256, 8000, 32
DEC = L - 1            # 31 decoder steps
NCORES = 8
BS = B // NCORES       # 8 batch rows per core
G = 4 * H              # 1024 gates
KF = FEAT // 128       # 32 feat contraction chunks
ROWS = DEC * BS        # 248 decoder (t, b) rows per core
NCH = 16               # vocab chunks for LSE
CSZ = V // NCH         # 500
TB = T * BS            # 640 encoder (t, b) rows per core
# gate chunk order used on-chip: i0 i1 f0 f1 o0 o1 g0 g1 (source chunks)
PERM = np.array([0, 1, 2, 3, 6, 7, 4, 5])

_cache = {}


def _build_program():
    import concourse.tile as tile
    from concourse import bacc, mybir
    from concourse.bass import ts, ds
    from concourse.masks import make_identity

    fp = mybir.dt.float32
    bf = mybir.dt.bfloat16
    AF = mybir.ActivationFunctionType
    AX = mybir.AxisListType

    nc = bacc.Bacc(None, target_bir_lowering=False)

    featT_d = nc.dram_tensor("featT", [128, KF, TB], bf, kind="ExternalInput")
    w1_d = nc.dram_tensor("w1t", [128, 8, KF, 128], bf, kind="ExternalInput")
    w1hh_d = nc.dram_tensor("w1hht", [128, 2, 8, 128], bf, kind="ExternalInput")
    w2_d = nc.dram_tensor("w2t", [128, 4, 8, 128], bf, kind="ExternalInput")
    wd1_d = nc.dram_tensor("wd1t", [128, 2, 8, 128], bf, kind="ExternalInput")
    wd2_d = nc.dram_tensor("wd2t", [128, 4, 8, 128], bf, kind="ExternalInput")
    wd2l_d = nc.dram_tensor("wd2lt", [128, 2, 8, 128], bf, kind="ExternalInput")
    b1col_d = nc.dram_tensor("b1col", [128, 8], fp, kind="ExternalInput")
    b2bc_d = nc.dram_tensor("b2bc", [128, 8, BS], bf, kind="ExternalInput")
    bd1bc_d = nc.dram_tensor("bd1bc", [128, 8, BS], bf, kind="ExternalInput")
    bd2col_d = nc.dram_tensor("bd2col", [128, 8], fp, kind="ExternalInput")
    capT_d = nc.dram_tensor("capT", [128, 2, ROWS], bf, kind="ExternalInput")
    wo_d = nc.dram_tensor("wot", [128, 2, V], bf, kind="ExternalInput")
    bo_d = nc.dram_tensor("bot", [1, V], bf, kind="ExternalInput")
    wtgt_d = nc.dram_tensor("wtgt", [128, 2, DEC, BS], bf, kind="ExternalInput")
    out_d = nc.dram_tensor("partial", [1, 1], fp, kind="ExternalOutput")

    with tile.TileContext(nc) as tc:
        from contextlib import ExitStack

        with ExitStack() as ctx:
            const = ctx.enter_context(tc.tile_pool(name="const", bufs=1))
            state = ctx.enter_context(tc.tile_pool(name="state", bufs=1))
            acts = ctx.enter_context(tc.tile_pool(name="acts", bufs=2))
            smsb = ctx.enter_context(tc.tile_pool(name="smsb", bufs=2))
            # psum pools: 3 + 2 + 2 = 7 banks
            gpp = ctx.enter_context(tc.tile_pool(name="gpp", bufs=3, space="PSUM"))
            bigp = ctx.enter_context(tc.tile_pool(name="bigp", bufs=2, space="PSUM"))
            attp = ctx.enter_context(tc.tile_pool(name="attp", bufs=2, space="PSUM"))

            # ---- constants ----
            identb = const.tile([128, 128], bf, tag="idb")
            make_identity(nc, identb)
            ones80 = const.tile([T, 1], bf, tag="o80")
            nc.vector.memset(ones80, 1.0)
            ones1x128b = const.tile([1, 128], bf, tag="o1r")
            nc.vector.memset(ones1x128b, 1.0)
            ones1x128f = const.tile([1, 128], fp, tag="o1rf")
            nc.vector.memset(ones1x128f, 1.0)
            ones128b = const.tile([128, 1], bf, tag="o1c")
            nc.vector.memset(ones128b, 1.0)
            ones128f = const.tile([128, 1], fp, tag="o1cf")
            nc.vector.memset(ones128f, 1.0)

            # ---- persistent weights / addends ----
            w1hh = state.tile([128, 2, 8, 128], bf, tag="w1hh")
            nc.sync.dma_start(w1hh, w1hh_d[:, :, :, :])
            w2 = state.tile([128, 4, 8, 128], bf, tag="w2")
            nc.sync.dma_start(w2, w2_d[:, :, :, :])
            wd1 = state.tile([128, 2, 8, 128], bf, tag="wd1")
            nc.sync.dma_start(wd1, wd1_d[:, :, :, :])
            wd2 = state.tile([128, 4, 8, 128], bf, tag="wd2")
            nc.sync.dma_start(wd2, wd2_d[:, :, :, :])
            b1col = state.tile([128, 8], fp, tag="b1c")
            nc.sync.dma_start(b1col, b1col_d[:, :])
            b2bc = state.tile([128, 8, BS], bf, tag="b2bc")
            nc.sync.dma_start(b2bc, b2bc_d[:, :, :])
            bd1bc = state.tile([128, 8, BS], bf, tag="bd1bc")
            nc.sync.dma_start(bd1bc, bd1bc_d[:, :, :])
            bd2col = state.tile([128, 8], fp, tag="bd2c")
            nc.sync.dma_start(bd2col, bd2col_d[:, :])
            wtgt = state.tile([128, 2, DEC, BS], bf, tag="wtgt")
            nc.sync.dma_start(wtgt, wtgt_d[:, :, :, :])

            # ---- persistent activations ----
            h2seqT = state.tile([128, 2, T, BS], bf, tag="h2seq")
            h2decT = state.tile([128, 2, DEC, BS], bf, tag="h2dec")
            A_sb = state.tile([T, BS, H], bf, tag="Asb")
            capgT = state.tile([128, 8, ROWS], bf, tag="capg")
            ce_acc = state.tile([1, 4], fp, tag="cea")

            ALU = mybir.AluOpType

            def lstm_elem_T(gates, c_old, out_h, tg_suffix):
                """gates [128, 8, BS] in chunk order i0 i1 f0 f1 o0 o1 g0 g1,
                with the i/f/o chunks pre-scaled by 0.5 on the host so a
                single Tanh covers all 8 chunks: sigmoid(x) = 0.5*tanh(x/2)
                + 0.5. Only Tanh/Exp/Copy are used in the loops -> one ACT
                table, no reloads. Writes h (bf16) to out_h [128, 2, BS];
                returns the new c tile."""
                tnh = acts.tile([128, 8, BS], fp, tag="tn" + tg_suffix)
                nc.scalar.activation(tnh, gates, AF.Tanh)
                sio = acts.tile([128, 6, BS], fp, tag="sio" + tg_suffix)
                if tg_suffix == "1":
                    # keep ACT/DVE balanced: cell1's affine on ACT
                    nc.scalar.activation(
                        sio, tnh[:, 0:6, :], AF.Copy, bias=0.5, scale=0.5
                    )
                else:
                    nc.vector.tensor_scalar(
                        sio, tnh[:, 0:6, :], 0.5, 0.5, ALU.mult, ALU.add
                    )
                t1 = acts.tile([128, 2, BS], fp, tag="t1" + tg_suffix)
                nc.vector.tensor_mul(t1, sio[:, 0:2, :], tnh[:, 6:8, :])
                if c_old is None:
                    c_new = t1
                else:
                    cm = acts.tile([128, 2, BS], fp, tag="cm" + tg_suffix)
                    nc.vector.tensor_mul(cm, sio[:, 2:4, :], c_old)
                    c_new = acts.tile([128, 2, BS], fp, tag="c" + tg_suffix)
                    nc.vector.tensor_add(c_new, cm, t1)
                th = acts.tile([128, 2, BS], fp, tag="th" + tg_suffix)
                nc.scalar.activation(th, c_new, AF.Tanh)
                nc.vector.tensor_mul(out_h, sio[:, 4:6, :], th)
                return c_new

            # ================ P1 + encoder (scoped: feat/w1/g1 freed after) ====
            with ExitStack() as p1ctx:
                p1w = p1ctx.enter_context(tc.tile_pool(name="p1w", bufs=1))

                capT = p1w.tile([128, 2, ROWS], bf, tag="capT")
                nc.sync.dma_start(capT, capT_d[:, :, :])
                wd2l = p1w.tile([128, 2, 8, 128], bf, tag="wd2l")
                nc.sync.dma_start(wd2l, wd2l_d[:, :, :, :])
                feat_sb = p1w.tile([128, KF, TB], bf, tag="feat")
                nc.sync.dma_start(feat_sb, featT_d[:, :, :])
                w1sb = p1w.tile([128, 8, KF, 128], bf, tag="w1")
                nc.sync.dma_start(w1sb, w1_d[:, :, :, :])
                g1Ts = [
                    p1w.tile([128, 8, 128], bf, tag=f"g1s{s}", name=f"g1s{s}")
                    for s in range(5)
                ]

                # cap_proj: capgT[:, m, r] = (cap @ d2_Wih_l.T + d2_b) gatesT
                for m in range(8):
                    ps = bigp.tile([128, CSZ], fp, tag="big")
                    for kc in range(2):
                        nc.tensor.matmul(
                            ps[:, 0:ROWS], wd2l[:, kc, m, :], capT[:, kc, :],
                            start=(kc == 0), stop=(kc == 1),
                        )
                    nc.vector.tensor_scalar_add(
                        capgT[:, m, :], ps[:, 0:ROWS], bd2col[:, m : m + 1]
                    )

                # G1 slice builder: 32-k accumulation for (s, m), in halves
                g1ps_box = [None]

                def g1_group(s, m, half):
                    if half == 0:
                        g1ps_box[0] = bigp.tile(
                            [128, CSZ], fp, tag="big", name=f"g1ps{s}_{m}"
                        )
                    ps = g1ps_box[0]
                    for k in range(16 * half, 16 * half + 16):
                        nc.tensor.matmul(
                            ps[:, 0:128],
                            w1sb[:, m, k, :],
                            feat_sb[:, k, ds(128 * s, 128)],
                            start=(k == 0), stop=(k == KF - 1),
                        )
                    if half == 1:
                        nc.vector.tensor_scalar_add(
                            g1Ts[s][:, m, :], ps[:, 0:128], b1col[:, m : m + 1]
                        )

                for m in range(8):  # slice 0 up front
                    g1_group(0, m, 0)
                    g1_group(0, m, 1)

                # ================ P2: encoder ================
                h1T = None
                c1 = None
                c2 = None
                for t in range(T):
                    s = t // 16
                    # ---- cell 1 (addend folded into the psum group) ----
                    if t == 0:
                        gates1 = g1Ts[0][:, :, 0:BS]
                    else:
                        ps = gpp.tile([128, 8, BS], fp, tag="g")
                        nc.tensor.matmul(
                            ps, identb, g1Ts[s][:, :, ds(BS * (t % 16), BS)],
                            start=True, stop=False,
                        )
                        for m in range(8):
                            for kc in range(2):
                                nc.tensor.matmul(
                                    ps[:, m, :], w1hh[:, kc, m, :], h1T[:, kc, :],
                                    start=False,
                                    stop=(m == 7 and kc == 1),
                                )
                        gates1 = ps
                    h1T_new = acts.tile([128, 2, BS], bf, tag="h1")
                    c1 = lstm_elem_T(gates1, c1, h1T_new, "1")
                    h1T = h1T_new
                    # ---- G1 interleave fills the PE gap while cell1's
                    # elementwise chain runs
                    if t < 64:
                        g1_group(1 + t // 16, (t % 16) // 2, t % 2)
                    # ---- cell 2 ----
                    ps2 = gpp.tile([128, 8, BS], fp, tag="g")
                    nc.tensor.matmul(ps2, identb, b2bc, start=True, stop=False)
                    nkc = 2 if t == 0 else 4
                    for m in range(8):
                        for kc in range(nkc):
                            rhs = (
                                h1T[:, kc, :]
                                if kc < 2
                                else h2seqT[:, kc - 2, t - 1, :]
                            )
                            nc.tensor.matmul(
                                ps2[:, m, :], w2[:, kc, m, :], rhs,
                                start=False,
                                stop=(m == 7 and kc == nkc - 1),
                            )
                    c2 = lstm_elem_T(ps2, c2, h2seqT[:, :, t, :], "2")

            # A_sb[te, b, :] = h2seq[b, te, :] (row layout for context matmul)
            for b in range(BS):
                for kc in range(2):
                    pA = attp.tile([T, 128], bf, tag="at")
                    nc.tensor.transpose(pA, h2seqT[:, kc, :, b], identb)
                    nc.vector.tensor_copy(A_sb[:, b, ts(kc, 128)], pA)

            # ================ P4 emitter ================
            wos = ctx.enter_context(tc.tile_pool(name="wos", bufs=3))
            junk = ctx.enter_context(tc.tile_pool(name="junk", bufs=2))
            p4sb = ctx.enter_context(tc.tile_pool(name="p4sb", bufs=2))

            def p4_begin(mi):
                t0 = 16 * mi
                tn = 16 if mi == 0 else DEC - 16
                s_all = p4sb.tile([128, NCH], fp, tag="sall", name=f"sall{mi}")
                return dict(mi=mi, t0=t0, tn=tn, R=tn * BS, s_all=s_all)

            def p4_chunk(st, c):
                t0, tn, R = st["t0"], st["tn"], st["R"]
                wot = wos.tile([128, 2, CSZ], bf, tag="wo")
                nc.sync.dma_start(wot, wo_d[:, :, ts(c, CSZ)])
                bot = wos.tile([1, CSZ], bf, tag="bo")
                nc.sync.dma_start(bot, bo_d[:, ts(c, CSZ)])
                psL = bigp.tile([128, CSZ], fp, tag="big")
                nc.tensor.matmul(
                    psL[:R], ones1x128b[:, :R], bot, start=True, stop=False
                )
                for kc in range(2):
                    nc.tensor.matmul(
                        psL[:R],
                        h2decT[:, kc, t0 : t0 + tn, :],
                        wot[:, kc, :],
                        start=False, stop=(kc == 1),
                    )
                jk = junk.tile([128, CSZ], bf, tag="jk")
                nc.scalar.activation(
                    jk[:R], psL[:R], AF.Exp,
                    accum_out=st["s_all"][:R, c : c + 1],
                )

            def p4_finish(st):
                mi, t0, tn, R = st["mi"], st["t0"], st["tn"], st["R"]
                ssum = p4sb.tile([128, 1], fp, tag="ssum")
                nc.vector.reduce_sum(ssum[:R], st["s_all"][:R], axis=AX.X)
                lse = p4sb.tile([128, 1], fp, tag="lse")
                nc.scalar.activation(lse[:R], ssum[:R], AF.Ln)
                at = attp.tile([128, 168], fp, tag="at")
                nc.tensor.matmul(
                    at[0:1, 33:34], lse[:R], ones128f[:R], start=True, stop=True
                )
                nc.vector.tensor_copy(ce_acc[:, 2 * mi : 2 * mi + 1], at[0:1, 33:34])
                # target-row dot: sum_rows h2dec . w_tgt
                prod = p4sb.tile([128, 2, 16, BS], bf, tag="prod")
                for kc in range(2):
                    nc.vector.tensor_mul(
                        prod[:, kc, 0:tn, :],
                        h2decT[:, kc, t0 : t0 + tn, :],
                        wtgt[:, kc, t0 : t0 + tn, :],
                    )
                for kc in range(2):
                    nc.tensor.matmul(
                        at[0:1, 40 : 40 + R], ones128b, prod[:, kc, 0:tn, :],
                        start=(kc == 0), stop=(kc == 1),
                    )
                ltsum = p4sb.tile([1, 1], fp, tag="lts")
                nc.vector.reduce_sum(ltsum, at[0:1, 40 : 40 + R], axis=AX.X)
                nc.vector.tensor_copy(ce_acc[:, 2 * mi + 1 : 2 * mi + 2], ltsum)

            # ================ P3: decoder ================
            # Software-pipelined: attention for step t-1 is emitted between
            # d2_t's h1-half (kc 0,1) and h2-half (kc 2,3), so d1_t and half
            # of d2_t overlap the previous step's attention chain.
            def attn_step(tq):
                """h2 <- softmax(h2seq . h2dec[tq]) . h2seq, returns h2aT."""
                at = attp.tile([128, 168], fp, tag="at", name=f"at{tq}")
                q = h2decT[:, :, tq, :]
                for b in range(BS):
                    for kc in range(2):
                        nc.tensor.matmul(
                            at[0:T, b : b + 1],
                            h2seqT[:, kc, :, b],
                            q[:, kc, b : b + 1],
                            start=(kc == 0), stop=(kc == 1),
                        )
                expT = smsb.tile([T, BS], bf, tag="exp")
                nc.scalar.activation(expT, at[0:T, 0:BS], AF.Exp)
                nc.tensor.matmul(
                    at[0:1, 32:40], ones80, expT, start=True, stop=True
                )
                recip = smsb.tile([1, BS], fp, tag="rcp")
                nc.vector.reciprocal(recip, at[0:1, 32:40])
                nc.tensor.matmul(
                    at[:, 24:32], ones1x128f, recip, start=True, stop=True
                )
                bcs = smsb.tile([128, BS], bf, tag="bcs")
                nc.vector.tensor_copy(bcs, at[:, 24:32])
                for b in range(BS):
                    for hc in range(2):
                        col = 8 + 8 * hc + b
                        nc.tensor.matmul(
                            at[:, col : col + 1],
                            A_sb[:, b, ts(hc, 128)],
                            expT[:, b : b + 1],
                            start=True, stop=True,
                        )
                h2a = smsb.tile([128, 2, BS], bf, tag="h2a")
                for hc in range(2):
                    nc.vector.tensor_mul(
                        h2a[:, hc, :], at[:, ds(8 + 8 * hc, 8)], bcs
                    )
                return h2a

            p4st = [None, None]
            for t in range(DEC):
                # d1: gates = d1_b + h1 @ d1_Whh.T
                ps = gpp.tile([128, 8, BS], fp, tag="g")
                nc.tensor.matmul(ps, identb, bd1bc, start=True, stop=False)
                for m in range(8):
                    for kc in range(2):
                        nc.tensor.matmul(
                            ps[:, m, :], wd1[:, kc, m, :], h1T[:, kc, :],
                            start=False, stop=(m == 7 and kc == 1),
                        )
                h1T_new = acts.tile([128, 2, BS], bf, tag="h1")
                c1 = lstm_elem_T(ps, c1, h1T_new, "1")
                h1T = h1T_new
                # d2 h1-half: addend + kc 0,1 (independent of attention t-1)
                ps2 = gpp.tile([128, 8, BS], fp, tag="g")
                nc.tensor.matmul(
                    ps2, identb, capgT[:, :, ds(BS * t, BS)],
                    start=True, stop=False,
                )
                for m in range(8):
                    for kc in range(2):
                        nc.tensor.matmul(
                            ps2[:, m, :], wd2[:, kc, m, :], h1T[:, kc, :],
                            start=False, stop=False,
                        )
                # attention for the previous step (produces this step's h2)
                h2rhs = h2seqT[:, :, T - 1, :] if t == 0 else attn_step(t - 1)
                for m in range(8):
                    for kc in range(2, 4):
                        nc.tensor.matmul(
                            ps2[:, m, :], wd2[:, kc, m, :], h2rhs[:, kc - 2, :],
                            start=False, stop=(m == 7 and kc == 3),
                        )
                c2 = lstm_elem_T(ps2, c2, h2decT[:, :, t, :], "2")
                # interleave first CE tile (rows of steps 0..15) two vocab
                # chunks per step once its h2dec rows are complete
                if t == 16:
                    p4st[0] = p4_begin(0)
                if 16 <= t <= 23:
                    p4_chunk(p4st[0], 2 * (t - 16))
                    p4_chunk(p4st[0], 2 * (t - 16) + 1)
                elif t == 24:
                    p4_finish(p4st[0])
            # second CE tile right after the last d2 (its attention is
            # never consumed, so it is skipped entirely)
            p4st[1] = p4_begin(1)
            for c in range(NCH):
                p4_chunk(p4st[1], c)
            p4_finish(p4st[1])

            # final: partial = (lse0 - dot0 + lse1 - dot1) / B^2
            d0 = smsb.tile([1, 1], fp, tag="d0")
            nc.vector.tensor_sub(d0, ce_acc[:, 0:1], ce_acc[:, 1:2])
            d1_ = smsb.tile([1, 1], fp, tag="d1")
            nc.vector.tensor_sub(d1_, ce_acc[:, 2:3], ce_acc[:, 3:4])
            tot = smsb.tile([1, 1], fp, tag="tot")
            nc.vector.tensor_add(tot, d0, d1_)
            outsb = smsb.tile([1, 1], fp, tag="osb")
            nc.scalar.mul(outsb, tot, 1.0 / (B * B))
            nc.sync.dma_start(out_d[:, :], outsb)

    nc.compile()
    return nc


def _shard_inputs(inputs):
    """Host-side relayout + shard. Returns (list of 8 in_maps, host_bias)."""
    import ml_dtypes

    f32 = np.float32
    bft = ml_dtypes.bfloat16
    feat = np.asarray(inputs["feat"], f32)
    caption = np.asarray(inputs["caption"], f32)
    oh = np.asarray(inputs["caption_one_hot"], f32)

    def w(name):
        return np.asarray(inputs[name], f32)

    # i/f/o gate chunks (first 6 in PERM order) pre-scaled by 0.5 so the
    # kernel computes sigmoid as 0.5*tanh(scaled_gates)+0.5 with a single
    # full-width Tanh
    scale_m = np.array([0.5] * 6 + [1.0] * 2, f32)

    def wtile(Wt):  # Wt [K, 1024] -> [128, K//128, 8, 128] bf16, m permuted
        Kc = Wt.shape[0] // 128
        a = Wt.reshape(Kc, 128, 8, 128)[:, :, PERM, :]
        a = a * scale_m[None, None, :, None]
        return np.ascontiguousarray(a.transpose(1, 0, 2, 3).astype(bft))

    def bcol(bv):  # [1024] -> [128, 8] fp32, m permuted + scaled
        a = bv.reshape(8, 128)[PERM].T * scale_m[None, :]
        return np.ascontiguousarray(a.astype(f32))

    def bbc(bv):  # [1024] -> [128, 8, BS] broadcast over batch, bf16
        return np.ascontiguousarray(
            np.repeat(bcol(bv)[:, :, None], BS, axis=2).astype(bft)
        )

    out_W = w("out_W")
    shared = dict(
        w1t=wtile(w("e1_Wih").T),
        w1hht=wtile(w("e1_Whh").T),
        w2t=wtile(np.concatenate([w("e2_Wih")[:, H:], w("e2_Whh")], 1).T),
        wd1t=wtile(w("d1_Whh").T),
        wd2t=wtile(np.concatenate([w("d2_Wih")[:, H:], w("d2_Whh")], 1).T),
        wd2lt=wtile(w("d2_Wih")[:, :H].T),
        b1col=bcol(w("e1_b")),
        b2bc=bbc(w("e2_b")),
        bd1bc=bbc(w("d1_b")),
        bd2col=bcol(w("d2_b")),
        wot=np.ascontiguousarray(
            out_W.T.reshape(2, 128, V).transpose(1, 0, 2).astype(bft)
        ),
        bot=np.ascontiguousarray(w("out_b").reshape(1, V).astype(bft)),
    )

    tgt = np.argmax(oh, axis=2)[:, 1:]  # [B, DEC]
    host_bias = f32(w("out_b")[tgt].sum()) / f32(B * B)

    in_maps = []
    for c in range(NCORES):
        b0 = c * BS
        featT = np.ascontiguousarray(
            feat[b0 : b0 + BS]
            .transpose(2, 1, 0).reshape(KF, 128, TB).transpose(1, 0, 2)
            .astype(bft)
        )
        capT = np.ascontiguousarray(
            caption[b0 : b0 + BS, :DEC]
            .transpose(2, 1, 0).reshape(2, 128, ROWS).transpose(1, 0, 2)
            .astype(bft)
        )
        wt = out_W[tgt[b0 : b0 + BS]]  # [BS, DEC, H]
        wtgt = np.ascontiguousarray(
            wt.transpose(2, 1, 0).reshape(2, 128, DEC, BS)
            .transpose(1, 0, 2, 3).astype(bft)
        )
        m = dict(shared)
        m.update(featT=featT, capT=capT, wtgt=wtgt)
        in_maps.append(m)
    return in_maps, host_bias


def kernel(**inputs):
    from concourse.bass_utils import run_bass_kernel_spmd

    if "nc" not in _cache:
        _cache["nc"] = _build_program()
    nc = _cache["nc"]
    in_maps, host_bias = _shard_inputs(inputs)
    res = run_bass_kernel_spmd(nc, in_maps, core_ids=list(range(NCORES)))
    total = np.float32(0.0)
    for r in res.results:
        total += np.float32(r["partial"][0, 0])
    total -= host_bias
    return np.asarray(total, np.float32)


# revision 19
# speedup vs baseline: 4.3954x; 1.0371x over previous
"""Trainium2 Bass kernel for nn_Net_74259984548321 (video-caption LSTM net).

v2 design (vs v1: all-fp32, row-layout gates, device-side one-hot argmax):
  * all matmuls bf16 (fp32 matmuls cost 4 cycles/row on trn2).
  * recurrence in transposed "gatesT" layout: weight tiles [K=128, M=128]
    stationary, gates/h/c live as [128 gate/h dims, batch] so elementwise
    runs on 128 partitions with tiny free dims (8-64 elems) instead of 8
    partitions x 256-512 elems; h is born in the layout the next matmul
    needs (no per-step PE transposes).
  * biases folded into precomputed addends (g1/cap projections) or constant
    broadcast tiles (one DVE add per cell, no K=1 bias matmuls in the loop).
  * attention: scores -> exp (no max subtraction; scores are O(1)) ->
    unnormalized context + reciprocal-broadcast matmul; no transposes.
  * CE: targets argmax'd on HOST, out_W target rows gathered on HOST;
    device computes full-vocab LSE (streamed out_W, online accum) plus a
    per-row dot with the gathered rows. caption_one_hot never touches HBM.
  * feat projection G1 (the only big GEMM) runs one 16-step slice ahead of
    the encoder, interleaved into the encoder's PE idle gaps.
Per core: 8 batch rows; host sums 8 partial scalars + target-bias term.
"""

import numpy as np

B, T, FEAT, H, V, L = 64, 80, 4096, 256, 8000, 32
DEC = L - 1            # 31 decoder steps
NCORES = 8
BS = B // NCORES       # 8 batch rows per core
G = 4 * H              # 1024 gates
KF = FEAT // 128       # 32 feat contraction chunks
ROWS = DEC * BS        # 248 decoder (t, b) rows per core
NCH = 16               # vocab chunks for LSE
CSZ = V // NCH         # 500
TB = T * BS            # 640 encoder (t, b) rows per core
# gate chunk order used on-chip: i0 i1 f0 f1 o0 o1 g0 g1 (source chunks)
PERM = np.array([0, 1, 2, 3, 6, 7, 4, 5])

_cache = {}


def _build_program():
    import concourse.tile as tile
    from concourse import bacc, mybir
    from concourse.bass import ts, ds
    from concourse.masks import make_identity

    fp = mybir.dt.float32
    bf = mybir.dt.bfloat16
    AF = mybir.ActivationFunctionType
    AX = mybir.AxisListType

    nc = bacc.Bacc(None, target_bir_lowering=False)

    featT_d = nc.dram_tensor("featT", [128, KF, TB], bf, kind="ExternalInput")
    w1_d = nc.dram_tensor("w1t", [128, 8, KF, 128], bf, kind="ExternalInput")
    w1hh_d = nc.dram_tensor("w1hht", [128, 2, 8, 128], bf, kind="ExternalInput")
    w2_d = nc.dram_tensor("w2t", [128, 4, 8, 128], bf, kind="ExternalInput")
    wd1_d = nc.dram_tensor("wd1t", [128, 2, 8, 128], bf, kind="ExternalInput")
    wd2_d = nc.dram_tensor("wd2t", [128, 4, 8, 128], bf, kind="ExternalInput")
    wd2l_d = nc.dram_tensor("wd2lt", [128, 2, 8, 128], bf, kind="ExternalInput")
    b1col_d = nc.dram_tensor("b1col", [128, 8], fp, kind="ExternalInput")
    b2bc_d = nc.dram_tensor("b2bc", [128, 8, BS], bf, kind="ExternalInput")
    bd1bc_d = nc.dram_tensor("bd1bc", [128, 8, BS], bf, kind="ExternalInput")
    bd2col_d = nc.dram_tensor("bd2col", [128, 8], fp, kind="ExternalInput")
    capT_d = nc.dram_tensor("capT", [128, 2, ROWS], bf, kind="ExternalInput")
    wo_d = nc.dram_tensor("wot", [128, 2, V], bf, kind="ExternalInput")
    bo_d = nc.dram_tensor("bot", [1, V], bf, kind="ExternalInput")
    wtgt_d = nc.dram_tensor("wtgt", [128, 2, DEC, BS], bf, kind="ExternalInput")
    out_d = nc.dram_tensor("partial", [1, 1], fp, kind="ExternalOutput")

    with tile.TileContext(nc) as tc:
        from contextlib import ExitStack

        with ExitStack() as ctx:
            const = ctx.enter_context(tc.tile_pool(name="const", bufs=1))
            state = ctx.enter_context(tc.tile_pool(name="state", bufs=1))
            acts = ctx.enter_context(tc.tile_pool(name="acts", bufs=2))
            hpool = ctx.enter_context(tc.tile_pool(name="hp", bufs=3))
            smsb = ctx.enter_context(tc.tile_pool(name="smsb", bufs=2))
            # psum pools: 3 + 2 + 2 = 7 banks
            gpp = ctx.enter_context(tc.tile_pool(name="gpp", bufs=3, space="PSUM"))
            bigp = ctx.enter_context(tc.tile_pool(name="bigp", bufs=2, space="PSUM"))
            attp = ctx.enter_context(tc.tile_pool(name="attp", bufs=2, space="PSUM"))

            # ---- constants ----
            identb = const.tile([128, 128], bf, tag="idb")
            make_identity(nc, identb)
            ones80 = const.tile([T, 1], bf, tag="o80")
            nc.vector.memset(ones80, 1.0)
            ones1x128b = const.tile([1, 128], bf, tag="o1r")
            nc.vector.memset(ones1x128b, 1.0)
            ones1x128f = const.tile([1, 128], fp, tag="o1rf")
            nc.vector.memset(ones1x128f, 1.0)
            ones128b = const.tile([128, 1], bf, tag="o1c")
            nc.vector.memset(ones128b, 1.0)
            ones128f = const.tile([128, 1], fp, tag="o1cf")
            nc.vector.memset(ones128f, 1.0)

            # ---- persistent weights / addends ----
            w1hh = state.tile([128, 2, 8, 128], bf, tag="w1hh")
            nc.sync.dma_start(w1hh, w1hh_d[:, :, :, :])
            w2 = state.tile([128, 4, 8, 128], bf, tag="w2")
            nc.sync.dma_start(w2, w2_d[:, :, :, :])
            wd1 = state.tile([128, 2, 8, 128], bf, tag="wd1")
            nc.sync.dma_start(wd1, wd1_d[:, :, :, :])
            wd2 = state.tile([128, 4, 8, 128], bf, tag="wd2")
            nc.sync.dma_start(wd2, wd2_d[:, :, :, :])
            b1col = state.tile([128, 8], fp, tag="b1c")
            nc.sync.dma_start(b1col, b1col_d[:, :])
            b2bc = state.tile([128, 8, BS], bf, tag="b2bc")
            nc.sync.dma_start(b2bc, b2bc_d[:, :, :])
            bd1bc = state.tile([128, 8, BS], bf, tag="bd1bc")
            nc.sync.dma_start(bd1bc, bd1bc_d[:, :, :])
            bd2col = state.tile([128, 8], fp, tag="bd2c")
            nc.sync.dma_start(bd2col, bd2col_d[:, :])
            wtgt = state.tile([128, 2, DEC, BS], bf, tag="wtgt")
            nc.sync.dma_start(wtgt, wtgt_d[:, :, :, :])

            # ---- persistent activations ----
            h2seqT = state.tile([128, 2, T, BS], bf, tag="h2seq")
            h2decT = state.tile([128, 2, DEC, BS], bf, tag="h2dec")
            A_sb = state.tile([T, BS, H], bf, tag="Asb")
            capgT = state.tile([128, 8, ROWS], bf, tag="capg")
            ce_acc = state.tile([1, 4], fp, tag="cea")

            ALU = mybir.AluOpType

            def lstm_elem_T(gates, c_old, out_h, tg_suffix):
                """gates [128, 8, BS] in chunk order i0 i1 f0 f1 o0 o1 g0 g1,
                with the i/f/o chunks pre-scaled by 0.5 on the host so a
                single Tanh covers all 8 chunks: sigmoid(x) = 0.5*tanh(x/2)
                + 0.5. Only Tanh/Exp/Copy are used in the loops -> one ACT
                table, no reloads. Writes h (bf16) to out_h [128, 2, BS];
                returns the new c tile."""
                tnh = acts.tile([128, 8, BS], fp, tag="tn" + tg_suffix)
                nc.scalar.activation(tnh, gates, AF.Tanh)
                sio = acts.tile([128, 6, BS], fp, tag="sio" + tg_suffix)
                if tg_suffix == "1":
                    # keep ACT/DVE balanced: cell1's affine on ACT
                    nc.scalar.activation(
                        sio, tnh[:, 0:6, :], AF.Copy, bias=0.5, scale=0.5
                    )
                else:
                    nc.vector.tensor_scalar(
                        sio, tnh[:, 0:6, :], 0.5, 0.5, ALU.mult, ALU.add
                    )
                t1 = acts.tile([128, 2, BS], fp, tag="t1" + tg_suffix)
                nc.vector.tensor_mul(t1, sio[:, 0:2, :], tnh[:, 6:8, :])
                if c_old is None:
                    c_new = t1
                else:
                    cm = acts.tile([128, 2, BS], fp, tag="cm" + tg_suffix)
                    nc.vector.tensor_mul(cm, sio[:, 2:4, :], c_old)
                    c_new = acts.tile([128, 2, BS], fp, tag="c" + tg_suffix)
                    nc.vector.tensor_add(c_new, cm, t1)
                th = acts.tile([128, 2, BS], fp, tag="th" + tg_suffix)
                nc.scalar.activation(th, c_new, AF.Tanh)
                nc.vector.tensor_mul(out_h, sio[:, 4:6, :], th)
                return c_new

            # ================ P1 + encoder (scoped: feat/w1/g1 freed after) ====
            with ExitStack() as p1ctx:
                p1w = p1ctx.enter_context(tc.tile_pool(name="p1w", bufs=1))

                capT = p1w.tile([128, 2, ROWS], bf, tag="capT")
                nc.sync.dma_start(capT, capT_d[:, :, :])
                wd2l = p1w.tile([128, 2, 8, 128], bf, tag="wd2l")
                nc.sync.dma_start(wd2l, wd2l_d[:, :, :, :])
                feat_sb = p1w.tile([128, KF, TB], bf, tag="feat")
                nc.sync.dma_start(feat_sb, featT_d[:, :, :])
                w1sb = p1w.tile([128, 8, KF, 128], bf, tag="w1")
                nc.sync.dma_start(w1sb, w1_d[:, :, :, :])
                g1Ts = [
                    p1w.tile([128, 8, 128], bf, tag=f"g1s{s}", name=f"g1s{s}")
                    for s in range(5)
                ]

                # cap_proj: capgT[:, m, r] = (cap @ d2_Wih_l.T + d2_b) gatesT
                for m in range(8):
                    ps = bigp.tile([128, CSZ], fp, tag="big")
                    for kc in range(2):
                        nc.tensor.matmul(
                            ps[:, 0:ROWS], wd2l[:, kc, m, :], capT[:, kc, :],
                            start=(kc == 0), stop=(kc == 1),
                        )
                    nc.vector.tensor_scalar_add(
                        capgT[:, m, :], ps[:, 0:ROWS], bd2col[:, m : m + 1]
                    )

                # G1 slice builder: 32-k accumulation for (s, m), in halves
                g1ps_box = [None]

                def g1_group(s, m, half):
                    if half == 0:
                        g1ps_box[0] = bigp.tile(
                            [128, CSZ], fp, tag="big", name=f"g1ps{s}_{m}"
                        )
                    ps = g1ps_box[0]
                    for k in range(16 * half, 16 * half + 16):
                        nc.tensor.matmul(
                            ps[:, 0:128],
                            w1sb[:, m, k, :],
                            feat_sb[:, k, ds(128 * s, 128)],
                            start=(k == 0), stop=(k == KF - 1),
                        )
                    if half == 1:
                        nc.vector.tensor_scalar_add(
                            g1Ts[s][:, m, :], ps[:, 0:128], b1col[:, m : m + 1]
                        )

                for m in range(8):  # slice 0 up front
                    g1_group(0, m, 0)
                    g1_group(0, m, 1)

                # ================ P2: encoder ================
                # cell2 is emitted one iteration late so cell1_{t+1}'s
                # matmuls+chain sit ahead of cell2_t in the PE stream and
                # start as soon as h1_t is ready (mid-ladder), instead of
                # the whole step serializing burst -> ladder -> burst.
                h1T = None
                h1T_prev = None
                c1 = None
                c2 = None

                def enc_cell2(tm):
                    nonlocal c2
                    ps2 = gpp.tile(
                        [128, 8, BS], fp, tag="g", name=f"c2ps{tm}"
                    )
                    nc.tensor.matmul(ps2, identb, b2bc, start=True, stop=False)
                    nkc = 2 if tm == 0 else 4
                    for m in range(8):
                        for kc in range(nkc):
                            rhs = (
                                h1T_prev[:, kc, :]
                                if kc < 2
                                else h2seqT[:, kc - 2, tm - 1, :]
                            )
                            nc.tensor.matmul(
                                ps2[:, m, :], w2[:, kc, m, :], rhs,
                                start=False,
                                stop=(m == 7 and kc == nkc - 1),
                            )
                    c2 = lstm_elem_T(ps2, c2, h2seqT[:, :, tm, :], "2")

                for t in range(T):
                    s = t // 16
                    # ---- cell 1 (addend folded into the psum group) ----
                    if t == 0:
                        gates1 = g1Ts[0][:, :, 0:BS]
                    else:
                        ps = gpp.tile([128, 8, BS], fp, tag="g")
                        nc.tensor.matmul(
                            ps, identb, g1Ts[s][:, :, ds(BS * (t % 16), BS)],
                            start=True, stop=False,
                        )
                        for m in range(8):
                            for kc in range(2):
                                nc.tensor.matmul(
                                    ps[:, m, :], w1hh[:, kc, m, :], h1T[:, kc, :],
                                    start=False,
                                    stop=(m == 7 and kc == 1),
                                )
                        gates1 = ps
                    h1T_new = hpool.tile([128, 2, BS], bf, tag="h1")
                    c1 = lstm_elem_T(gates1, c1, h1T_new, "1")
                    h1T_prev = h1T
                    h1T = h1T_new
                    # ---- G1 interleave fills the PE gap while cell1's
                    # elementwise chain runs
                    if t < 64:
                        g1_group(1 + t // 16, (t % 16) // 2, t % 2)
                    # ---- cell 2 for the previous step ----
                    if t >= 1:
                        enc_cell2(t - 1)
                h1T_prev = h1T
                enc_cell2(T - 1)

            # A_sb[te, b, :] = h2seq[b, te, :] (row layout for context matmul)
            for b in range(BS):
                for kc in range(2):
                    pA = attp.tile([T, 128], bf, tag="at")
                    nc.tensor.transpose(pA, h2seqT[:, kc, :, b], identb)
                    nc.vector.tensor_copy(A_sb[:, b, ts(kc, 128)], pA)

            # ================ P4 emitter ================
            wos = ctx.enter_context(tc.tile_pool(name="wos", bufs=3))
            junk = ctx.enter_context(tc.tile_pool(name="junk", bufs=2))
            p4sb = ctx.enter_context(tc.tile_pool(name="p4sb", bufs=2))

            def p4_begin(mi):
                t0 = 16 * mi
                tn = 16 if mi == 0 else DEC - 16
                s_all = p4sb.tile([128, NCH], fp, tag="sall", name=f"sall{mi}")
                return dict(mi=mi, t0=t0, tn=tn, R=tn * BS, s_all=s_all)

            def p4_chunk(st, c):
                t0, tn, R = st["t0"], st["tn"], st["R"]
                wot = wos.tile([128, 2, CSZ], bf, tag="wo")
                nc.sync.dma_start(wot, wo_d[:, :, ts(c, CSZ)])
                bot = wos.tile([1, CSZ], bf, tag="bo")
                nc.sync.dma_start(bot, bo_d[:, ts(c, CSZ)])
                psL = bigp.tile([128, CSZ], fp, tag="big")
                nc.tensor.matmul(
                    psL[:R], ones1x128b[:, :R], bot, start=True, stop=False
                )
                for kc in range(2):
                    nc.tensor.matmul(
                        psL[:R],
                        h2decT[:, kc, t0 : t0 + tn, :],
                        wot[:, kc, :],
                        start=False, stop=(kc == 1),
                    )
                jk = junk.tile([128, CSZ], bf, tag="jk")
                nc.scalar.activation(
                    jk[:R], psL[:R], AF.Exp,
                    accum_out=st["s_all"][:R, c : c + 1],
                )

            def p4_finish(st):
                mi, t0, tn, R = st["mi"], st["t0"], st["tn"], st["R"]
                ssum = p4sb.tile([128, 1], fp, tag="ssum")
                nc.vector.reduce_sum(ssum[:R], st["s_all"][:R], axis=AX.X)
                lse = p4sb.tile([128, 1], fp, tag="lse")
                nc.scalar.activation(lse[:R], ssum[:R], AF.Ln)
                at = attp.tile([128, 168], fp, tag="at")
                nc.tensor.matmul(
                    at[0:1, 33:34], lse[:R], ones128f[:R], start=True, stop=True
                )
                nc.vector.tensor_copy(ce_acc[:, 2 * mi : 2 * mi + 1], at[0:1, 33:34])
                # target-row dot: sum_rows h2dec . w_tgt
                prod = p4sb.tile([128, 2, 16, BS], bf, tag="prod")
                for kc in range(2):
                    nc.vector.tensor_mul(
                        prod[:, kc, 0:tn, :],
                        h2decT[:, kc, t0 : t0 + tn, :],
                        wtgt[:, kc, t0 : t0 + tn, :],
                    )
                for kc in range(2):
                    nc.tensor.matmul(
                        at[0:1, 40 : 40 + R], ones128b, prod[:, kc, 0:tn, :],
                        start=(kc == 0), stop=(kc == 1),
                    )
                ltsum = p4sb.tile([1, 1], fp, tag="lts")
                nc.vector.reduce_sum(ltsum, at[0:1, 40 : 40 + R], axis=AX.X)
                nc.vector.tensor_copy(ce_acc[:, 2 * mi + 1 : 2 * mi + 2], ltsum)

            # ================ P3: decoder ================
            # Software-pipelined: attention for step t-1 is emitted between
            # d2_t's h1-half (kc 0,1) and h2-half (kc 2,3), so d1_t and half
            # of d2_t overlap the previous step's attention chain.
            def attn_step(tq):
                """h2 <- softmax(h2seq . h2dec[tq]) . h2seq, returns h2aT."""
                at = attp.tile([128, 168], fp, tag="at", name=f"at{tq}")
                q = h2decT[:, :, tq, :]
                for b in range(BS):
                    for kc in range(2):
                        nc.tensor.matmul(
                            at[0:T, b : b + 1],
                            h2seqT[:, kc, :, b],
                            q[:, kc, b : b + 1],
                            start=(kc == 0), stop=(kc == 1),
                        )
                expT = smsb.tile([T, BS], bf, tag="exp")
                nc.scalar.activation(expT, at[0:T, 0:BS], AF.Exp)
                nc.tensor.matmul(
                    at[0:1, 32:40], ones80, expT, start=True, stop=True
                )
                recip = smsb.tile([1, BS], fp, tag="rcp")
                nc.vector.reciprocal(recip, at[0:1, 32:40])
                nc.tensor.matmul(
                    at[:, 24:32], ones1x128f, recip, start=True, stop=True
                )
                bcs = smsb.tile([128, BS], bf, tag="bcs")
                nc.vector.tensor_copy(bcs, at[:, 24:32])
                for b in range(BS):
                    for hc in range(2):
                        col = 8 + 8 * hc + b
                        nc.tensor.matmul(
                            at[:, col : col + 1],
                            A_sb[:, b, ts(hc, 128)],
                            expT[:, b : b + 1],
                            start=True, stop=True,
                        )
                h2a = smsb.tile([128, 2, BS], bf, tag="h2a")
                for hc in range(2):
                    nc.vector.tensor_mul(
                        h2a[:, hc, :], at[:, ds(8 + 8 * hc, 8)], bcs
                    )
                return h2a

            p4st = [None, None]

            def dec_d2(tm):
                nonlocal c2
                ps2 = gpp.tile([128, 8, BS], fp, tag="g", name=f"d2ps{tm}")
                nc.tensor.matmul(
                    ps2, identb, capgT[:, :, ds(BS * tm, BS)],
                    start=True, stop=False,
                )
                for m in range(8):
                    for kc in range(2):
                        nc.tensor.matmul(
                            ps2[:, m, :], wd2[:, kc, m, :], h1T_prev[:, kc, :],
                            start=False, stop=False,
                        )
                # attention for the previous step (produces this step's h2)
                h2rhs = (
                    h2seqT[:, :, T - 1, :] if tm == 0 else attn_step(tm - 1)
                )
                for m in range(8):
                    for kc in range(2, 4):
                        nc.tensor.matmul(
                            ps2[:, m, :], wd2[:, kc, m, :], h2rhs[:, kc - 2, :],
                            start=False, stop=(m == 7 and kc == 3),
                        )
                c2 = lstm_elem_T(ps2, c2, h2decT[:, :, tm, :], "2")
                # interleave first CE tile (rows of steps 0..15) two vocab
                # chunks per step once its h2dec rows are complete
                if tm == 16:
                    p4st[0] = p4_begin(0)
                if 16 <= tm <= 23:
                    p4_chunk(p4st[0], 2 * (tm - 16))
                    p4_chunk(p4st[0], 2 * (tm - 16) + 1)
                elif tm == 24:
                    p4_finish(p4st[0])

            for t in range(DEC):
                # d1: gates = d1_b + h1 @ d1_Whh.T
                ps = gpp.tile([128, 8, BS], fp, tag="g")
                nc.tensor.matmul(ps, identb, bd1bc, start=True, stop=False)
                for m in range(8):
                    for kc in range(2):
                        nc.tensor.matmul(
                            ps[:, m, :], wd1[:, kc, m, :], h1T[:, kc, :],
                            start=False, stop=(m == 7 and kc == 1),
                        )
                h1T_new = hpool.tile([128, 2, BS], bf, tag="h1")
                c1 = lstm_elem_T(ps, c1, h1T_new, "1")
                h1T_prev = h1T
                h1T = h1T_new
                # d2 for the previous step (same one-late emission as the
                # encoder, so d1_{t+1} never sits behind a stalled d2)
                if t >= 1:
                    dec_d2(t - 1)
            h1T_prev = h1T
            dec_d2(DEC - 1)
            # second CE tile right after the last d2 (its attention is
            # never consumed, so it is skipped entirely)
            p4st[1] = p4_begin(1)
            for c in range(NCH):
                p4_chunk(p4st[1], c)
            p4_finish(p4st[1])

            # final: partial = (lse0 - dot0 + lse1 - dot1) / B^2
            d0 = smsb.tile([1, 1], fp, tag="d0")
            nc.vector.tensor_sub(d0, ce_acc[:, 0:1], ce_acc[:, 1:2])
            d1_ = smsb.tile([1, 1], fp, tag="d1")
            nc.vector.tensor_sub(d1_, ce_acc[:, 2:3], ce_acc[:, 3:4])
            tot = smsb.tile([1, 1], fp, tag="tot")
            nc.vector.tensor_add(tot, d0, d1_)
            outsb = smsb.tile([1, 1], fp, tag="osb")
            nc.scalar.mul(outsb, tot, 1.0 / (B * B))
            nc.sync.dma_start(out_d[:, :], outsb)

    nc.compile()
    return nc


def _shard_inputs(inputs):
    """Host-side relayout + shard. Returns (list of 8 in_maps, host_bias)."""
    import ml_dtypes

    f32 = np.float32
    bft = ml_dtypes.bfloat16
    feat = np.asarray(inputs["feat"], f32)
    caption = np.asarray(inputs["caption"], f32)
    oh = np.asarray(inputs["caption_one_hot"], f32)

    def w(name):
        return np.asarray(inputs[name], f32)

    # i/f/o gate chunks (first 6 in PERM order) pre-scaled by 0.5 so the
    # kernel computes sigmoid as 0.5*tanh(scaled_gates)+0.5 with a single
    # full-width Tanh
    scale_m = np.array([0.5] * 6 + [1.0] * 2, f32)

    def wtile(Wt):  # Wt [K, 1024] -> [128, K//128, 8, 128] bf16, m permuted
        Kc = Wt.shape[0] // 128
        a = Wt.reshape(Kc, 128, 8, 128)[:, :, PERM, :]
        a = a * scale_m[None, None, :, None]
        return np.ascontiguousarray(a.transpose(1, 0, 2, 3).astype(bft))

    def bcol(bv):  # [1024] -> [128, 8] fp32, m permuted + scaled
        a = bv.reshape(8, 128)[PERM].T * scale_m[None, :]
        return np.ascontiguousarray(a.astype(f32))

    def bbc(bv):  # [1024] -> [128, 8, BS] broadcast over batch, bf16
        return np.ascontiguousarray(
            np.repeat(bcol(bv)[:, :, None], BS, axis=2).astype(bft)
        )

    out_W = w("out_W")
    shared = dict(
        w1t=wtile(w("e1_Wih").T),
        w1hht=wtile(w("e1_Whh").T),
        w2t=wtile(np.concatenate([w("e2_Wih")[:, H:], w("e2_Whh")], 1).T),
        wd1t=wtile(w("d1_Whh").T),
        wd2t=wtile(np.concatenate([w("d2_Wih")[:, H:], w("d2_Whh")], 1).T),
        wd2lt=wtile(w("d2_Wih")[:, :H].T),
        b1col=bcol(w("e1_b")),
        b2bc=bbc(w("e2_b")),
        bd1bc=bbc(w("d1_b")),
        bd2col=bcol(w("d2_b")),
        wot=np.ascontiguousarray(
            out_W.T.reshape(2, 128, V).transpose(1, 0, 2).astype(bft)
        ),
        bot=np.ascontiguousarray(w("out_b").reshape(1, V).astype(bft)),
    )

    tgt = np.argmax(oh, axis=2)[:, 1:]  # [B, DEC]
    host_bias = f32(w("out_b")[tgt].sum()) / f32(B * B)

    in_maps = []
    for c in range(NCORES):
        b0 = c * BS
        featT = np.ascontiguousarray(
            feat[b0 : b0 + BS]
            .transpose(2, 1, 0).reshape(KF, 128, TB).transpose(1, 0, 2)
            .astype(bft)
        )
        capT = np.ascontiguousarray(
            caption[b0 : b0 + BS, :DEC]
            .transpose(2, 1, 0).reshape(2, 128, ROWS).transpose(1, 0, 2)
            .astype(bft)
        )
        wt = out_W[tgt[b0 : b0 + BS]]  # [BS, DEC, H]
        wtgt = np.ascontiguousarray(
            wt.transpose(2, 1, 0).reshape(2, 128, DEC, BS)
            .transpose(1, 0, 2, 3).astype(bft)
        )
        m = dict(shared)
        m.update(featT=featT, capT=capT, wtgt=wtgt)
        in_maps.append(m)
    return in_maps, host_bias


def kernel(**inputs):
    from concourse.bass_utils import run_bass_kernel_spmd

    if "nc" not in _cache:
        _cache["nc"] = _build_program()
    nc = _cache["nc"]
    in_maps, host_bias = _shard_inputs(inputs)
    res = run_bass_kernel_spmd(nc, in_maps, core_ids=list(range(NCORES)))
    total = np.float32(0.0)
    for r in res.results:
        total += np.float32(r["partial"][0, 0])
    total -= host_bias
    return np.asarray(total, np.float32)


# revision 20
# speedup vs baseline: 4.7360x; 1.0775x over previous
"""Trainium2 Bass kernel for nn_Net_74259984548321 (video-caption LSTM net).

v2 design (vs v1: all-fp32, row-layout gates, device-side one-hot argmax):
  * all matmuls bf16 (fp32 matmuls cost 4 cycles/row on trn2).
  * recurrence in transposed "gatesT" layout: weight tiles [K=128, M=128]
    stationary, gates/h/c live as [128 gate/h dims, batch] so elementwise
    runs on 128 partitions with tiny free dims (8-64 elems) instead of 8
    partitions x 256-512 elems; h is born in the layout the next matmul
    needs (no per-step PE transposes).
  * biases folded into precomputed addends (g1/cap projections) or constant
    broadcast tiles (one DVE add per cell, no K=1 bias matmuls in the loop).
  * attention: scores -> exp (no max subtraction; scores are O(1)) ->
    unnormalized context + reciprocal-broadcast matmul; no transposes.
  * CE: targets argmax'd on HOST, out_W target rows gathered on HOST;
    device computes full-vocab LSE (streamed out_W, online accum) plus a
    per-row dot with the gathered rows. caption_one_hot never touches HBM.
  * feat projection G1 (the only big GEMM) runs one 16-step slice ahead of
    the encoder, interleaved into the encoder's PE idle gaps.
Per core: 8 batch rows; host sums 8 partial scalars + target-bias term.
"""

import numpy as np

B, T, FEAT, H, V, L = 64, 80, 4096, 256, 8000, 32
DEC = L - 1            # 31 decoder steps
NCORES = 8
BS = B // NCORES       # 8 batch rows per core
G = 4 * H              # 1024 gates
KF = FEAT // 128       # 32 feat contraction chunks
ROWS = DEC * BS        # 248 decoder (t, b) rows per core
NCH = 16               # vocab chunks for LSE
CSZ = V // NCH         # 500
TB = T * BS            # 640 encoder (t, b) rows per core
# gate chunk order used on-chip: i0 i1 f0 f1 o0 o1 g0 g1 (source chunks)
PERM = np.array([0, 1, 2, 3, 6, 7, 4, 5])

_cache = {}


def _build_program():
    import concourse.tile as tile
    from concourse import bacc, mybir
    from concourse.bass import ts, ds
    from concourse.masks import make_identity

    fp = mybir.dt.float32
    bf = mybir.dt.bfloat16
    AF = mybir.ActivationFunctionType
    AX = mybir.AxisListType

    nc = bacc.Bacc(None, target_bir_lowering=False)

    featT_d = nc.dram_tensor("featT", [128, KF, TB], bf, kind="ExternalInput")
    w1_d = nc.dram_tensor("w1t", [128, 8, KF, 128], bf, kind="ExternalInput")
    w1hh_d = nc.dram_tensor("w1hht", [128, 2, 8, 128], bf, kind="ExternalInput")
    w2_d = nc.dram_tensor("w2t", [128, 4, 8, 128], bf, kind="ExternalInput")
    wd1_d = nc.dram_tensor("wd1t", [128, 2, 8, 128], bf, kind="ExternalInput")
    wd2_d = nc.dram_tensor("wd2t", [128, 4, 8, 128], bf, kind="ExternalInput")
    wd2l_d = nc.dram_tensor("wd2lt", [128, 2, 8, 128], bf, kind="ExternalInput")
    b1col_d = nc.dram_tensor("b1col", [128, 8], fp, kind="ExternalInput")
    b2bc_d = nc.dram_tensor("b2bc", [128, 8, BS], bf, kind="ExternalInput")
    bd1bc_d = nc.dram_tensor("bd1bc", [128, 8, BS], bf, kind="ExternalInput")
    bd2col_d = nc.dram_tensor("bd2col", [128, 8], fp, kind="ExternalInput")
    capT_d = nc.dram_tensor("capT", [128, 2, ROWS], bf, kind="ExternalInput")
    wo_d = nc.dram_tensor("wot", [128, 2, V], bf, kind="ExternalInput")
    bo_d = nc.dram_tensor("bot", [1, V], bf, kind="ExternalInput")
    wtgt_d = nc.dram_tensor("wtgt", [128, 2, DEC, BS], bf, kind="ExternalInput")
    out_d = nc.dram_tensor("partial", [1, 1], fp, kind="ExternalOutput")

    with tile.TileContext(nc) as tc:
        from contextlib import ExitStack

        with ExitStack() as ctx:
            const = ctx.enter_context(tc.tile_pool(name="const", bufs=1))
            state = ctx.enter_context(tc.tile_pool(name="state", bufs=1))
            acts = ctx.enter_context(tc.tile_pool(name="acts", bufs=2))
            hpool = ctx.enter_context(tc.tile_pool(name="hp", bufs=3))
            smsb = ctx.enter_context(tc.tile_pool(name="smsb", bufs=2))
            # psum pools: 3 + 2 + 2 = 7 banks
            gpp = ctx.enter_context(tc.tile_pool(name="gpp", bufs=3, space="PSUM"))
            bigp = ctx.enter_context(tc.tile_pool(name="bigp", bufs=2, space="PSUM"))
            attp = ctx.enter_context(tc.tile_pool(name="attp", bufs=2, space="PSUM"))

            # ---- constants ----
            identb = const.tile([128, 128], bf, tag="idb")
            make_identity(nc, identb)
            ones80 = const.tile([T, 1], bf, tag="o80")
            nc.vector.memset(ones80, 1.0)
            ones1x128b = const.tile([1, 128], bf, tag="o1r")
            nc.vector.memset(ones1x128b, 1.0)
            ones1x128f = const.tile([1, 128], fp, tag="o1rf")
            nc.vector.memset(ones1x128f, 1.0)
            ones128b = const.tile([128, 1], bf, tag="o1c")
            nc.vector.memset(ones128b, 1.0)
            ones128f = const.tile([128, 1], fp, tag="o1cf")
            nc.vector.memset(ones128f, 1.0)

            # ---- persistent weights / addends ----
            w1hh = state.tile([128, 2, 8, 128], bf, tag="w1hh")
            nc.sync.dma_start(w1hh, w1hh_d[:, :, :, :])
            w2 = state.tile([128, 4, 8, 128], bf, tag="w2")
            nc.sync.dma_start(w2, w2_d[:, :, :, :])
            wd1 = state.tile([128, 2, 8, 128], bf, tag="wd1")
            nc.sync.dma_start(wd1, wd1_d[:, :, :, :])
            wd2 = state.tile([128, 4, 8, 128], bf, tag="wd2")
            nc.sync.dma_start(wd2, wd2_d[:, :, :, :])
            b1col = state.tile([128, 8], fp, tag="b1c")
            nc.sync.dma_start(b1col, b1col_d[:, :])
            b2bc = state.tile([128, 8, BS], bf, tag="b2bc")
            nc.sync.dma_start(b2bc, b2bc_d[:, :, :])
            bd1bc = state.tile([128, 8, BS], bf, tag="bd1bc")
            nc.sync.dma_start(bd1bc, bd1bc_d[:, :, :])
            bd2col = state.tile([128, 8], fp, tag="bd2c")
            nc.sync.dma_start(bd2col, bd2col_d[:, :])
            wtgt = state.tile([128, 2, DEC, BS], bf, tag="wtgt")
            nc.sync.dma_start(wtgt, wtgt_d[:, :, :, :])

            # ---- persistent activations ----
            h2seqB = state.tile([128, 2, BS, T], bf, tag="h2seq")
            h2decT = state.tile([128, 2, DEC, BS], bf, tag="h2dec")
            A_sb = state.tile([T, BS, H], bf, tag="Asb")
            capgT = state.tile([128, 8, ROWS], bf, tag="capg")
            ce_acc = state.tile([1, 4], fp, tag="cea")

            ALU = mybir.AluOpType

            def lstm_elem_T(gates, c_old, out_h, tg_suffix):
                """gates [128, 8, BS] in chunk order i0 i1 f0 f1 o0 o1 g0 g1,
                with the i/f/o chunks pre-scaled by 0.5 on the host so a
                single Tanh covers all 8 chunks: sigmoid(x) = 0.5*tanh(x/2)
                + 0.5. Only Tanh/Exp/Copy are used in the loops -> one ACT
                table, no reloads. Writes h (bf16) to out_h [128, 2, BS];
                returns the new c tile."""
                tnh = acts.tile([128, 8, BS], fp, tag="tn" + tg_suffix)
                nc.scalar.activation(tnh, gates, AF.Tanh)
                sio = acts.tile([128, 6, BS], fp, tag="sio" + tg_suffix)
                if tg_suffix == "1":
                    # keep ACT/DVE balanced: cell1's affine on ACT
                    nc.scalar.activation(
                        sio, tnh[:, 0:6, :], AF.Copy, bias=0.5, scale=0.5
                    )
                else:
                    nc.vector.tensor_scalar(
                        sio, tnh[:, 0:6, :], 0.5, 0.5, ALU.mult, ALU.add
                    )
                t1 = acts.tile([128, 2, BS], fp, tag="t1" + tg_suffix)
                nc.vector.tensor_mul(t1, sio[:, 0:2, :], tnh[:, 6:8, :])
                if c_old is None:
                    c_new = t1
                else:
                    cm = acts.tile([128, 2, BS], fp, tag="cm" + tg_suffix)
                    nc.vector.tensor_mul(cm, sio[:, 2:4, :], c_old)
                    c_new = acts.tile([128, 2, BS], fp, tag="c" + tg_suffix)
                    nc.vector.tensor_add(c_new, cm, t1)
                th = acts.tile([128, 2, BS], fp, tag="th" + tg_suffix)
                nc.scalar.activation(th, c_new, AF.Tanh)
                nc.vector.tensor_mul(out_h, sio[:, 4:6, :], th)
                return c_new

            # ================ P1 + encoder (scoped: feat/w1/g1 freed after) ====
            with ExitStack() as p1ctx:
                p1w = p1ctx.enter_context(tc.tile_pool(name="p1w", bufs=1))

                capT = p1w.tile([128, 2, ROWS], bf, tag="capT")
                nc.sync.dma_start(capT, capT_d[:, :, :])
                wd2l = p1w.tile([128, 2, 8, 128], bf, tag="wd2l")
                nc.sync.dma_start(wd2l, wd2l_d[:, :, :, :])
                feat_sb = p1w.tile([128, KF, TB], bf, tag="feat")
                nc.sync.dma_start(feat_sb, featT_d[:, :, :])
                w1sb = p1w.tile([128, 8, KF, 128], bf, tag="w1")
                nc.sync.dma_start(w1sb, w1_d[:, :, :, :])
                g1Ts = [
                    p1w.tile([128, 8, 128], bf, tag=f"g1s{s}", name=f"g1s{s}")
                    for s in range(5)
                ]

                # cap_proj: capgT[:, m, r] = (cap @ d2_Wih_l.T + d2_b) gatesT
                for m in range(8):
                    ps = bigp.tile([128, CSZ], fp, tag="big")
                    for kc in range(2):
                        nc.tensor.matmul(
                            ps[:, 0:ROWS], wd2l[:, kc, m, :], capT[:, kc, :],
                            start=(kc == 0), stop=(kc == 1),
                        )
                    nc.vector.tensor_scalar_add(
                        capgT[:, m, :], ps[:, 0:ROWS], bd2col[:, m : m + 1]
                    )

                # G1 slice builder: 32-k accumulation for (s, m), in halves
                g1ps_box = [None]

                def g1_group(s, m, half):
                    if half == 0:
                        g1ps_box[0] = bigp.tile(
                            [128, CSZ], fp, tag="big", name=f"g1ps{s}_{m}"
                        )
                    ps = g1ps_box[0]
                    for k in range(16 * half, 16 * half + 16):
                        nc.tensor.matmul(
                            ps[:, 0:128],
                            w1sb[:, m, k, :],
                            feat_sb[:, k, ds(128 * s, 128)],
                            start=(k == 0), stop=(k == KF - 1),
                        )
                    if half == 1:
                        nc.vector.tensor_scalar_add(
                            g1Ts[s][:, m, :], ps[:, 0:128], b1col[:, m : m + 1]
                        )

                for m in range(8):  # slice 0 up front
                    g1_group(0, m, 0)
                    g1_group(0, m, 1)

                # ================ P2: encoder ================
                # cell2 is emitted one iteration late so cell1_{t+1}'s
                # matmuls+chain sit ahead of cell2_t in the PE stream and
                # start as soon as h1_t is ready (mid-ladder), instead of
                # the whole step serializing burst -> ladder -> burst.
                h1T = None
                h1T_prev = None
                c1 = None
                c2 = None

                def enc_cell2(tm):
                    nonlocal c2
                    ps2 = gpp.tile(
                        [128, 8, BS], fp, tag="g", name=f"c2ps{tm}"
                    )
                    nc.tensor.matmul(ps2, identb, b2bc, start=True, stop=False)
                    nkc = 2 if tm == 0 else 4
                    for m in range(8):
                        for kc in range(nkc):
                            rhs = (
                                h1T_prev[:, kc, :]
                                if kc < 2
                                else h2seqB[:, kc - 2, :, tm - 1]
                            )
                            nc.tensor.matmul(
                                ps2[:, m, :], w2[:, kc, m, :], rhs,
                                start=False,
                                stop=(m == 7 and kc == nkc - 1),
                            )
                    c2 = lstm_elem_T(ps2, c2, h2seqB[:, :, :, tm], "2")

                for t in range(T):
                    s = t // 16
                    # ---- cell 1 (addend folded into the psum group) ----
                    if t == 0:
                        gates1 = g1Ts[0][:, :, 0:BS]
                    else:
                        ps = gpp.tile([128, 8, BS], fp, tag="g")
                        nc.tensor.matmul(
                            ps, identb, g1Ts[s][:, :, ds(BS * (t % 16), BS)],
                            start=True, stop=False,
                        )
                        for m in range(8):
                            for kc in range(2):
                                nc.tensor.matmul(
                                    ps[:, m, :], w1hh[:, kc, m, :], h1T[:, kc, :],
                                    start=False,
                                    stop=(m == 7 and kc == 1),
                                )
                        gates1 = ps
                    h1T_new = hpool.tile([128, 2, BS], bf, tag="h1")
                    c1 = lstm_elem_T(gates1, c1, h1T_new, "1")
                    h1T_prev = h1T
                    h1T = h1T_new
                    # ---- G1 interleave fills the PE gap while cell1's
                    # elementwise chain runs
                    if t < 64:
                        g1_group(1 + t // 16, (t % 16) // 2, t % 2)
                    # ---- cell 2 for the previous step ----
                    if t >= 1:
                        enc_cell2(t - 1)
                h1T_prev = h1T
                enc_cell2(T - 1)

            # A_sb[te, b, :] = h2seq[b, te, :] (row layout for context matmul)
            for b in range(BS):
                for kc in range(2):
                    pA = attp.tile([T, 128], bf, tag="at")
                    nc.tensor.transpose(pA, h2seqB[:, kc, b, :], identb)
                    nc.vector.tensor_copy(A_sb[:, b, ts(kc, 128)], pA)

            # ================ P4 emitter ================
            wos = ctx.enter_context(tc.tile_pool(name="wos", bufs=4))
            junk = ctx.enter_context(tc.tile_pool(name="junk", bufs=2))
            p4sb = ctx.enter_context(tc.tile_pool(name="p4sb", bufs=2))

            def p4_begin(mi):
                t0 = 16 * mi
                tn = 16 if mi == 0 else DEC - 16
                s_all = p4sb.tile([128, NCH], fp, tag="sall", name=f"sall{mi}")
                return dict(mi=mi, t0=t0, tn=tn, R=tn * BS, s_all=s_all)

            def p4_chunk(st, c):
                t0, tn, R = st["t0"], st["tn"], st["R"]
                wot = wos.tile([128, 2, CSZ], bf, tag="wo")
                nc.sync.dma_start(wot, wo_d[:, :, ts(c, CSZ)])
                bot = wos.tile([1, CSZ], bf, tag="bo")
                nc.sync.dma_start(bot, bo_d[:, ts(c, CSZ)])
                psL = bigp.tile([128, CSZ], fp, tag="big")
                nc.tensor.matmul(
                    psL[:R], ones1x128b[:, :R], bot, start=True, stop=False
                )
                for kc in range(2):
                    nc.tensor.matmul(
                        psL[:R],
                        h2decT[:, kc, t0 : t0 + tn, :],
                        wot[:, kc, :],
                        start=False, stop=(kc == 1),
                    )
                jk = junk.tile([128, CSZ], bf, tag="jk")
                nc.scalar.activation(
                    jk[:R], psL[:R], AF.Exp,
                    accum_out=st["s_all"][:R, c : c + 1],
                )

            def p4_finish(st):
                mi, t0, tn, R = st["mi"], st["t0"], st["tn"], st["R"]
                ssum = p4sb.tile([128, 1], fp, tag="ssum")
                nc.vector.reduce_sum(ssum[:R], st["s_all"][:R], axis=AX.X)
                lse = p4sb.tile([128, 1], fp, tag="lse")
                nc.scalar.activation(lse[:R], ssum[:R], AF.Ln)
                at = attp.tile([128, 168], fp, tag="at")
                nc.tensor.matmul(
                    at[0:1, 33:34], lse[:R], ones128f[:R], start=True, stop=True
                )
                nc.vector.tensor_copy(ce_acc[:, 2 * mi : 2 * mi + 1], at[0:1, 33:34])
                # target-row dot: sum_rows h2dec . w_tgt
                prod = p4sb.tile([128, 2, 16, BS], bf, tag="prod")
                for kc in range(2):
                    nc.vector.tensor_mul(
                        prod[:, kc, 0:tn, :],
                        h2decT[:, kc, t0 : t0 + tn, :],
                        wtgt[:, kc, t0 : t0 + tn, :],
                    )
                for kc in range(2):
                    nc.tensor.matmul(
                        at[0:1, 40 : 40 + R], ones128b, prod[:, kc, 0:tn, :],
                        start=(kc == 0), stop=(kc == 1),
                    )
                ltsum = p4sb.tile([1, 1], fp, tag="lts")
                nc.vector.reduce_sum(ltsum, at[0:1, 40 : 40 + R], axis=AX.X)
                nc.vector.tensor_copy(ce_acc[:, 2 * mi + 1 : 2 * mi + 2], ltsum)

            # ================ P3: decoder ================
            # Software-pipelined: attention for step t-1 is emitted between
            # d2_t's h1-half (kc 0,1) and h2-half (kc 2,3), so d1_t and half
            # of d2_t overlap the previous step's attention chain.
            def attn_step(tq):
                """h2 <- softmax(h2seq . h2dec[tq]) . h2seq, returns h2aT."""
                at = attp.tile([128, 168], fp, tag="at", name=f"at{tq}")
                q = h2decT[:, :, tq, :]
                for b in range(BS):
                    for kc in range(2):
                        nc.tensor.matmul(
                            at[0:T, b : b + 1],
                            h2seqB[:, kc, b, :],
                            q[:, kc, b : b + 1],
                            start=(kc == 0), stop=(kc == 1),
                        )
                expT = smsb.tile([T, BS], bf, tag="exp")
                nc.scalar.activation(expT, at[0:T, 0:BS], AF.Exp)
                for b in range(BS):
                    for hc in range(2):
                        col = 8 + 8 * hc + b
                        nc.tensor.matmul(
                            at[:, col : col + 1],
                            A_sb[:, b, ts(hc, 128)],
                            expT[:, b : b + 1],
                            start=True, stop=True,
                        )
                nc.tensor.matmul(
                    at[0:1, 32:40], ones80, expT, start=True, stop=True
                )
                recip = smsb.tile([1, BS], fp, tag="rcp")
                nc.vector.reciprocal(recip, at[0:1, 32:40])
                nc.tensor.matmul(
                    at[:, 24:32], ones1x128f, recip, start=True, stop=True
                )
                bcs = smsb.tile([128, BS], bf, tag="bcs")
                nc.vector.tensor_copy(bcs, at[:, 24:32])
                h2a = smsb.tile([128, 2, BS], bf, tag="h2a")
                for hc in range(2):
                    nc.vector.tensor_mul(
                        h2a[:, hc, :], at[:, ds(8 + 8 * hc, 8)], bcs
                    )
                return h2a

            p4st = [None, None]

            def dec_d2(tm):
                nonlocal c2
                ps2 = gpp.tile([128, 8, BS], fp, tag="g", name=f"d2ps{tm}")
                nc.tensor.matmul(
                    ps2, identb, capgT[:, :, ds(BS * tm, BS)],
                    start=True, stop=False,
                )
                for m in range(8):
                    for kc in range(2):
                        nc.tensor.matmul(
                            ps2[:, m, :], wd2[:, kc, m, :], h1T_prev[:, kc, :],
                            start=False, stop=False,
                        )
                # attention for the previous step (produces this step's h2)
                h2rhs = (
                    h2seqB[:, :, :, T - 1] if tm == 0 else attn_step(tm - 1)
                )
                for m in range(8):
                    for kc in range(2, 4):
                        nc.tensor.matmul(
                            ps2[:, m, :], wd2[:, kc, m, :], h2rhs[:, kc - 2, :],
                            start=False, stop=(m == 7 and kc == 3),
                        )
                c2 = lstm_elem_T(ps2, c2, h2decT[:, :, tm, :], "2")
                # interleave first CE tile (rows of steps 0..15) two vocab
                # chunks per step once its h2dec rows are complete
                if tm == 16:
                    p4st[0] = p4_begin(0)
                if 16 <= tm <= 23:
                    p4_chunk(p4st[0], 2 * (tm - 16))
                    p4_chunk(p4st[0], 2 * (tm - 16) + 1)
                elif tm == 24:
                    p4_finish(p4st[0])

            for t in range(DEC):
                # d1: gates = d1_b + h1 @ d1_Whh.T
                ps = gpp.tile([128, 8, BS], fp, tag="g")
                nc.tensor.matmul(ps, identb, bd1bc, start=True, stop=False)
                for m in range(8):
                    for kc in range(2):
                        nc.tensor.matmul(
                            ps[:, m, :], wd1[:, kc, m, :], h1T[:, kc, :],
                            start=False, stop=(m == 7 and kc == 1),
                        )
                h1T_new = hpool.tile([128, 2, BS], bf, tag="h1")
                c1 = lstm_elem_T(ps, c1, h1T_new, "1")
                h1T_prev = h1T
                h1T = h1T_new
                # d2 for the previous step (same one-late emission as the
                # encoder, so d1_{t+1} never sits behind a stalled d2)
                if t >= 1:
                    dec_d2(t - 1)
            h1T_prev = h1T
            dec_d2(DEC - 1)
            # second CE tile right after the last d2 (its attention is
            # never consumed, so it is skipped entirely)
            p4st[1] = p4_begin(1)
            for c in range(NCH):
                p4_chunk(p4st[1], c)
            p4_finish(p4st[1])

            # final: partial = (lse0 - dot0 + lse1 - dot1) / B^2
            d0 = smsb.tile([1, 1], fp, tag="d0")
            nc.vector.tensor_sub(d0, ce_acc[:, 0:1], ce_acc[:, 1:2])
            d1_ = smsb.tile([1, 1], fp, tag="d1")
            nc.vector.tensor_sub(d1_, ce_acc[:, 2:3], ce_acc[:, 3:4])
            tot = smsb.tile([1, 1], fp, tag="tot")
            nc.vector.tensor_add(tot, d0, d1_)
            outsb = smsb.tile([1, 1], fp, tag="osb")
            nc.scalar.mul(outsb, tot, 1.0 / (B * B))
            nc.sync.dma_start(out_d[:, :], outsb)

    nc.compile()
    return nc


def _shard_inputs(inputs):
    """Host-side relayout + shard. Returns (list of 8 in_maps, host_bias)."""
    import ml_dtypes

    f32 = np.float32
    bft = ml_dtypes.bfloat16
    feat = np.asarray(inputs["feat"], f32)
    caption = np.asarray(inputs["caption"], f32)
    oh = np.asarray(inputs["caption_one_hot"], f32)

    def w(name):
        return np.asarray(inputs[name], f32)

    # i/f/o gate chunks (first 6 in PERM order) pre-scaled by 0.5 so the
    # kernel computes sigmoid as 0.5*tanh(scaled_gates)+0.5 with a single
    # full-width Tanh
    scale_m = np.array([0.5] * 6 + [1.0] * 2, f32)

    def wtile(Wt):  # Wt [K, 1024] -> [128, K//128, 8, 128] bf16, m permuted
        Kc = Wt.shape[0] // 128
        a = Wt.reshape(Kc, 128, 8, 128)[:, :, PERM, :]
        a = a * scale_m[None, None, :, None]
        return np.ascontiguousarray(a.transpose(1, 0, 2, 3).astype(bft))

    def bcol(bv):  # [1024] -> [128, 8] fp32, m permuted + scaled
        a = bv.reshape(8, 128)[PERM].T * scale_m[None, :]
        return np.ascontiguousarray(a.astype(f32))

    def bbc(bv):  # [1024] -> [128, 8, BS] broadcast over batch, bf16
        return np.ascontiguousarray(
            np.repeat(bcol(bv)[:, :, None], BS, axis=2).astype(bft)
        )

    out_W = w("out_W")
    shared = dict(
        w1t=wtile(w("e1_Wih").T),
        w1hht=wtile(w("e1_Whh").T),
        w2t=wtile(np.concatenate([w("e2_Wih")[:, H:], w("e2_Whh")], 1).T),
        wd1t=wtile(w("d1_Whh").T),
        wd2t=wtile(np.concatenate([w("d2_Wih")[:, H:], w("d2_Whh")], 1).T),
        wd2lt=wtile(w("d2_Wih")[:, :H].T),
        b1col=bcol(w("e1_b")),
        b2bc=bbc(w("e2_b")),
        bd1bc=bbc(w("d1_b")),
        bd2col=bcol(w("d2_b")),
        wot=np.ascontiguousarray(
            out_W.T.reshape(2, 128, V).transpose(1, 0, 2).astype(bft)
        ),
        bot=np.ascontiguousarray(w("out_b").reshape(1, V).astype(bft)),
    )

    tgt = np.argmax(oh, axis=2)[:, 1:]  # [B, DEC]
    host_bias = f32(w("out_b")[tgt].sum()) / f32(B * B)

    in_maps = []
    for c in range(NCORES):
        b0 = c * BS
        featT = np.ascontiguousarray(
            feat[b0 : b0 + BS]
            .transpose(2, 1, 0).reshape(KF, 128, TB).transpose(1, 0, 2)
            .astype(bft)
        )
        capT = np.ascontiguousarray(
            caption[b0 : b0 + BS, :DEC]
            .transpose(2, 1, 0).reshape(2, 128, ROWS).transpose(1, 0, 2)
            .astype(bft)
        )
        wt = out_W[tgt[b0 : b0 + BS]]  # [BS, DEC, H]
        wtgt = np.ascontiguousarray(
            wt.transpose(2, 1, 0).reshape(2, 128, DEC, BS)
            .transpose(1, 0, 2, 3).astype(bft)
        )
        m = dict(shared)
        m.update(featT=featT, capT=capT, wtgt=wtgt)
        in_maps.append(m)
    return in_maps, host_bias


def kernel(**inputs):
    from concourse.bass_utils import run_bass_kernel_spmd

    if "nc" not in _cache:
        _cache["nc"] = _build_program()
    nc = _cache["nc"]
    in_maps, host_bias = _shard_inputs(inputs)
    res = run_bass_kernel_spmd(nc, in_maps, core_ids=list(range(NCORES)))
    total = np.float32(0.0)
    for r in res.results:
        total += np.float32(r["partial"][0, 0])
    total -= host_bias
    return np.asarray(total, np.float32)


# revision 22
# speedup vs baseline: 4.7380x; 1.0004x over previous
"""Trainium2 Bass kernel for nn_Net_74259984548321 (video-caption LSTM net).

v2 design (vs v1: all-fp32, row-layout gates, device-side one-hot argmax):
  * all matmuls bf16 (fp32 matmuls cost 4 cycles/row on trn2).
  * recurrence in transposed "gatesT" layout: weight tiles [K=128, M=128]
    stationary, gates/h/c live as [128 gate/h dims, batch] so elementwise
    runs on 128 partitions with tiny free dims (8-64 elems) instead of 8
    partitions x 256-512 elems; h is born in the layout the next matmul
    needs (no per-step PE transposes).
  * biases folded into precomputed addends (g1/cap projections) or constant
    broadcast tiles (one DVE add per cell, no K=1 bias matmuls in the loop).
  * attention: scores -> exp (no max subtraction; scores are O(1)) ->
    unnormalized context + reciprocal-broadcast matmul; no transposes.
  * CE: targets argmax'd on HOST, out_W target rows gathered on HOST;
    device computes full-vocab LSE (streamed out_W, online accum) plus a
    per-row dot with the gathered rows. caption_one_hot never touches HBM.
  * feat projection G1 (the only big GEMM) runs one 16-step slice ahead of
    the encoder, interleaved into the encoder's PE idle gaps.
Per core: 8 batch rows; host sums 8 partial scalars + target-bias term.
"""

import numpy as np

B, T, FEAT, H, V, L = 64, 80, 4096, 256, 8000, 32
DEC = L - 1            # 31 decoder steps
NCORES = 8
BS = B // NCORES       # 8 batch rows per core
G = 4 * H              # 1024 gates
KF = FEAT // 128       # 32 feat contraction chunks
ROWS = DEC * BS        # 248 decoder (t, b) rows per core
NCH = 16               # vocab chunks for LSE
CSZ = V // NCH         # 500
TB = T * BS            # 640 encoder (t, b) rows per core
# gate chunk order used on-chip: i0 i1 f0 f1 o0 o1 g0 g1 (source chunks)
PERM = np.array([0, 1, 2, 3, 6, 7, 4, 5])

_cache = {}


def _build_program():
    import concourse.tile as tile
    from concourse import bacc, mybir
    from concourse.bass import ts, ds
    from concourse.masks import make_identity

    fp = mybir.dt.float32
    bf = mybir.dt.bfloat16
    AF = mybir.ActivationFunctionType
    AX = mybir.AxisListType

    nc = bacc.Bacc(None, target_bir_lowering=False)

    featT_d = nc.dram_tensor("featT", [128, KF, TB], bf, kind="ExternalInput")
    w1_d = nc.dram_tensor("w1t", [128, 8, KF, 128], bf, kind="ExternalInput")
    w1hh_d = nc.dram_tensor("w1hht", [128, 2, 8, 128], bf, kind="ExternalInput")
    w2_d = nc.dram_tensor("w2t", [128, 4, 8, 128], bf, kind="ExternalInput")
    wd1_d = nc.dram_tensor("wd1t", [128, 2, 8, 128], bf, kind="ExternalInput")
    wd2_d = nc.dram_tensor("wd2t", [128, 4, 8, 128], bf, kind="ExternalInput")
    wd2l_d = nc.dram_tensor("wd2lt", [128, 2, 8, 128], bf, kind="ExternalInput")
    b1col_d = nc.dram_tensor("b1col", [128, 8], fp, kind="ExternalInput")
    b2bc_d = nc.dram_tensor("b2bc", [128, 8, BS], bf, kind="ExternalInput")
    bd1bc_d = nc.dram_tensor("bd1bc", [128, 8, BS], bf, kind="ExternalInput")
    bd2col_d = nc.dram_tensor("bd2col", [128, 8], fp, kind="ExternalInput")
    capT_d = nc.dram_tensor("capT", [128, 2, ROWS], bf, kind="ExternalInput")
    wo_d = nc.dram_tensor("wot", [128, 2, V], bf, kind="ExternalInput")
    bo_d = nc.dram_tensor("bot", [1, V], bf, kind="ExternalInput")
    wtgt_d = nc.dram_tensor("wtgt", [128, 2, DEC, BS], bf, kind="ExternalInput")
    out_d = nc.dram_tensor("partial", [1, 1], fp, kind="ExternalOutput")

    with tile.TileContext(nc) as tc:
        from contextlib import ExitStack

        with ExitStack() as ctx:
            const = ctx.enter_context(tc.tile_pool(name="const", bufs=1))
            state = ctx.enter_context(tc.tile_pool(name="state", bufs=1))
            acts = ctx.enter_context(tc.tile_pool(name="acts", bufs=2))
            hpool = ctx.enter_context(tc.tile_pool(name="hp", bufs=3))
            smsb = ctx.enter_context(tc.tile_pool(name="smsb", bufs=2))
            # psum pools: 3 + 2 + 2 = 7 banks
            gpp = ctx.enter_context(tc.tile_pool(name="gpp", bufs=3, space="PSUM"))
            bigp = ctx.enter_context(tc.tile_pool(name="bigp", bufs=2, space="PSUM"))
            attp = ctx.enter_context(tc.tile_pool(name="attp", bufs=2, space="PSUM"))

            # ---- constants ----
            identb = const.tile([128, 128], bf, tag="idb")
            make_identity(nc, identb)
            ones80 = const.tile([T, 1], bf, tag="o80")
            nc.vector.memset(ones80, 1.0)
            ones1x128b = const.tile([1, 128], bf, tag="o1r")
            nc.vector.memset(ones1x128b, 1.0)
            ones1x128f = const.tile([1, 128], fp, tag="o1rf")
            nc.vector.memset(ones1x128f, 1.0)
            ones128b = const.tile([128, 1], bf, tag="o1c")
            nc.vector.memset(ones128b, 1.0)
            ones128f = const.tile([128, 1], fp, tag="o1cf")
            nc.vector.memset(ones128f, 1.0)

            # ---- persistent weights / addends ----
            w1hh = state.tile([128, 2, 8, 128], bf, tag="w1hh")
            nc.sync.dma_start(w1hh, w1hh_d[:, :, :, :])
            w2 = state.tile([128, 4, 8, 128], bf, tag="w2")
            nc.sync.dma_start(w2, w2_d[:, :, :, :])
            wd1 = state.tile([128, 2, 8, 128], bf, tag="wd1")
            nc.sync.dma_start(wd1, wd1_d[:, :, :, :])
            wd2 = state.tile([128, 4, 8, 128], bf, tag="wd2")
            nc.sync.dma_start(wd2, wd2_d[:, :, :, :])
            b1col = state.tile([128, 8], fp, tag="b1c")
            nc.sync.dma_start(b1col, b1col_d[:, :])
            b2bc = state.tile([128, 8, BS], bf, tag="b2bc")
            nc.sync.dma_start(b2bc, b2bc_d[:, :, :])
            bd1bc = state.tile([128, 8, BS], bf, tag="bd1bc")
            nc.sync.dma_start(bd1bc, bd1bc_d[:, :, :])
            bd2col = state.tile([128, 8], fp, tag="bd2c")
            nc.sync.dma_start(bd2col, bd2col_d[:, :])
            wtgt = state.tile([128, 2, DEC, BS], bf, tag="wtgt")
            nc.sync.dma_start(wtgt, wtgt_d[:, :, :, :])

            # ---- persistent activations ----
            h2seqB = state.tile([128, 2, BS, T], bf, tag="h2seq")
            h2decT = state.tile([128, 2, DEC, BS], bf, tag="h2dec")
            A_sb = state.tile([T, BS, H], bf, tag="Asb")
            capgT = state.tile([128, 8, ROWS], bf, tag="capg")
            ce_acc = state.tile([1, 4], fp, tag="cea")

            ALU = mybir.AluOpType

            def lstm_elem_T(gates, c_old, out_h, tg_suffix):
                """gates [128, 8, BS] in chunk order i0 i1 f0 f1 o0 o1 g0 g1,
                with the i/f/o chunks pre-scaled by 0.5 on the host so a
                single Tanh covers all 8 chunks: sigmoid(x) = 0.5*tanh(x/2)
                + 0.5. Only Tanh/Exp/Copy are used in the loops -> one ACT
                table, no reloads. Writes h (bf16) to out_h [128, 2, BS];
                returns the new c tile."""
                tnh = acts.tile([128, 8, BS], fp, tag="tn" + tg_suffix)
                nc.scalar.activation(tnh, gates, AF.Tanh)
                sio = acts.tile([128, 6, BS], fp, tag="sio" + tg_suffix)
                if tg_suffix == "1":
                    # keep ACT/DVE balanced: cell1's affine on ACT
                    nc.scalar.activation(
                        sio, tnh[:, 0:6, :], AF.Copy, bias=0.5, scale=0.5
                    )
                else:
                    nc.vector.tensor_scalar(
                        sio, tnh[:, 0:6, :], 0.5, 0.5, ALU.mult, ALU.add
                    )
                t1 = acts.tile([128, 2, BS], fp, tag="t1" + tg_suffix)
                nc.vector.tensor_mul(t1, sio[:, 0:2, :], tnh[:, 6:8, :])
                if c_old is None:
                    c_new = t1
                else:
                    cm = acts.tile([128, 2, BS], fp, tag="cm" + tg_suffix)
                    nc.vector.tensor_mul(cm, sio[:, 2:4, :], c_old)
                    c_new = acts.tile([128, 2, BS], fp, tag="c" + tg_suffix)
                    nc.vector.tensor_add(c_new, cm, t1)
                th = acts.tile([128, 2, BS], fp, tag="th" + tg_suffix)
                nc.scalar.activation(th, c_new, AF.Tanh)
                nc.vector.tensor_mul(out_h, sio[:, 4:6, :], th)
                return c_new

            # ================ P1 + encoder (scoped: feat/w1/g1 freed after) ====
            with ExitStack() as p1ctx:
                p1w = p1ctx.enter_context(tc.tile_pool(name="p1w", bufs=1))

                capT = p1w.tile([128, 2, ROWS], bf, tag="capT")
                nc.sync.dma_start(capT, capT_d[:, :, :])
                wd2l = p1w.tile([128, 2, 8, 128], bf, tag="wd2l")
                nc.sync.dma_start(wd2l, wd2l_d[:, :, :, :])
                feat_sb = p1w.tile([128, KF, TB], bf, tag="feat")
                nc.sync.dma_start(feat_sb, featT_d[:, :, :])
                w1sb = p1w.tile([128, 8, KF, 128], bf, tag="w1")
                nc.sync.dma_start(w1sb, w1_d[:, :, :, :])
                g1Ts = [
                    p1w.tile([128, 8, 128], bf, tag=f"g1s{s}", name=f"g1s{s}")
                    for s in range(5)
                ]

                # cap_proj: capgT[:, m, r] = (cap @ d2_Wih_l.T + d2_b) gatesT
                for m in range(8):
                    ps = bigp.tile([128, CSZ], fp, tag="big")
                    for kc in range(2):
                        nc.tensor.matmul(
                            ps[:, 0:ROWS], wd2l[:, kc, m, :], capT[:, kc, :],
                            start=(kc == 0), stop=(kc == 1),
                        )
                    nc.vector.tensor_scalar_add(
                        capgT[:, m, :], ps[:, 0:ROWS], bd2col[:, m : m + 1]
                    )

                # G1 slice builder: 32-k accumulation for (s, m), in halves
                g1ps_box = [None]

                def g1_group(s, m, half):
                    if half == 0:
                        g1ps_box[0] = bigp.tile(
                            [128, CSZ], fp, tag="big", name=f"g1ps{s}_{m}"
                        )
                    ps = g1ps_box[0]
                    for k in range(16 * half, 16 * half + 16):
                        nc.tensor.matmul(
                            ps[:, 0:128],
                            w1sb[:, m, k, :],
                            feat_sb[:, k, ds(128 * s, 128)],
                            start=(k == 0), stop=(k == KF - 1),
                        )
                    if half == 1:
                        nc.vector.tensor_scalar_add(
                            g1Ts[s][:, m, :], ps[:, 0:128], b1col[:, m : m + 1]
                        )

                for m in range(8):  # slice 0 up front
                    g1_group(0, m, 0)
                    g1_group(0, m, 1)

                # ================ P2: encoder ================
                # cell2 is emitted one iteration late so cell1_{t+1}'s
                # matmuls+chain sit ahead of cell2_t in the PE stream and
                # start as soon as h1_t is ready (mid-ladder), instead of
                # the whole step serializing burst -> ladder -> burst.
                h1T = None
                h1T_prev = None
                c1 = None
                c2 = None

                def enc_cell2(tm):
                    nonlocal c2
                    ps2 = gpp.tile(
                        [128, 8, BS], fp, tag="g", name=f"c2ps{tm}"
                    )
                    nc.tensor.matmul(ps2, identb, b2bc, start=True, stop=False)
                    nkc = 2 if tm == 0 else 4
                    for m in range(8):
                        for kc in range(nkc):
                            rhs = (
                                h1T_prev[:, kc, :]
                                if kc < 2
                                else h2seqB[:, kc - 2, :, tm - 1]
                            )
                            nc.tensor.matmul(
                                ps2[:, m, :], w2[:, kc, m, :], rhs,
                                start=False,
                                stop=(m == 7 and kc == nkc - 1),
                            )
                    c2 = lstm_elem_T(ps2, c2, h2seqB[:, :, :, tm], "2")

                for t in range(T):
                    s = t // 16
                    # ---- cell 1 (addend folded into the psum group) ----
                    if t == 0:
                        gates1 = g1Ts[0][:, :, 0:BS]
                    else:
                        ps = gpp.tile([128, 8, BS], fp, tag="g")
                        nc.tensor.matmul(
                            ps, identb, g1Ts[s][:, :, ds(BS * (t % 16), BS)],
                            start=True, stop=False,
                        )
                        for m in range(8):
                            for kc in range(2):
                                nc.tensor.matmul(
                                    ps[:, m, :], w1hh[:, kc, m, :], h1T[:, kc, :],
                                    start=False,
                                    stop=(m == 7 and kc == 1),
                                )
                        gates1 = ps
                    h1T_new = hpool.tile([128, 2, BS], bf, tag="h1")
                    c1 = lstm_elem_T(gates1, c1, h1T_new, "1")
                    h1T_prev = h1T
                    h1T = h1T_new
                    # ---- G1 interleave fills the PE gap while cell1's
                    # elementwise chain runs
                    if t < 64:
                        g1_group(1 + t // 16, (t % 16) // 2, t % 2)
                    # ---- cell 2 for the previous step ----
                    if t >= 1:
                        enc_cell2(t - 1)
                h1T_prev = h1T
                enc_cell2(T - 1)

            # A_sb[te, b, :] = h2seq[b, te, :] (row layout for context matmul)
            for b in range(BS):
                for kc in range(2):
                    pA = attp.tile([T, 128], bf, tag="at")
                    nc.tensor.transpose(pA, h2seqB[:, kc, b, :], identb)
                    nc.vector.tensor_copy(A_sb[:, b, ts(kc, 128)], pA)

            # ================ P4 emitter ================
            wos = ctx.enter_context(tc.tile_pool(name="wos", bufs=4))
            junk = ctx.enter_context(tc.tile_pool(name="junk", bufs=2))
            p4sb = ctx.enter_context(tc.tile_pool(name="p4sb", bufs=2))

            def p4_begin(mi):
                t0 = 16 * mi
                tn = 16 if mi == 0 else DEC - 16
                s_all = p4sb.tile([128, NCH], fp, tag="sall", name=f"sall{mi}")
                return dict(mi=mi, t0=t0, tn=tn, R=tn * BS, s_all=s_all)

            def p4_chunk(st, c):
                t0, tn, R = st["t0"], st["tn"], st["R"]
                wot = wos.tile([128, 2, CSZ], bf, tag="wo")
                nc.sync.dma_start(wot, wo_d[:, :, ts(c, CSZ)])
                bot = wos.tile([1, CSZ], bf, tag="bo")
                nc.sync.dma_start(bot, bo_d[:, ts(c, CSZ)])
                psL = bigp.tile([128, CSZ], fp, tag="big")
                nc.tensor.matmul(
                    psL[:R], ones1x128b[:, :R], bot, start=True, stop=False
                )
                for kc in range(2):
                    nc.tensor.matmul(
                        psL[:R],
                        h2decT[:, kc, t0 : t0 + tn, :],
                        wot[:, kc, :],
                        start=False, stop=(kc == 1),
                    )
                jk = junk.tile([128, CSZ], bf, tag="jk")
                nc.scalar.activation(
                    jk[:R], psL[:R], AF.Exp,
                    accum_out=st["s_all"][:R, c : c + 1],
                )

            def p4_finish(st):
                mi, t0, tn, R = st["mi"], st["t0"], st["tn"], st["R"]
                ssum = p4sb.tile([128, 1], fp, tag="ssum")
                nc.vector.reduce_sum(ssum[:R], st["s_all"][:R], axis=AX.X)
                lse = p4sb.tile([128, 1], fp, tag="lse")
                nc.scalar.activation(lse[:R], ssum[:R], AF.Ln)
                at = attp.tile([128, 168], fp, tag="at")
                nc.tensor.matmul(
                    at[0:1, 33:34], lse[:R], ones128f[:R], start=True, stop=True
                )
                nc.vector.tensor_copy(ce_acc[:, 2 * mi : 2 * mi + 1], at[0:1, 33:34])
                # target-row dot: sum_rows h2dec . w_tgt
                prod = p4sb.tile([128, 2, 16, BS], bf, tag="prod")
                for kc in range(2):
                    nc.vector.tensor_mul(
                        prod[:, kc, 0:tn, :],
                        h2decT[:, kc, t0 : t0 + tn, :],
                        wtgt[:, kc, t0 : t0 + tn, :],
                    )
                for kc in range(2):
                    nc.tensor.matmul(
                        at[0:1, 40 : 40 + R], ones128b, prod[:, kc, 0:tn, :],
                        start=(kc == 0), stop=(kc == 1),
                    )
                ltsum = p4sb.tile([1, 1], fp, tag="lts")
                nc.vector.reduce_sum(ltsum, at[0:1, 40 : 40 + R], axis=AX.X)
                nc.vector.tensor_copy(ce_acc[:, 2 * mi + 1 : 2 * mi + 2], ltsum)

            # ================ P3: decoder ================
            # Software-pipelined: attention for step t-1 is emitted between
            # d2_t's h1-half (kc 0,1) and h2-half (kc 2,3), so d1_t and half
            # of d2_t overlap the previous step's attention chain.
            def attn_step(tq):
                """h2 <- softmax(h2seq . h2dec[tq]) . h2seq, returns h2aT."""
                at = attp.tile([128, 168], fp, tag="at", name=f"at{tq}")
                q = h2decT[:, :, tq, :]
                for b in range(BS):
                    for kc in range(2):
                        nc.tensor.matmul(
                            at[0:T, b : b + 1],
                            h2seqB[:, kc, b, :],
                            q[:, kc, b : b + 1],
                            start=(kc == 0), stop=(kc == 1),
                        )
                expT = smsb.tile([T, BS], bf, tag="exp")
                nc.scalar.activation(expT, at[0:T, 0:BS], AF.Exp)
                for b in range(BS):
                    for hc in range(2):
                        col = 8 + 8 * hc + b
                        nc.tensor.matmul(
                            at[:, col : col + 1],
                            A_sb[:, b, ts(hc, 128)],
                            expT[:, b : b + 1],
                            start=True, stop=True,
                        )
                nc.tensor.matmul(
                    at[0:1, 32:40], ones80, expT, start=True, stop=True
                )
                recip = smsb.tile([1, BS], fp, tag="rcp")
                nc.vector.reciprocal(recip, at[0:1, 32:40])
                nc.tensor.matmul(
                    at[:, 24:32], ones1x128f, recip, start=True, stop=True
                )
                # unnormalized context to SBUF in parallel with the
                # recip/broadcast chain; the final muls read the broadcast
                # straight from PSUM (single psum operand is fine)
                ctxs = smsb.tile([128, 16], bf, tag="ctxs")
                nc.vector.tensor_copy(ctxs, at[:, 8:24])
                h2a = smsb.tile([128, 2, BS], bf, tag="h2a")
                for hc in range(2):
                    nc.vector.tensor_mul(
                        h2a[:, hc, :], ctxs[:, ds(8 * hc, 8)], at[:, 24:32]
                    )
                return h2a

            p4st = [None, None]

            def dec_d2_mm(tm):
                """d2's matmul groups + the attention feeding its h2 input.
                Spine-ordered: kc01 (free inputs), attention for tm-1,
                then kc23 behind it."""
                ps2 = gpp.tile([128, 8, BS], fp, tag="g", name=f"d2ps{tm}")
                nc.tensor.matmul(
                    ps2, identb, capgT[:, :, ds(BS * tm, BS)],
                    start=True, stop=False,
                )
                for m in range(8):
                    for kc in range(2):
                        nc.tensor.matmul(
                            ps2[:, m, :], wd2[:, kc, m, :], h1T[:, kc, :],
                            start=False, stop=False,
                        )
                # attention for the previous step (produces this step's h2)
                h2rhs = (
                    h2seqB[:, :, :, T - 1] if tm == 0 else attn_step(tm - 1)
                )
                for m in range(8):
                    for kc in range(2, 4):
                        nc.tensor.matmul(
                            ps2[:, m, :], wd2[:, kc, m, :], h2rhs[:, kc - 2, :],
                            start=False, stop=(m == 7 and kc == 3),
                        )
                return ps2

            def dec_d2_post(tm):
                # interleave first CE tile (rows of steps 0..15) two vocab
                # chunks per step once its h2dec rows are complete
                if tm == 16:
                    p4st[0] = p4_begin(0)
                if 16 <= tm <= 23:
                    p4_chunk(p4st[0], 2 * (tm - 16))
                    p4_chunk(p4st[0], 2 * (tm - 16) + 1)
                elif tm == 24:
                    p4_finish(p4st[0])

            for t in range(DEC):
                # all matmul groups first (PE FIFO carries the spine), all
                # elementwise chains last: d2's chain (on the spine), then
                # d1's (a full spine period of slack).
                ps = gpp.tile([128, 8, BS], fp, tag="g")
                nc.tensor.matmul(ps, identb, bd1bc, start=True, stop=False)
                for m in range(8):
                    for kc in range(2):
                        nc.tensor.matmul(
                            ps[:, m, :], wd1[:, kc, m, :], h1T[:, kc, :],
                            start=False, stop=(m == 7 and kc == 1),
                        )
                if t >= 1:
                    ps2 = dec_d2_mm(t - 1)
                    c2 = lstm_elem_T(ps2, c2, h2decT[:, :, t - 1, :], "2")
                    dec_d2_post(t - 1)
                h1T_new = hpool.tile([128, 2, BS], bf, tag="h1")
                c1 = lstm_elem_T(ps, c1, h1T_new, "1")
                h1T = h1T_new
            ps2 = dec_d2_mm(DEC - 1)
            c2 = lstm_elem_T(ps2, c2, h2decT[:, :, DEC - 1, :], "2")
            # second CE tile right after the last d2 (its attention is
            # never consumed, so it is skipped entirely)
            p4st[1] = p4_begin(1)
            for c in range(NCH):
                p4_chunk(p4st[1], c)
            p4_finish(p4st[1])

            # final: partial = (lse0 - dot0 + lse1 - dot1) / B^2
            d0 = smsb.tile([1, 1], fp, tag="d0")
            nc.vector.tensor_sub(d0, ce_acc[:, 0:1], ce_acc[:, 1:2])
            d1_ = smsb.tile([1, 1], fp, tag="d1")
            nc.vector.tensor_sub(d1_, ce_acc[:, 2:3], ce_acc[:, 3:4])
            tot = smsb.tile([1, 1], fp, tag="tot")
            nc.vector.tensor_add(tot, d0, d1_)
            outsb = smsb.tile([1, 1], fp, tag="osb")
            nc.scalar.mul(outsb, tot, 1.0 / (B * B))
            nc.sync.dma_start(out_d[:, :], outsb)

    nc.compile()
    return nc


def _shard_inputs(inputs):
    """Host-side relayout + shard. Returns (list of 8 in_maps, host_bias)."""
    import ml_dtypes

    f32 = np.float32
    bft = ml_dtypes.bfloat16
    feat = np.asarray(inputs["feat"], f32)
    caption = np.asarray(inputs["caption"], f32)
    oh = np.asarray(inputs["caption_one_hot"], f32)

    def w(name):
        return np.asarray(inputs[name], f32)

    # i/f/o gate chunks (first 6 in PERM order) pre-scaled by 0.5 so the
    # kernel computes sigmoid as 0.5*tanh(scaled_gates)+0.5 with a single
    # full-width Tanh
    scale_m = np.array([0.5] * 6 + [1.0] * 2, f32)

    def wtile(Wt):  # Wt [K, 1024] -> [128, K//128, 8, 128] bf16, m permuted
        Kc = Wt.shape[0] // 128
        a = Wt.reshape(Kc, 128, 8, 128)[:, :, PERM, :]
        a = a * scale_m[None, None, :, None]
        return np.ascontiguousarray(a.transpose(1, 0, 2, 3).astype(bft))

    def bcol(bv):  # [1024] -> [128, 8] fp32, m permuted + scaled
        a = bv.reshape(8, 128)[PERM].T * scale_m[None, :]
        return np.ascontiguousarray(a.astype(f32))

    def bbc(bv):  # [1024] -> [128, 8, BS] broadcast over batch, bf16
        return np.ascontiguousarray(
            np.repeat(bcol(bv)[:, :, None], BS, axis=2).astype(bft)
        )

    out_W = w("out_W")
    shared = dict(
        w1t=wtile(w("e1_Wih").T),
        w1hht=wtile(w("e1_Whh").T),
        w2t=wtile(np.concatenate([w("e2_Wih")[:, H:], w("e2_Whh")], 1).T),
        wd1t=wtile(w("d1_Whh").T),
        wd2t=wtile(np.concatenate([w("d2_Wih")[:, H:], w("d2_Whh")], 1).T),
        wd2lt=wtile(w("d2_Wih")[:, :H].T),
        b1col=bcol(w("e1_b")),
        b2bc=bbc(w("e2_b")),
        bd1bc=bbc(w("d1_b")),
        bd2col=bcol(w("d2_b")),
        wot=np.ascontiguousarray(
            out_W.T.reshape(2, 128, V).transpose(1, 0, 2).astype(bft)
        ),
        bot=np.ascontiguousarray(w("out_b").reshape(1, V).astype(bft)),
    )

    tgt = np.argmax(oh, axis=2)[:, 1:]  # [B, DEC]
    host_bias = f32(w("out_b")[tgt].sum()) / f32(B * B)

    in_maps = []
    for c in range(NCORES):
        b0 = c * BS
        featT = np.ascontiguousarray(
            feat[b0 : b0 + BS]
            .transpose(2, 1, 0).reshape(KF, 128, TB).transpose(1, 0, 2)
            .astype(bft)
        )
        capT = np.ascontiguousarray(
            caption[b0 : b0 + BS, :DEC]
            .transpose(2, 1, 0).reshape(2, 128, ROWS).transpose(1, 0, 2)
            .astype(bft)
        )
        wt = out_W[tgt[b0 : b0 + BS]]  # [BS, DEC, H]
        wtgt = np.ascontiguousarray(
            wt.transpose(2, 1, 0).reshape(2, 128, DEC, BS)
            .transpose(1, 0, 2, 3).astype(bft)
        )
        m = dict(shared)
        m.update(featT=featT, capT=capT, wtgt=wtgt)
        in_maps.append(m)
    return in_maps, host_bias


def kernel(**inputs):
    from concourse.bass_utils import run_bass_kernel_spmd

    if "nc" not in _cache:
        _cache["nc"] = _build_program()
    nc = _cache["nc"]
    in_maps, host_bias = _shard_inputs(inputs)
    res = run_bass_kernel_spmd(nc, in_maps, core_ids=list(range(NCORES)))
    total = np.float32(0.0)
    for r in res.results:
        total += np.float32(r["partial"][0, 0])
    total -= host_bias
    return np.asarray(total, np.float32)
